# revision 18
# baseline (speedup 1.0000x reference)
"""BERT-base encoder layer on 8 Trainium2 NeuronCores (Bass/Tile).

Sharding: data-parallel over batch. Full inputs [32, 512, 768] split into 8
shards of 4 batches (2048 tokens); every core runs the same NEFF on its shard
(SPMD, no collectives); host concatenates the outputs.

Attention is computed k-major: scores are built transposed (ST[k, q] = K·Q^T)
so that softmax probabilities come out already in the layout the P·V matmul
needs — no PE transpose of P, and the key mask becomes a per-partition bias
on the exp activation (free) instead of rank-1 matmuls. The softmax
denominator comes from a ones-column appended to V (row 64 of the AV PSUM);
normalization is a rank-1 broadcast matmul + one vector multiply.

QKV/V/AV/O-projection GEMMs run in fp8(e4m3) DoubleRow mode (2 contraction
rows per PE pass); Wso/Wi/Wout GEMMs stay bf16 for accuracy. PSUM accumulation
is fp32 everywhere; layernorm statistics fp32.
"""

import os
import numpy as np
import ml_dtypes

B, S, E, H, DK, FF = 32, 512, 768, 12, 64, 3072
NCORES = 8
BL = B // NCORES          # batches per core = 4
T = BL * S                # tokens per core = 2048
EPS = 1e-12
MASK_NEG = -87.0          # exp(-87) == 0 in fp8/bf16
KT_E = E // 128           # 6 feature blocks
KP = KT_E // 2            # 3 fp8 contraction pairs
NT = S // 128             # 4 token tiles
FT = FF // 128            # 24
HP = H // 2               # 6 head pairs

_CACHE = {}
_DEBUG_XA = False


def _bf(a):
    return np.ascontiguousarray(np.asarray(a, np.float32).astype(ml_dtypes.bfloat16))


def _f8(a):
    a = np.clip(np.asarray(a, np.float32), -240.0, 240.0)
    return np.ascontiguousarray(a.astype(ml_dtypes.float8_e4m3))


def _pack_blk(w):
    """Moving-operand block format: [K, N] -> [128, (K//256)*2*N] fp8; slice p
    gives [128, 2, N] with element [r, i, m] = w[256p + 128i + r, m]."""
    K, N = w.shape
    p = K // 256
    arr = np.asarray(w, np.float32).reshape(p, 2, 128, N).transpose(2, 0, 1, 3)
    return _f8(arr.reshape(128, p * 2 * N))


def _pack_sw(w):
    """Stationary sw-interleave format for dual-fp8 LDWEIGHTS: [K, N] ->
    [128, (K//256)*(N//128)*256]; block (p, nb) holds column m of k-pair i at
    position 2*(127-m)+i."""
    K, N = w.shape
    P, NB = K // 256, N // 128
    a = np.asarray(w, np.float32).reshape(P, 2, 128, NB, 128)
    a = a.transpose(2, 0, 3, 4, 1)[:, :, :, ::-1, :]     # [r, p, nb, m_rev, i]
    return _f8(a.reshape(128, P * NB * 256))


def _build(flags):
    import concourse.bass as bass
    import concourse.bacc as bacc
    import concourse.mybir as mybir
    import concourse.tile as tile
    from contextlib import ExitStack

    (use_bq, use_bk, use_bv, use_bo, use_bso, use_bi, use_bout,
     use_g1, use_b1, use_g2, use_b2) = flags

    AF = mybir.ActivationFunctionType
    OP = mybir.AluOpType
    AX = mybir.AxisListType
    BF16 = mybir.dt.bfloat16
    F32 = mybir.dt.float32
    F8 = mybir.dt.float8e4
    DRS = mybir.MatmulPerfMode.DoubleRowSwInterleave
    ACT_E = mybir.EngineType.Activation

    nc = bacc.Bacc("TRN2", target_bir_lowering=False)

    # x^T per batch in three layouts (transposed/packed on host):
    # bf16 feature-major (residual), fp8 moving blocks (Q/K), fp8 interleaved
    # stationary (V)
    d_xtb = nc.dram_tensor("xtb", (BL * 128, KT_E * 512), BF16,
                           kind="ExternalInput")
    d_xq8 = nc.dram_tensor("xq8", (BL * 128, KP * 2 * 512), F8,
                           kind="ExternalInput")
    d_xv8 = nc.dram_tensor("xv8", (BL * 128, KP * NT * 256), F8,
                           kind="ExternalInput")
    d_wq8 = nc.dram_tensor("wq8", (128, KP * KT_E * 256), F8, kind="ExternalInput")
    d_wk8 = nc.dram_tensor("wk8", (128, KP * KT_E * 256), F8, kind="ExternalInput")
    d_wv8 = nc.dram_tensor("wv8", (128, KP * 2 * E), F8, kind="ExternalInput")
    d_wo8 = nc.dram_tensor("wo8", (128, KP * KT_E * 256), F8, kind="ExternalInput")
    d_wso = nc.dram_tensor("wso", (E, E), BF16, kind="ExternalInput")
    d_wi = nc.dram_tensor("wi", (E, FF), BF16, kind="ExternalInput")
    d_wout = nc.dram_tensor("wout", (FF, E), BF16, kind="ExternalInput")
    d_mcol = nc.dram_tensor("mcol", (128, BL * NT), F32, kind="ExternalInput")
    d_ones65 = nc.dram_tensor("ones65", (65, 128), BF16, kind="ExternalInput")
    d_id = nc.dram_tensor("ident", (128, 128), BF16, kind="ExternalInput")
    d_onesr = nc.dram_tensor("onesrow", (1, 512), BF16, kind="ExternalInput")
    # bias rows: 0=bq/8, 1=bk, 2=bv, 3=bo, 4=bso, 5=bout, 6=bi (full FF width)
    d_brow = nc.dram_tensor("brow", (7, FF), BF16, kind="ExternalInput")
    d_bic = nc.dram_tensor("bicol", (128, FT), F32, kind="ExternalInput")
    # gamma1 | gamma2 | beta1 | beta2, each [128, 768] partition-broadcast
    d_gb = nc.dram_tensor("gb", (128, 4 * E), F32, kind="ExternalInput")
    d_out = nc.dram_tensor("out", (T, E), F32, kind="ExternalOutput")
    d_dbg = nc.dram_tensor("dbg", (BL * 128, KT_E * 512), F32,
                           kind="ExternalOutput") if _DEBUG_XA else None

    need_gb = use_g1 or use_b1 or use_g2 or use_b2
    need_brow = use_bq or use_bk or use_bv or use_bo or use_bso or use_bout

    with ExitStack() as ctx:
        tc = ctx.enter_context(tile.TileContext(nc))

        p_mm = ctx.enter_context(tc.tile_pool(name="p_mm", bufs=8, space="PSUM"))

        c_pool = ctx.enter_context(tc.tile_pool(name="consts", bufs=1))
        wa_pool = ctx.enter_context(tc.tile_pool(name="wa", bufs=1))
        wso_pool = ctx.enter_context(tc.tile_pool(name="wso", bufs=KT_E))
        wi_pool = ctx.enter_context(tc.tile_pool(name="wi", bufs=KT_E))
        wout_pool = ctx.enter_context(tc.tile_pool(name="wout", bufs=FT))
        xa_pool = ctx.enter_context(tc.tile_pool(name="xa", bufs=BL))

        ident = c_pool.tile_from(d_id[:, :], name="ident")
        mcol = c_pool.tile_from(d_mcol[:, :], name="mcol")
        ones65 = c_pool.tile_from(d_ones65[:, :], name="ones65")
        onesr = c_pool.tile_from(d_onesr[:, :], name="onesr") \
            if (use_bv or use_bso or use_bout or use_bq or use_bk or use_bo) else None
        brow = c_pool.tile_from(d_brow[:, :], name="brow") if need_brow else None
        gb = c_pool.tile_from(d_gb[:, :], name="gb") if need_gb else None
        bic = c_pool.tile_from(d_bic[:, :], name="bic") if use_bi else None

        # phase-A weights (fp8, small): default (SP) DMA queue
        wq8 = wa_pool.tile_from(d_wq8[:, :], name="wq8t")
        wk8 = wa_pool.tile_from(d_wk8[:, :], name="wk8t")
        wv8 = wa_pool.tile_from(d_wv8[:, :], name="wv8t")
        wo8 = wa_pool.tile_from(d_wo8[:, :], name="wo8t")

        # stationary (sw-interleaved) weights: slice (p, et) -> [128, 256]
        WQ8 = wq8.rearrange("r (p e c) -> r p e c", p=KP, e=KT_E)
        WK8 = wk8.rearrange("r (p e c) -> r p e c", p=KP, e=KT_E)
        WO8 = wo8.rearrange("r (p e c) -> r p e c", p=KP, e=KT_E)
        # moving (block) V weights: slice p -> [128, 2, E]
        WV8 = wv8.rearrange("r (p i m) -> r p i m", p=KP, i=2)

        # phase-B weights (bf16, 10.6MB): stream on the Activation DMA queue
        # during phase A
        WSO = [wso_pool.tile_from(d_wso[k * 128:(k + 1) * 128, :], name="wsot",
                                  forced_dma_engine=ACT_E) for k in range(KT_E)]
        WI = [wi_pool.tile_from(d_wi[k * 128:(k + 1) * 128, :], name="wit",
                                forced_dma_engine=ACT_E) for k in range(KT_E)]
        WOUT = [wout_pool.tile_from(d_wout[f * 128:(f + 1) * 128, :], name="woutt",
                                    forced_dma_engine=ACT_E) for f in range(FT)]

        XA = [None] * BL   # [128, KT_E*512] bf16, feature-major x+att@Wo

        # ================= superphase A: QKV, attention, O-proj ==============
        with ExitStack() as sa:
            xtb_pool = sa.enter_context(tc.tile_pool(name="xtb", bufs=2))
            xq8_pool = sa.enter_context(tc.tile_pool(name="xq8", bufs=1))
            xv8_pool = sa.enter_context(tc.tile_pool(name="xv8", bufs=1))
            qt_pool = sa.enter_context(tc.tile_pool(name="qt", bufs=1))
            kt_pool = sa.enter_context(tc.tile_pool(name="kt", bufs=1))
            va_pool = sa.enter_context(tc.tile_pool(name="va", bufs=1))
            se_pool = sa.enter_context(tc.tile_pool(name="se", bufs=24))
            at_pool = sa.enter_context(tc.tile_pool(name="at", bufs=4))
            sg_pool = sa.enter_context(tc.tile_pool(name="sg", bufs=4))
            rs_pool = sa.enter_context(tc.tile_pool(name="rs", bufs=4))
            rb_pool = sa.enter_context(tc.tile_pool(name="rb", bufs=4))

            # persistent V tiles (2 sets x 2 token-pair tiles), ones column
            # preset once; V-projection overwrites only the V part each batch
            VAUG = [[va_pool.tile([128, H * 256], F8, name="vaug", tag="va",
                                  bufs=4) for _ in range(2)] for _ in range(2)]
            for st in range(2):
                for pp in range(2):
                    v4 = VAUG[st][pp].rearrange("r (h c) -> r h c", h=H)
                    nc.gpsimd.memset(v4[:, :, 0:126], 0.0)
                    nc.gpsimd.memset(v4[:, :, 126:128], 1.0)

            ST = {}   # per-batch state

            def s1(b):
                """x^T loads (pre-transposed on host) + QKV projections."""
                st = {}
                xtb = xtb_pool.tile([128, KT_E * 512], BF16, name="xtb",
                                    tag="xtb")
                xq8 = xq8_pool.tile([128, KP * 2 * 512], F8, name="xq8",
                                    tag="xq8")
                xv8 = xv8_pool.tile([128, KP * NT * 256], F8, name="xv8",
                                    tag="xv8")
                nc.gpsimd.dma_start(xtb[:, :], d_xtb[b * 128:(b + 1) * 128, :])
                nc.gpsimd.dma_start(xq8[:, :], d_xq8[b * 128:(b + 1) * 128, :])
                nc.gpsimd.dma_start(xv8[:, :], d_xv8[b * 128:(b + 1) * 128, :])
                xq8_p = xq8.rearrange("r (p i t) -> r p i t", p=KP, i=2)
                xv8_p = xv8.rearrange("r (p t c) -> r p t c", p=KP, t=NT)

                # Q/K projections -> feature-major bf16 [128, HP*512]
                qtt = qt_pool.tile([128, HP * 512], BF16, name="qtt", tag="qt")
                ktt = kt_pool.tile([128, HP * 512], BF16, name="ktt", tag="kt")
                for W8, dst, ub, brx in ((WQ8, qtt, use_bq, 0),
                                         (WK8, ktt, use_bk, 1)):
                    for et in range(KT_E):
                        ps = p_mm.tile([128, 512], F32, name="qkps", tag="mm")
                        for p in range(KP):
                            nc.tensor.matmul(
                                ps[:, :], W8[:, p, et, :], xq8_p[:, p, :, :],
                                perf_mode=DRS,
                                start=(p == 0), stop=(p == KP - 1 and not ub))
                        if ub:
                            nc.tensor.matmul(
                                ps[:, :],
                                brow[brx:brx + 1, et * 128:(et + 1) * 128],
                                onesr[0:1, 0:S], start=False, stop=True)
                        nc.vector.tensor_copy(dst[:, et * 512:(et + 1) * 512],
                                              ps[:, :])
                st["qt"], st["kt"] = qtt, ktt

                # V projection (token-major, per-head columns reversed on the
                # host so the interleaved write is an ascending stride-2 copy)
                vset = VAUG[b % 2]
                for tt in range(NT):
                    for ec, n in ((0, 512), (512, 256)):
                        ps = (p_mm.tile([128, 512], F32, name="vps", tag="mm")
                              if n == 512 else
                              p_mm.tile([128, 256], F32, name="vps2", tag="mm"))
                        for p in range(KP):
                            nc.tensor.matmul(
                                ps[:, :n], xv8_p[:, p, tt, :],
                                WV8[:, p, :, ec:ec + n], perf_mode=DRS,
                                start=(p == 0), stop=(p == KP - 1 and not use_bv))
                        if use_bv:
                            nc.tensor.matmul(
                                ps[:, :n], onesr[0:1, 0:128],
                                brow[2:3, ec:ec + n], start=False, stop=True)
                        h0, nh = ec // 64, n // 64
                        v6 = vset[tt // 2].rearrange(
                            "r (h a c i) -> r h a c i", h=H, a=2, c=64)
                        nc.vector.tensor_copy(
                            v6[:, h0:h0 + nh, 1, :, tt % 2],
                            ps[:, :n].rearrange("r (h c) -> r h c", h=nh))
                st["xtb"], st["vset"] = xtb, vset
                ST[b] = st

            def sc_hp(b, hp):
                """scores for one head-pair (k-major, row-group dual-issue)
                + masked exp."""
                st = ST[b]
                qtt, ktt = st["qt"], st["kt"]
                stexp = st.setdefault("stexp", {})
                for kt in range(NT):
                    pss = []
                    for hh in range(2):
                        o = hh * 64
                        ps = p_mm.tile([128, 512], F32, name="scps", tag="mm")
                        nc.tensor.matmul(
                            ps[:, :],
                            ktt[o:o + 64,
                                hp * 512 + kt * 128:hp * 512 + (kt + 1) * 128],
                            qtt[o:o + 64, hp * 512:(hp + 1) * 512],
                            start=True, stop=True)
                        pss.append(ps)
                    for hh in range(2):
                        if (hp, hh, kt // 2) not in stexp:
                            stexp[(hp, hh, kt // 2)] = se_pool.tile(
                                [128, 2 * 512], F8, name="sexp", tag="se")
                        dst = stexp[(hp, hh, kt // 2)]
                        nc.scalar.activation(
                            dst[:, (kt % 2) * 512:(kt % 2 + 1) * 512],
                            pss[hh][:, :], AF.Exp,
                            bias=mcol[:, b * NT + kt:b * NT + kt + 1])

            def s2_hp(b, hp):
                """AV for one head-pair (fp8 DoubleRow, ones-column
                denominators), normalize, assemble ATT pairs."""
                st = ST[b]
                stexp, vset = st["stexp"], st["vset"]
                if "att8" not in st:
                    st["att8"] = [at_pool.tile([128, 2 * 512], F8,
                                               name="att8", tag="at")
                                  for _ in range(KP)]
                att8 = st["att8"]
                for hh in range(2):
                    h = 2 * hp + hh
                    av = p_mm.tile([128, 512], F32, name="avps", tag="mm")
                    for pp in range(2):
                        nc.tensor.matmul(
                            av[:, :], vset[pp][:, h * 256:(h + 1) * 256],
                            stexp[(hp, hh, pp)].rearrange(
                                "r (i t) -> r i t", i=2),
                            perf_mode=DRS, start=(pp == 0), stop=(pp == 1))
                    rs = rs_pool.tile([65, 512], BF16, name="rst", tag="rs")
                    with nc.allow_low_precision(
                            reason="bf16 1/s for broadcast (0.4% rel)"):
                        nc.vector.reciprocal(rs[64:65, :], av[64:65, :])
                    rbc = p_mm.tile([64, 512], F32, name="rbc", tag="mm")
                    nc.tensor.matmul(rbc[:, :], ones65[64:65, 0:64],
                                     rs[64:65, :], start=True, stop=True)
                    rbs = rb_pool.tile([64, 512], BF16, name="rbs", tag="rb")
                    nc.vector.tensor_copy(rbs[:, :], rbc[:, :])
                    kp, half = hp // 2, hp % 2
                    if hh == 0:
                        nc.vector.scalar_tensor_tensor(
                            att8[kp][0:64, half * 512:(half + 1) * 512],
                            av[0:64, :], 1.0, rbs[:, :],
                            op0=OP.mult, op1=OP.mult)
                    else:
                        stg = sg_pool.tile([64, 512], F8, name="stg", tag="sg")
                        nc.vector.scalar_tensor_tensor(
                            stg[:, :], av[0:64, :], 1.0, rbs[:, :],
                            op0=OP.mult, op1=OP.mult)
                        nc.sync.dma_start(
                            att8[kp][64:128, half * 512:(half + 1) * 512],
                            stg[:, :])

            def s2_o(b):
                """O-projection (fp8) + residual -> xa (feature-major bf16)."""
                st = ST[b]
                att8, xtb = st["att8"], st["xtb"]
                xa = xa_pool.tile([128, KT_E * 512], BF16, name="xat", tag="xa")
                for et in range(KT_E):
                    ps = p_mm.tile([128, 512], F32, name="ops", tag="mm")
                    for kp in range(KP):
                        nc.tensor.matmul(
                            ps[:, :], WO8[:, kp, et, :],
                            att8[kp].rearrange("r (i t) -> r i t", i=2),
                            perf_mode=DRS,
                            start=(kp == 0), stop=(kp == KP - 1 and not use_bo))
                    if use_bo:
                        nc.tensor.matmul(
                            ps[:, :], brow[3:4, et * 128:(et + 1) * 128],
                            onesr[0:1, 0:S], start=False, stop=True)
                    nc.vector.scalar_tensor_tensor(
                        xa[:, et * 512:(et + 1) * 512], ps[:, :], 1.0,
                        xtb[:, et * 512:(et + 1) * 512],
                        op0=OP.mult, op1=OP.add)
                XA[b] = xa
                if _DEBUG_XA:
                    nc.gpsimd.dma_start(d_dbg[b * 128:(b + 1) * 128, :],
                                        xa[:, :])
                del ST[b]

            # software-pipelined emission: the exp burst of batch b drains on
            # the ACT engine while the PE runs batch b+1's projections, and
            # batch b+1's scores interleave with batch b's AV at head-pair
            # grain so neither engine stalls the other
            s1(0)
            for hp in range(HP):
                sc_hp(0, hp)
            s1(1)
            for bn in (1, 2, 3):
                for hp in range(HP):
                    s2_hp(bn - 1, hp)
                    sc_hp(bn, hp)
                s2_o(bn - 1)
                if bn < 3:
                    s1(bn + 1)
            for hp in range(HP):
                s2_hp(3, hp)
            s2_o(3)

        # ============ superphase B: SelfOutput LN, FFN, LN ===================
        with ExitStack() as sb:
            h_pool = sb.enter_context(tc.tile_pool(name="h", bufs=NT + 1))
            ht_pool = sb.enter_context(tc.tile_pool(name="ht", bufs=2))
            fft_pool = sb.enter_context(tc.tile_pool(name="fft", bufs=FT + 2))
            sq_pool = sb.enter_context(tc.tile_pool(name="sq", bufs=2))
            rs_pool = sb.enter_context(tc.tile_pool(name="rsd", bufs=3))
            out_pool = sb.enter_context(tc.tile_pool(name="outp", bufs=2))
            t_pool = sb.enter_context(tc.tile_pool(name="sb_s", bufs=12))

            def layernorm(chunks, h_dst, gcol, use_g, use_bb, resid=None):
                """chunks: [(psum_ap, col0, n)]; h_dst: [128, E] out.
                Drains PSUM chunks to SBUF immediately (rtile) so the banks
                free early; stats then run from SBUF."""
                rtile = rs_pool.tile([128, E], F32, name="rt", tag="rsd")
                if resid is not None:
                    for (ps, c0, n), rext in zip(chunks, resid):
                        nc.vector.scalar_tensor_tensor(
                            rtile[:, c0:c0 + n], ps, 1.0, rext,
                            op0=OP.mult, op1=OP.add)
                else:
                    for (ps, c0, n) in chunks:
                        nc.vector.tensor_copy(rtile[:, c0:c0 + n], ps)
                srcs = [(rtile[:, c0:c0 + n], c0, n) for (_, c0, n) in chunks]
                s1t = t_pool.tile([128, 1], F32, name="s1", tag="s1")
                s1b = t_pool.tile([128, 1], F32, name="s1b", tag="s1b")
                nc.vector.reduce_sum(s1t[:, :], srcs[0][0], axis=AX.X)
                nc.vector.reduce_sum(s1b[:, :], srcs[1][0], axis=AX.X)
                mu_n = t_pool.tile([128, 1], F32, name="mun", tag="mun")
                tmp = t_pool.tile([128, 1], F32, name="tmps", tag="tmps")
                nc.vector.scalar_tensor_tensor(
                    tmp[:, :], s1t[:, :], 1.0, s1b[:, :], op0=OP.mult, op1=OP.add)
                nc.vector.tensor_scalar_mul(mu_n[:, :], tmp[:, :], -1.0 / E)
                ss = t_pool.tile([128, 1], F32, name="ssa", tag="ssa", bufs=34)
                ssb = t_pool.tile([128, 1], F32, name="ssb", tag="ssb", bufs=34)
                for (src, c0, n), acc in zip(srcs, (ss, ssb)):
                    sq = sq_pool.tile([128, 512], BF16, name="sqt", tag="sq")
                    nc.scalar.activation(sq[:, :n], src, AF.Square,
                                         accum_out=acc[:, :])
                musq = t_pool.tile([128, 1], F32, name="musq", tag="musq")
                nc.vector.scalar_tensor_tensor(
                    musq[:, :], mu_n[:, :], 1.0, mu_n[:, :],
                    op0=OP.mult, op1=OP.mult)
                veps = t_pool.tile([128, 1], F32, name="veps", tag="veps")
                nc.vector.scalar_tensor_tensor(
                    veps[:, :], ss[:, :], 1.0, ssb[:, :],
                    op0=OP.mult, op1=OP.add)
                veps2 = t_pool.tile([128, 1], F32, name="veps2", tag="veps2")
                nc.vector.tensor_scalar(
                    veps2[:, :], veps[:, :], 1.0 / E, EPS,
                    op0=OP.mult, op1=OP.add)
                veps3 = t_pool.tile([128, 1], F32, name="veps3", tag="veps3")
                nc.vector.scalar_tensor_tensor(
                    veps3[:, :], musq[:, :], -1.0, veps2[:, :],
                    op0=OP.mult, op1=OP.add)
                sd = t_pool.tile([128, 1], F32, name="sd", tag="sd")
                nc.scalar.sqrt(sd[:, :], veps3[:, :])
                rstd = t_pool.tile([128, 1], F32, name="rstd", tag="rstd")
                nc.vector.reciprocal(rstd[:, :], sd[:, :])
                for (src, c0, n) in srcs:
                    nc.vector.tensor_scalar(
                        h_dst[:, c0:c0 + n], src, mu_n[:, :], rstd[:, :],
                        op0=OP.add, op1=OP.mult)
                if use_g:
                    nc.vector.scalar_tensor_tensor(
                        h_dst[:, :], h_dst[:, :], 1.0,
                        gb[:, gcol * E:(gcol + 1) * E], op0=OP.mult, op1=OP.mult)
                if use_bb:
                    nc.vector.scalar_tensor_tensor(
                        h_dst[:, :], h_dst[:, :], 1.0,
                        gb[:, (gcol + 2) * E:(gcol + 3) * E],
                        op0=OP.mult, op1=OP.add)

            for b in range(BL):
                t0 = b * S
                xa = XA[b].rearrange("r (e t) -> r e t", e=KT_E)

                # ---- SelfOutput GEMM + LN1 -> h (token-major), hT ----
                hh_t = [None] * NT
                hT = ht_pool.tile([128, KT_E * S], BF16, name="htt", tag="ht")

                def emit_htrans(tt):
                    tps = [p_mm.tile([128, 512], BF16, name="htp", tag="mm")
                           for _ in range(2)]
                    for et in range(KT_E):
                        sl = tps[et // 4][:, (et % 4) * 128:(et % 4 + 1) * 128]
                        nc.tensor.transpose(
                            sl, hh_t[tt][:, et * 128:(et + 1) * 128],
                            ident[:, :])
                    for et in range(KT_E):
                        sl = tps[et // 4][:, (et % 4) * 128:(et % 4 + 1) * 128]
                        nc.vector.tensor_copy(
                            hT[:, et * S + tt * 128:et * S + (tt + 1) * 128], sl)

                for tt in range(NT):
                    ch = []
                    for ec, n in ((0, 512), (512, 256)):
                        ps = (p_mm.tile([128, 512], F32, name="sops", tag="mm")
                              if n == 512 else
                              p_mm.tile([128, 256], F32, name="sops2", tag="mm"))
                        for k in range(KT_E):
                            nc.tensor.matmul(
                                ps[:, :n], xa[:, k, tt * 128:(tt + 1) * 128],
                                WSO[k][:, ec:ec + n],
                                start=(k == 0),
                                stop=(k == KT_E - 1 and not use_bso))
                        if use_bso:
                            nc.tensor.matmul(
                                ps[:, :n], onesr[0:1, 0:128],
                                brow[4:5, ec:ec + n], start=False, stop=True)
                        ch.append((ps[:, :n], ec, n))
                    hh_t[tt] = h_pool.tile([128, E], BF16, name="hht", tag="h")
                    layernorm(ch, hh_t[tt], 0, use_g1, use_b1)
                    if tt > 0:
                        emit_htrans(tt - 1)
                emit_htrans(NT - 1)

                # ---- FFN + LN2 ----
                ffT = [None] * FT
                for ft in range(FT):
                    ps = p_mm.tile([128, 512], F32, name="fips", tag="mm")
                    for k in range(KT_E):
                        nc.tensor.matmul(
                            ps[:, :], WI[k][:, ft * 128:(ft + 1) * 128],
                            hT[:, k * S:k * S + 512],
                            start=(k == 0), stop=(k == KT_E - 1))
                    ffT[ft] = fft_pool.tile([128, 512], BF16, name="fftt",
                                            tag="fft")
                    if use_bi:
                        nc.scalar.activation(ffT[ft][:, :], ps[:, :], AF.Gelu,
                                             bias=bic[:, ft:ft + 1])
                    else:
                        nc.scalar.activation(ffT[ft][:, :], ps[:, :], AF.Gelu)
                for tt in range(NT):
                    ch = []
                    for ec, n in ((0, 512), (512, 256)):
                        ps = (p_mm.tile([128, 512], F32, name="wops", tag="mm")
                              if n == 512 else
                              p_mm.tile([128, 256], F32, name="wops2", tag="mm"))
                        for f in range(FT):
                            nc.tensor.matmul(
                                ps[:, :n],
                                ffT[f][:, tt * 128:(tt + 1) * 128],
                                WOUT[f][:, ec:ec + n],
                                start=(f == 0),
                                stop=(f == FT - 1 and not use_bout))
                        if use_bout:
                            nc.tensor.matmul(
                                ps[:, :n], onesr[0:1, 0:128],
                                brow[5:6, ec:ec + n], start=False, stop=True)
                        ch.append((ps[:, :n], ec, n))
                    otile = out_pool.tile([128, E], F32, name="ot", tag="outp")
                    resid = [hh_t[tt][:, ec:ec + n] for (_, ec, n) in ch]
                    layernorm(ch, otile, 1, use_g2, use_b2, resid=resid)
                    nc.gpsimd.dma_start(
                        d_out[t0 + tt * 128:t0 + (tt + 1) * 128, :],
                        otile[:, :])
    nc.compile()
    return nc


def _get_program(flags):
    key = ("prog", flags)
    if key not in _CACHE:
        _CACHE[key] = _build(flags)
    return _CACHE[key]


def kernel(x, mask, Wq, bq, Wk, bk, Wv, bv, Wo, bo,
           Wso, bso, gso, beso, Wi, bi, Wout, bout, gout, beout):
    from concourse.bass_utils import run_bass_kernel_spmd

    x = np.asarray(x, np.float32)
    mask = np.asarray(mask)
    sc = 1.0 / float(np.sqrt(np.float32(DK)))

    z = lambda a: not np.any(np.asarray(a))
    one = lambda a: bool(np.all(np.asarray(a) == 1.0))
    flags = (not z(bq), not z(bk), not z(bv), not z(bo), not z(bso),
             not z(bi), not z(bout),
             not one(gso), not z(beso), not one(gout), not z(beout))
    nc = _get_program(flags)

    wq8 = _pack_sw(np.asarray(Wq, np.float32) * sc)
    wk8, wo8 = _pack_sw(Wk), _pack_sw(Wo)
    # reverse V's 64 columns within each head so the on-device interleaved
    # write of the AV stationary is an ascending stride-2 copy
    wv_re = np.asarray(Wv, np.float32).reshape(E, H, DK)[:, :, ::-1].reshape(E, E)
    wv8 = _pack_blk(wv_re)
    wso_b, wi_b, wout_b = _bf(Wso), _bf(Wi), _bf(Wout)
    identb = _bf(np.eye(128))
    onesr = _bf(np.ones((1, 512)))

    brow = np.zeros((7, FF), np.float32)
    brow[0, :E] = np.asarray(bq, np.float32) * sc
    for i, v in enumerate((bk, bv, bo, bso, bout)):
        brow[i + 1, :E] = v
    brow[6, :] = bi
    brow = _bf(brow)
    bicol = np.asarray(bi, np.float32).reshape(FT, 128).T.copy()
    gbt = np.zeros((128, 4 * E), np.float32)
    for i, g in enumerate((gso, gout, beso, beout)):
        gbt[:, i * E:(i + 1) * E] = np.broadcast_to(
            np.asarray(g, np.float32).reshape(1, E), (128, E))

    in_maps = []
    for c in range(NCORES):
        xs = x[c * BL:(c + 1) * BL]            # [BL, S, E]
        ms = np.asarray(mask[c * BL:(c + 1) * BL]).reshape(BL, S)
        # mcol[r, b*NT + kt] = bias for key token kt*128 + r of batch b
        mb = np.where(ms == 0, np.float32(MASK_NEG), np.float32(0.0))
        mcol = np.ascontiguousarray(
            mb.reshape(BL, NT, 128).transpose(2, 0, 1).reshape(128, BL * NT))
        # x^T per batch in the three on-device layouts
        xtb = np.empty((BL * 128, KT_E * 512), np.float32)
        xq8 = np.empty((BL * 128, KP * 2 * 512), ml_dtypes.float8_e4m3)
        xv8 = np.empty((BL * 128, KP * NT * 256), ml_dtypes.float8_e4m3)
        for b in range(BL):
            xt = np.ascontiguousarray(xs[b].T)               # [E, S]
            xtb[b * 128:(b + 1) * 128] = xt.reshape(
                KT_E, 128, S).transpose(1, 0, 2).reshape(128, KT_E * S)
            xq8[b * 128:(b + 1) * 128] = _pack_blk(xt)
            xv8[b * 128:(b + 1) * 128] = _pack_sw(xt)
        in_maps.append({
            "ones65": _bf(np.ones((65, 128))),
            "xtb": _bf(xtb), "xq8": xq8, "xv8": xv8,
            "wq8": wq8, "wk8": wk8, "wv8": wv8, "wo8": wo8,
            "wso": wso_b, "wi": wi_b, "wout": wout_b, "mcol": mcol,
            "ident": identb,
            "onesrow": onesr, "brow": brow, "bicol": bicol, "gb": gbt,
        })

    trace = os.environ.get("KERNEL_TRACE", "0") == "1"
    res = run_bass_kernel_spmd(nc, in_maps, core_ids=list(range(NCORES)),
                               trace=trace)
    if trace and res.exec_time_ns is not None:
        print(f"HW exec time: {res.exec_time_ns} ns")
        if res.instructions_and_trace is not None:
            print(f"trace: {res.instructions_and_trace[1]}")
    out = np.concatenate([r["out"].reshape(BL, S, E) for r in res.results],
                         axis=0)
    return np.ascontiguousarray(out.astype(np.float32))


# revision 21
# speedup vs baseline: 1.0118x; 1.0118x over previous
"""BERT-base encoder layer on 8 Trainium2 NeuronCores (Bass/Tile).

Sharding: data-parallel over batch. Full inputs [32, 512, 768] split into 8
shards of 4 batches (2048 tokens); every core runs the same NEFF on its shard
(SPMD, no collectives); host concatenates the outputs.

Attention is computed k-major: scores are built transposed (ST[k, q] = K·Q^T)
so that softmax probabilities come out already in the layout the P·V matmul
needs — no PE transpose of P, and the key mask becomes a per-partition bias
on the exp activation (free) instead of rank-1 matmuls. The softmax
denominator comes from a ones-column appended to V (row 64 of the AV PSUM);
normalization is a rank-1 broadcast matmul + one vector multiply.

QKV/V/AV/O-projection GEMMs run in fp8(e4m3) DoubleRow mode (2 contraction
rows per PE pass); Wso/Wi/Wout GEMMs stay bf16 for accuracy. PSUM accumulation
is fp32 everywhere; layernorm statistics fp32.
"""

import os
import numpy as np
import ml_dtypes

B, S, E, H, DK, FF = 32, 512, 768, 12, 64, 3072
NCORES = 8
BL = B // NCORES          # batches per core = 4
T = BL * S                # tokens per core = 2048
EPS = 1e-12
MASK_NEG = -87.0          # exp(-87) == 0 in fp8/bf16
KT_E = E // 128           # 6 feature blocks
KP = KT_E // 2            # 3 fp8 contraction pairs
NT = S // 128             # 4 token tiles
FT = FF // 128            # 24
HP = H // 2               # 6 head pairs

_CACHE = {}


def _bf(a):
    return np.ascontiguousarray(np.asarray(a, np.float32).astype(ml_dtypes.bfloat16))


def _f8(a):
    a = np.clip(np.asarray(a, np.float32), -240.0, 240.0)
    return np.ascontiguousarray(a.astype(ml_dtypes.float8_e4m3))


def _pack_blk(w):
    """Moving-operand block format: [K, N] -> [128, (K//256)*2*N] fp8; slice p
    gives [128, 2, N] with element [r, i, m] = w[256p + 128i + r, m]."""
    K, N = w.shape
    p = K // 256
    arr = np.asarray(w, np.float32).reshape(p, 2, 128, N).transpose(2, 0, 1, 3)
    return _f8(arr.reshape(128, p * 2 * N))


def _pack_sw(w):
    """Stationary sw-interleave format for dual-fp8 LDWEIGHTS: [K, N] ->
    [128, (K//256)*(N//128)*256]; block (p, nb) holds column m of k-pair i at
    position 2*(127-m)+i."""
    K, N = w.shape
    P, NB = K // 256, N // 128
    a = np.asarray(w, np.float32).reshape(P, 2, 128, NB, 128)
    a = a.transpose(2, 0, 3, 4, 1)[:, :, :, ::-1, :]     # [r, p, nb, m_rev, i]
    return _f8(a.reshape(128, P * NB * 256))


def _build(flags):
    import concourse.bass as bass
    import concourse.bacc as bacc
    import concourse.mybir as mybir
    import concourse.tile as tile
    from contextlib import ExitStack

    (use_bq, use_bk, use_bv, use_bo, use_bso, use_bi, use_bout,
     use_g1, use_b1, use_g2, use_b2) = flags

    AF = mybir.ActivationFunctionType
    OP = mybir.AluOpType
    AX = mybir.AxisListType
    BF16 = mybir.dt.bfloat16
    F32 = mybir.dt.float32
    F8 = mybir.dt.float8e4
    DRS = mybir.MatmulPerfMode.DoubleRowSwInterleave
    ACT_E = mybir.EngineType.Activation

    nc = bacc.Bacc("TRN2", target_bir_lowering=False)

    # x^T per batch in three layouts (transposed/packed on host):
    # bf16 feature-major (residual), fp8 moving blocks (Q/K), fp8 interleaved
    # stationary (V)
    d_xtb = nc.dram_tensor("xtb", (BL * 128, KT_E * 512), BF16,
                           kind="ExternalInput")
    d_xq8 = nc.dram_tensor("xq8", (BL * 128, KP * 2 * 512), F8,
                           kind="ExternalInput")
    d_xv8 = nc.dram_tensor("xv8", (BL * 128, KP * NT * 256), F8,
                           kind="ExternalInput")
    d_wq8 = nc.dram_tensor("wq8", (128, KP * KT_E * 256), F8, kind="ExternalInput")
    d_wk8 = nc.dram_tensor("wk8", (128, KP * KT_E * 256), F8, kind="ExternalInput")
    d_wv8 = nc.dram_tensor("wv8", (128, KP * 2 * E), F8, kind="ExternalInput")
    d_wo8 = nc.dram_tensor("wo8", (128, KP * KT_E * 256), F8, kind="ExternalInput")
    d_wso = nc.dram_tensor("wso", (E, E), BF16, kind="ExternalInput")
    d_wi = nc.dram_tensor("wi", (E, FF), BF16, kind="ExternalInput")
    d_wout = nc.dram_tensor("wout", (FF, E), BF16, kind="ExternalInput")
    d_mcol = nc.dram_tensor("mcol", (128, BL * NT), F32, kind="ExternalInput")
    d_ones65 = nc.dram_tensor("ones65", (65, 128), BF16, kind="ExternalInput")
    d_id = nc.dram_tensor("ident", (128, 128), BF16, kind="ExternalInput")
    d_onesr = nc.dram_tensor("onesrow", (1, 512), BF16, kind="ExternalInput")
    # bias rows: 0=bq/8, 1=bk, 2=bv, 3=bo, 4=bso, 5=bout, 6=bi (full FF width)
    d_brow = nc.dram_tensor("brow", (7, FF), BF16, kind="ExternalInput")
    d_bic = nc.dram_tensor("bicol", (128, FT), F32, kind="ExternalInput")
    # gamma1 | gamma2 | beta1 | beta2, each [128, 768] partition-broadcast
    d_gb = nc.dram_tensor("gb", (128, 4 * E), F32, kind="ExternalInput")
    d_out = nc.dram_tensor("out", (T, E), F32, kind="ExternalOutput")
    # xa (x + att@Wo, feature-major bf16) spills to DRAM between superphases
    d_xa = nc.dram_tensor("xasp", (BL * 128, KT_E * 512), BF16, kind="Internal")

    need_gb = use_g1 or use_b1 or use_g2 or use_b2
    need_brow = use_bq or use_bk or use_bv or use_bo or use_bso or use_bout

    with ExitStack() as ctx:
        tc = ctx.enter_context(tile.TileContext(nc))

        p_mm = ctx.enter_context(tc.tile_pool(name="p_mm", bufs=8, space="PSUM"))

        c_pool = ctx.enter_context(tc.tile_pool(name="consts", bufs=1))
        wa_pool = ctx.enter_context(tc.tile_pool(name="wa", bufs=1))
        wso_pool = ctx.enter_context(tc.tile_pool(name="wso", bufs=KT_E))
        wi_pool = ctx.enter_context(tc.tile_pool(name="wi", bufs=KT_E))
        wout_pool = ctx.enter_context(tc.tile_pool(name="wout", bufs=FT))
        xa_pool = ctx.enter_context(tc.tile_pool(name="xa", bufs=2))

        ident = c_pool.tile_from(d_id[:, :], name="ident")
        mcol = c_pool.tile_from(d_mcol[:, :], name="mcol")
        ones65 = c_pool.tile_from(d_ones65[:, :], name="ones65")
        onesr = c_pool.tile_from(d_onesr[:, :], name="onesr") \
            if (use_bv or use_bso or use_bout or use_bq or use_bk or use_bo) else None
        brow = c_pool.tile_from(d_brow[:, :], name="brow") if need_brow else None
        gb = c_pool.tile_from(d_gb[:, :], name="gb") if need_gb else None
        bic = c_pool.tile_from(d_bic[:, :], name="bic") if use_bi else None

        # phase-A weights (fp8, small): default (SP) DMA queue
        wq8 = wa_pool.tile_from(d_wq8[:, :], name="wq8t")
        wk8 = wa_pool.tile_from(d_wk8[:, :], name="wk8t")
        wv8 = wa_pool.tile_from(d_wv8[:, :], name="wv8t")
        wo8 = wa_pool.tile_from(d_wo8[:, :], name="wo8t")

        # stationary (sw-interleaved) weights: slice (p, et) -> [128, 256]
        WQ8 = wq8.rearrange("r (p e c) -> r p e c", p=KP, e=KT_E)
        WK8 = wk8.rearrange("r (p e c) -> r p e c", p=KP, e=KT_E)
        WO8 = wo8.rearrange("r (p e c) -> r p e c", p=KP, e=KT_E)
        # moving (block) V weights: slice p -> [128, 2, E]
        WV8 = wv8.rearrange("r (p i m) -> r p i m", p=KP, i=2)

        # phase-B weights (bf16, 10.6MB) stream on the Activation DMA queue,
        # staggered through phase A so they never compete with critical loads
        WSO, WI, WOUT = [], [], []

        def load_b_weights(stage):
            if stage == 0:
                WSO.extend(wso_pool.tile_from(
                    d_wso[k * 128:(k + 1) * 128, :], name="wsot",
                    forced_dma_engine=ACT_E) for k in range(KT_E))
            elif stage == 1:
                WI.extend(wi_pool.tile_from(
                    d_wi[k * 128:(k + 1) * 128, :], name="wit",
                    forced_dma_engine=ACT_E) for k in range(KT_E))
            else:
                f0 = 0 if stage == 2 else FT // 2
                f1 = FT // 2 if stage == 2 else FT
                WOUT.extend(wout_pool.tile_from(
                    d_wout[f * 128:(f + 1) * 128, :], name="woutt",
                    forced_dma_engine=ACT_E) for f in range(f0, f1))

        # ================= superphase A: QKV, attention, O-proj ==============
        with ExitStack() as sa:
            xtb_pool = sa.enter_context(tc.tile_pool(name="xtb", bufs=2))
            xq8_pool = sa.enter_context(tc.tile_pool(name="xq8", bufs=1))
            xv8_pool = sa.enter_context(tc.tile_pool(name="xv8", bufs=1))
            qt_pool = sa.enter_context(tc.tile_pool(name="qt", bufs=2))
            kt_pool = sa.enter_context(tc.tile_pool(name="kt", bufs=2))
            va_pool = sa.enter_context(tc.tile_pool(name="va", bufs=1))
            se_pool = sa.enter_context(tc.tile_pool(name="se", bufs=24))
            at_pool = sa.enter_context(tc.tile_pool(name="at", bufs=4))
            sg_pool = sa.enter_context(tc.tile_pool(name="sg", bufs=2))
            rs_pool = sa.enter_context(tc.tile_pool(name="rs", bufs=3))
            rb_pool = sa.enter_context(tc.tile_pool(name="rb", bufs=3))

            # persistent V tiles (2 sets x 2 token-pair tiles), ones column
            # preset once; V-projection overwrites only the V part each batch
            VAUG = [[va_pool.tile([128, H * 256], F8, name="vaug", tag="va",
                                  bufs=4) for _ in range(2)] for _ in range(2)]
            for st in range(2):
                for pp in range(2):
                    v4 = VAUG[st][pp].rearrange("r (h c) -> r h c", h=H)
                    nc.gpsimd.memset(v4[:, :, 0:126], 0.0)
                    nc.gpsimd.memset(v4[:, :, 126:128], 1.0)

            ST = {}   # per-batch state

            def s1_dma(b):
                """x^T loads (pre-transposed/packed on host)."""
                st = {}
                xtb = xtb_pool.tile([128, KT_E * 512], BF16, name="xtb",
                                    tag="xtb")
                xq8 = xq8_pool.tile([128, KP * 2 * 512], F8, name="xq8",
                                    tag="xq8")
                xv8 = xv8_pool.tile([128, KP * NT * 256], F8, name="xv8",
                                    tag="xv8")
                nc.gpsimd.dma_start(xtb[:, :], d_xtb[b * 128:(b + 1) * 128, :])
                nc.gpsimd.dma_start(xq8[:, :], d_xq8[b * 128:(b + 1) * 128, :])
                nc.gpsimd.dma_start(xv8[:, :], d_xv8[b * 128:(b + 1) * 128, :])
                st["xtb"], st["xq8"], st["xv8"] = xtb, xq8, xv8
                st["qt"] = qt_pool.tile([128, HP * 512], BF16, name="qtt",
                                        tag="qt")
                st["kt"] = kt_pool.tile([128, HP * 512], BF16, name="ktt",
                                        tag="kt")
                ST[b] = st

            def s1qk_chunk(b, j):
                """Two feature-blocks of the Q (j<3) or K (j>=3) projection."""
                st = ST[b]
                xq8_p = st["xq8"].rearrange("r (p i t) -> r p i t", p=KP, i=2)
                W8, dst, ub, brx = ((WQ8, st["qt"], use_bq, 0) if j < 3 else
                                    (WK8, st["kt"], use_bk, 1))
                for et in (2 * (j % 3), 2 * (j % 3) + 1):
                    ps = p_mm.tile([128, 512], F32, name="qkps", tag="mm")
                    for p in range(KP):
                        nc.tensor.matmul(
                            ps[:, :], W8[:, p, et, :], xq8_p[:, p, :, :],
                            perf_mode=DRS,
                            start=(p == 0), stop=(p == KP - 1 and not ub))
                    if ub:
                        nc.tensor.matmul(
                            ps[:, :],
                            brow[brx:brx + 1, et * 128:(et + 1) * 128],
                            onesr[0:1, 0:S], start=False, stop=True)
                    nc.vector.tensor_copy(dst[:, et * 512:(et + 1) * 512],
                                          ps[:, :])

            def s1v(b):
                """V projection (token-major; per-head columns reversed on the
                host so the interleaved write is an ascending stride-2 copy)."""
                st = ST[b]
                xv8_p = st["xv8"].rearrange("r (p t c) -> r p t c", p=KP, t=NT)
                vset = VAUG[b % 2]
                for tt in range(NT):
                    for ec, n in ((0, 512), (512, 256)):
                        ps = (p_mm.tile([128, 512], F32, name="vps", tag="mm")
                              if n == 512 else
                              p_mm.tile([128, 256], F32, name="vps2", tag="mm"))
                        for p in range(KP):
                            nc.tensor.matmul(
                                ps[:, :n], xv8_p[:, p, tt, :],
                                WV8[:, p, :, ec:ec + n], perf_mode=DRS,
                                start=(p == 0), stop=(p == KP - 1 and not use_bv))
                        if use_bv:
                            nc.tensor.matmul(
                                ps[:, :n], onesr[0:1, 0:128],
                                brow[2:3, ec:ec + n], start=False, stop=True)
                        h0, nh = ec // 64, n // 64
                        v6 = vset[tt // 2].rearrange(
                            "r (h a c i) -> r h a c i", h=H, a=2, c=64)
                        nc.vector.tensor_copy(
                            v6[:, h0:h0 + nh, 1, :, tt % 2],
                            ps[:, :n].rearrange("r (h c) -> r h c", h=nh))
                st["vset"] = vset

            def sc_hp(b, hp):
                """scores for one head-pair (k-major, row-group dual-issue)
                + masked exp."""
                st = ST[b]
                qtt, ktt = st["qt"], st["kt"]
                stexp = st.setdefault("stexp", {})
                for kt in range(NT):
                    pss = []
                    for hh in range(2):
                        o = hh * 64
                        ps = p_mm.tile([128, 512], F32, name="scps", tag="mm")
                        nc.tensor.matmul(
                            ps[:, :],
                            ktt[o:o + 64,
                                hp * 512 + kt * 128:hp * 512 + (kt + 1) * 128],
                            qtt[o:o + 64, hp * 512:(hp + 1) * 512],
                            start=True, stop=True)
                        pss.append(ps)
                    for hh in range(2):
                        if (hp, hh, kt // 2) not in stexp:
                            stexp[(hp, hh, kt // 2)] = se_pool.tile(
                                [128, 2 * 512], F8, name="sexp", tag="se")
                        dst = stexp[(hp, hh, kt // 2)]
                        nc.scalar.activation(
                            dst[:, (kt % 2) * 512:(kt % 2 + 1) * 512],
                            pss[hh][:, :], AF.Exp,
                            bias=mcol[:, b * NT + kt:b * NT + kt + 1])

            def s2_hp(b, hp):
                """AV for one head-pair (fp8 DoubleRow, ones-column
                denominators), normalize, assemble ATT pairs."""
                st = ST[b]
                stexp, vset = st["stexp"], st["vset"]
                if "att8" not in st:
                    st["att8"] = [at_pool.tile([128, 2 * 512], F8,
                                               name="att8", tag="at")
                                  for _ in range(KP)]
                att8 = st["att8"]
                for hh in range(2):
                    h = 2 * hp + hh
                    av = p_mm.tile([128, 512], F32, name="avps", tag="mm")
                    for pp in range(2):
                        nc.tensor.matmul(
                            av[:, :], vset[pp][:, h * 256:(h + 1) * 256],
                            stexp[(hp, hh, pp)].rearrange(
                                "r (i t) -> r i t", i=2),
                            perf_mode=DRS, start=(pp == 0), stop=(pp == 1))
                    rs = rs_pool.tile([65, 512], BF16, name="rst", tag="rs")
                    with nc.allow_low_precision(
                            reason="bf16 1/s for broadcast (0.4% rel)"):
                        nc.vector.reciprocal(rs[64:65, :], av[64:65, :])
                    rbc = p_mm.tile([64, 512], F32, name="rbc", tag="mm")
                    nc.tensor.matmul(rbc[:, :], ones65[64:65, 0:64],
                                     rs[64:65, :], start=True, stop=True)
                    rbs = rb_pool.tile([64, 512], BF16, name="rbs", tag="rb")
                    nc.vector.tensor_copy(rbs[:, :], rbc[:, :])
                    kp, half = hp // 2, hp % 2
                    if hh == 0:
                        nc.vector.scalar_tensor_tensor(
                            att8[kp][0:64, half * 512:(half + 1) * 512],
                            av[0:64, :], 1.0, rbs[:, :],
                            op0=OP.mult, op1=OP.mult)
                    else:
                        stg = sg_pool.tile([64, 512], F8, name="stg", tag="sg")
                        nc.vector.scalar_tensor_tensor(
                            stg[:, :], av[0:64, :], 1.0, rbs[:, :],
                            op0=OP.mult, op1=OP.mult)
                        nc.gpsimd.dma_start(
                            att8[kp][64:128, half * 512:(half + 1) * 512],
                            stg[:, :])

            def s2_o(b):
                """O-projection (fp8) + residual -> xa (feature-major bf16)."""
                st = ST[b]
                att8, xtb = st["att8"], st["xtb"]
                xa = xa_pool.tile([128, KT_E * 512], BF16, name="xat", tag="xa")
                for et in range(KT_E):
                    ps = p_mm.tile([128, 512], F32, name="ops", tag="mm")
                    for kp in range(KP):
                        nc.tensor.matmul(
                            ps[:, :], WO8[:, kp, et, :],
                            att8[kp].rearrange("r (i t) -> r i t", i=2),
                            perf_mode=DRS,
                            start=(kp == 0), stop=(kp == KP - 1 and not use_bo))
                    if use_bo:
                        nc.tensor.matmul(
                            ps[:, :], brow[3:4, et * 128:(et + 1) * 128],
                            onesr[0:1, 0:S], start=False, stop=True)
                    nc.vector.scalar_tensor_tensor(
                        xa[:, et * 512:(et + 1) * 512], ps[:, :], 1.0,
                        xtb[:, et * 512:(et + 1) * 512],
                        op0=OP.mult, op1=OP.add)
                nc.gpsimd.dma_start(d_xa[b * 128:(b + 1) * 128, :],
                                    xa[:, :])
                del ST[b]

            # software-pipelined emission: each head-pair iteration carries
            # the previous batch's AV/normalize, a chunk of the next batch's
            # Q/K projection, and the current batch's scores, so the in-order
            # PE queue always holds work that is independent of the ACT
            # engine's (serial, 28us/batch) exp backlog. V projections and the
            # O-projection sit at block boundaries; phase-B weights stream in
            # stages on the Activation DMA queue.
            s1_dma(0)
            for j in range(HP):
                s1qk_chunk(0, j)
            s1v(0)
            s1_dma(1)
            for hp in range(HP):
                s1qk_chunk(1, hp)
                sc_hp(0, hp)
            s1v(1)
            load_b_weights(0)
            for bn in (1, 2):
                s1_dma(bn + 1)
                for hp in range(HP):
                    s2_hp(bn - 1, hp)
                    s1qk_chunk(bn + 1, hp)
                    sc_hp(bn, hp)
                s2_o(bn - 1)
                s1v(bn + 1)
                load_b_weights(bn)
            for hp in range(HP):
                s2_hp(2, hp)
                sc_hp(3, hp)
            s2_o(2)
            load_b_weights(3)
            for hp in range(HP):
                s2_hp(3, hp)
            s2_o(3)

        # ============ superphase B: SelfOutput LN, FFN, LN ===================
        with ExitStack() as sb:
            h_pool = sb.enter_context(tc.tile_pool(name="h", bufs=NT + 1))
            ht_pool = sb.enter_context(tc.tile_pool(name="ht", bufs=2))
            fft_pool = sb.enter_context(tc.tile_pool(name="fft", bufs=FT + 2))
            sq_pool = sb.enter_context(tc.tile_pool(name="sq", bufs=2))
            rs_pool = sb.enter_context(tc.tile_pool(name="rsd", bufs=3))
            out_pool = sb.enter_context(tc.tile_pool(name="outp", bufs=2))
            t_pool = sb.enter_context(tc.tile_pool(name="sb_s", bufs=12))

            def layernorm(chunks, h_dst, gcol, use_g, use_bb, resid=None):
                """chunks: [(psum_ap, col0, n)]; h_dst: [128, E] out.
                Drains PSUM chunks to SBUF immediately (rtile) so the banks
                free early; stats then run from SBUF."""
                if resid is not None:
                    rtile = rs_pool.tile([128, E], F32, name="rt", tag="rsd")
                    for (ps, c0, n), rext in zip(chunks, resid):
                        nc.vector.scalar_tensor_tensor(
                            rtile[:, c0:c0 + n], ps, 1.0, rext,
                            op0=OP.mult, op1=OP.add)
                    srcs = [(rtile[:, c0:c0 + n], c0, n)
                            for (_, c0, n) in chunks]
                else:
                    srcs = chunks
                s1t = t_pool.tile([128, 1], F32, name="s1", tag="s1")
                s1b = t_pool.tile([128, 1], F32, name="s1b", tag="s1b")
                nc.vector.reduce_sum(s1t[:, :], srcs[0][0], axis=AX.X)
                nc.vector.reduce_sum(s1b[:, :], srcs[1][0], axis=AX.X)
                mu_n = t_pool.tile([128, 1], F32, name="mun", tag="mun")
                tmp = t_pool.tile([128, 1], F32, name="tmps", tag="tmps")
                nc.vector.scalar_tensor_tensor(
                    tmp[:, :], s1t[:, :], 1.0, s1b[:, :], op0=OP.mult, op1=OP.add)
                nc.vector.tensor_scalar_mul(mu_n[:, :], tmp[:, :], -1.0 / E)
                ss = t_pool.tile([128, 1], F32, name="ssa", tag="ssa", bufs=34)
                ssb = t_pool.tile([128, 1], F32, name="ssb", tag="ssb", bufs=34)
                for (src, c0, n), acc in zip(srcs, (ss, ssb)):
                    sq = sq_pool.tile([128, 512], BF16, name="sqt", tag="sq")
                    nc.scalar.activation(sq[:, :n], src, AF.Square,
                                         accum_out=acc[:, :])
                musq = t_pool.tile([128, 1], F32, name="musq", tag="musq")
                nc.vector.scalar_tensor_tensor(
                    musq[:, :], mu_n[:, :], 1.0, mu_n[:, :],
                    op0=OP.mult, op1=OP.mult)
                veps = t_pool.tile([128, 1], F32, name="veps", tag="veps")
                nc.vector.scalar_tensor_tensor(
                    veps[:, :], ss[:, :], 1.0, ssb[:, :],
                    op0=OP.mult, op1=OP.add)
                veps2 = t_pool.tile([128, 1], F32, name="veps2", tag="veps2")
                nc.vector.tensor_scalar(
                    veps2[:, :], veps[:, :], 1.0 / E, EPS,
                    op0=OP.mult, op1=OP.add)
                veps3 = t_pool.tile([128, 1], F32, name="veps3", tag="veps3")
                nc.vector.scalar_tensor_tensor(
                    veps3[:, :], musq[:, :], -1.0, veps2[:, :],
                    op0=OP.mult, op1=OP.add)
                sd = t_pool.tile([128, 1], F32, name="sd", tag="sd")
                nc.scalar.sqrt(sd[:, :], veps3[:, :])
                rstd = t_pool.tile([128, 1], F32, name="rstd", tag="rstd")
                nc.vector.reciprocal(rstd[:, :], sd[:, :])
                for (src, c0, n) in srcs:
                    nc.vector.tensor_scalar(
                        h_dst[:, c0:c0 + n], src, mu_n[:, :], rstd[:, :],
                        op0=OP.add, op1=OP.mult)
                if use_g:
                    nc.vector.scalar_tensor_tensor(
                        h_dst[:, :], h_dst[:, :], 1.0,
                        gb[:, gcol * E:(gcol + 1) * E], op0=OP.mult, op1=OP.mult)
                if use_bb:
                    nc.vector.scalar_tensor_tensor(
                        h_dst[:, :], h_dst[:, :], 1.0,
                        gb[:, (gcol + 2) * E:(gcol + 3) * E],
                        op0=OP.mult, op1=OP.add)

            XAB = {}

            def load_xa(b):
                xab = xa_pool.tile([128, KT_E * 512], BF16, name="xab",
                                   tag="xa")
                nc.gpsimd.dma_start(xab[:, :],
                                    d_xa[b * 128:(b + 1) * 128, :])
                XAB[b] = xab

            load_xa(0)
            for b in range(BL):
                t0 = b * S
                if b + 1 < BL:
                    load_xa(b + 1)
                xa = XAB.pop(b).rearrange("r (e t) -> r e t", e=KT_E)

                # ---- SelfOutput GEMM + LN1 -> h (token-major), hT ----
                hh_t = [None] * NT
                hT = ht_pool.tile([128, KT_E * S], BF16, name="htt", tag="ht")

                def emit_htrans(tt):
                    tps = [p_mm.tile([128, 512], BF16, name="htp", tag="mm")
                           for _ in range(2)]
                    for et in range(KT_E):
                        sl = tps[et // 4][:, (et % 4) * 128:(et % 4 + 1) * 128]
                        nc.tensor.transpose(
                            sl, hh_t[tt][:, et * 128:(et + 1) * 128],
                            ident[:, :])
                    for et in range(KT_E):
                        sl = tps[et // 4][:, (et % 4) * 128:(et % 4 + 1) * 128]
                        nc.vector.tensor_copy(
                            hT[:, et * S + tt * 128:et * S + (tt + 1) * 128], sl)

                for tt in range(NT):
                    ch = []
                    for ec, n in ((0, 512), (512, 256)):
                        ps = (p_mm.tile([128, 512], F32, name="sops", tag="mm")
                              if n == 512 else
                              p_mm.tile([128, 256], F32, name="sops2", tag="mm"))
                        for k in range(KT_E):
                            nc.tensor.matmul(
                                ps[:, :n], xa[:, k, tt * 128:(tt + 1) * 128],
                                WSO[k][:, ec:ec + n],
                                start=(k == 0),
                                stop=(k == KT_E - 1 and not use_bso))
                        if use_bso:
                            nc.tensor.matmul(
                                ps[:, :n], onesr[0:1, 0:128],
                                brow[4:5, ec:ec + n], start=False, stop=True)
                        ch.append((ps[:, :n], ec, n))
                    hh_t[tt] = h_pool.tile([128, E], BF16, name="hht", tag="h")
                    layernorm(ch, hh_t[tt], 0, use_g1, use_b1)
                    if tt > 0:
                        emit_htrans(tt - 1)
                emit_htrans(NT - 1)

                # ---- FFN + LN2 ----
                ffT = [None] * FT
                for ft in range(FT):
                    ps = p_mm.tile([128, 512], F32, name="fips", tag="mm")
                    for k in range(KT_E):
                        nc.tensor.matmul(
                            ps[:, :], WI[k][:, ft * 128:(ft + 1) * 128],
                            hT[:, k * S:k * S + 512],
                            start=(k == 0), stop=(k == KT_E - 1))
                    ffT[ft] = fft_pool.tile([128, 512], BF16, name="fftt",
                                            tag="fft")
                    if use_bi:
                        nc.scalar.activation(ffT[ft][:, :], ps[:, :], AF.Gelu,
                                             bias=bic[:, ft:ft + 1])
                    else:
                        nc.scalar.activation(ffT[ft][:, :], ps[:, :], AF.Gelu)
                for tt in range(NT):
                    ch = []
                    for ec, n in ((0, 512), (512, 256)):
                        ps = (p_mm.tile([128, 512], F32, name="wops", tag="mm")
                              if n == 512 else
                              p_mm.tile([128, 256], F32, name="wops2", tag="mm"))
                        for f in range(FT):
                            nc.tensor.matmul(
                                ps[:, :n],
                                ffT[f][:, tt * 128:(tt + 1) * 128],
                                WOUT[f][:, ec:ec + n],
                                start=(f == 0),
                                stop=(f == FT - 1 and not use_bout))
                        if use_bout:
                            nc.tensor.matmul(
                                ps[:, :n], onesr[0:1, 0:128],
                                brow[5:6, ec:ec + n], start=False, stop=True)
                        ch.append((ps[:, :n], ec, n))
                    otile = out_pool.tile([128, E], F32, name="ot", tag="outp")
                    resid = [hh_t[tt][:, ec:ec + n] for (_, ec, n) in ch]
                    layernorm(ch, otile, 1, use_g2, use_b2, resid=resid)
                    nc.gpsimd.dma_start(
                        d_out[t0 + tt * 128:t0 + (tt + 1) * 128, :],
                        otile[:, :])
    nc.compile()
    return nc


def _get_program(flags):
    key = ("prog", flags)
    if key not in _CACHE:
        _CACHE[key] = _build(flags)
    return _CACHE[key]


def kernel(x, mask, Wq, bq, Wk, bk, Wv, bv, Wo, bo,
           Wso, bso, gso, beso, Wi, bi, Wout, bout, gout, beout):
    from concourse.bass_utils import run_bass_kernel_spmd

    x = np.asarray(x, np.float32)
    mask = np.asarray(mask)
    sc = 1.0 / float(np.sqrt(np.float32(DK)))

    z = lambda a: not np.any(np.asarray(a))
    one = lambda a: bool(np.all(np.asarray(a) == 1.0))
    flags = (not z(bq), not z(bk), not z(bv), not z(bo), not z(bso),
             not z(bi), not z(bout),
             not one(gso), not z(beso), not one(gout), not z(beout))
    nc = _get_program(flags)

    wq8 = _pack_sw(np.asarray(Wq, np.float32) * sc)
    wk8, wo8 = _pack_sw(Wk), _pack_sw(Wo)
    # reverse V's 64 columns within each head so the on-device interleaved
    # write of the AV stationary is an ascending stride-2 copy
    wv_re = np.asarray(Wv, np.float32).reshape(E, H, DK)[:, :, ::-1].reshape(E, E)
    wv8 = _pack_blk(wv_re)
    wso_b, wi_b, wout_b = _bf(Wso), _bf(Wi), _bf(Wout)
    identb = _bf(np.eye(128))
    onesr = _bf(np.ones((1, 512)))

    brow = np.zeros((7, FF), np.float32)
    brow[0, :E] = np.asarray(bq, np.float32) * sc
    for i, v in enumerate((bk, bv, bo, bso, bout)):
        brow[i + 1, :E] = v
    brow[6, :] = bi
    brow = _bf(brow)
    bicol = np.asarray(bi, np.float32).reshape(FT, 128).T.copy()
    gbt = np.zeros((128, 4 * E), np.float32)
    for i, g in enumerate((gso, gout, beso, beout)):
        gbt[:, i * E:(i + 1) * E] = np.broadcast_to(
            np.asarray(g, np.float32).reshape(1, E), (128, E))

    in_maps = []
    for c in range(NCORES):
        xs = x[c * BL:(c + 1) * BL]            # [BL, S, E]
        ms = np.asarray(mask[c * BL:(c + 1) * BL]).reshape(BL, S)
        # mcol[r, b*NT + kt] = bias for key token kt*128 + r of batch b
        mb = np.where(ms == 0, np.float32(MASK_NEG), np.float32(0.0))
        mcol = np.ascontiguousarray(
            mb.reshape(BL, NT, 128).transpose(2, 0, 1).reshape(128, BL * NT))
        # x^T per batch in the three on-device layouts
        xtb = np.empty((BL * 128, KT_E * 512), np.float32)
        xq8 = np.empty((BL * 128, KP * 2 * 512), ml_dtypes.float8_e4m3)
        xv8 = np.empty((BL * 128, KP * NT * 256), ml_dtypes.float8_e4m3)
        for b in range(BL):
            xt = np.ascontiguousarray(xs[b].T)               # [E, S]
            xtb[b * 128:(b + 1) * 128] = xt.reshape(
                KT_E, 128, S).transpose(1, 0, 2).reshape(128, KT_E * S)
            xq8[b * 128:(b + 1) * 128] = _pack_blk(xt)
            xv8[b * 128:(b + 1) * 128] = _pack_sw(xt)
        in_maps.append({
            "ones65": _bf(np.ones((65, 128))),
            "xtb": _bf(xtb), "xq8": xq8, "xv8": xv8,
            "wq8": wq8, "wk8": wk8, "wv8": wv8, "wo8": wo8,
            "wso": wso_b, "wi": wi_b, "wout": wout_b, "mcol": mcol,
            "ident": identb,
            "onesrow": onesr, "brow": brow, "bicol": bicol, "gb": gbt,
        })

    trace = os.environ.get("KERNEL_TRACE", "0") == "1"
    res = run_bass_kernel_spmd(nc, in_maps, core_ids=list(range(NCORES)),
                               trace=trace)
    if trace and res.exec_time_ns is not None:
        print(f"HW exec time: {res.exec_time_ns} ns")
        if res.instructions_and_trace is not None:
            print(f"trace: {res.instructions_and_trace[1]}")
    out = np.concatenate([r["out"].reshape(BL, S, E) for r in res.results],
                         axis=0)
    return np.ascontiguousarray(out.astype(np.float32))


# revision 22
# speedup vs baseline: 1.0624x; 1.0501x over previous
"""BERT-base encoder layer on 8 Trainium2 NeuronCores (Bass/Tile).

Sharding: data-parallel over batch. Full inputs [32, 512, 768] split into 8
shards of 4 batches (2048 tokens); every core runs the same NEFF on its shard
(SPMD, no collectives); host concatenates the outputs.

Attention is computed k-major: scores are built transposed (ST[k, q] = K·Q^T)
so that softmax probabilities come out already in the layout the P·V matmul
needs — no PE transpose of P, and the key mask becomes a per-partition bias
on the exp activation (free) instead of rank-1 matmuls. The softmax
denominator comes from a ones-column appended to V (row 64 of the AV PSUM);
normalization is a rank-1 broadcast matmul + one vector multiply.

QKV/V/AV/O-projection GEMMs run in fp8(e4m3) DoubleRow mode (2 contraction
rows per PE pass); Wso/Wi/Wout GEMMs stay bf16 for accuracy. PSUM accumulation
is fp32 everywhere; layernorm statistics fp32.
"""

import os
import numpy as np
import ml_dtypes

B, S, E, H, DK, FF = 32, 512, 768, 12, 64, 3072
NCORES = 8
BL = B // NCORES          # batches per core = 4
T = BL * S                # tokens per core = 2048
EPS = 1e-12
MASK_NEG = -87.0          # exp(-87) == 0 in fp8/bf16
KT_E = E // 128           # 6 feature blocks
KP = KT_E // 2            # 3 fp8 contraction pairs
NT = S // 128             # 4 token tiles
FT = FF // 128            # 24
HP = H // 2               # 6 head pairs

_CACHE = {}


def _bf(a):
    return np.ascontiguousarray(np.asarray(a, np.float32).astype(ml_dtypes.bfloat16))


def _f8(a):
    a = np.clip(np.asarray(a, np.float32), -240.0, 240.0)
    return np.ascontiguousarray(a.astype(ml_dtypes.float8_e4m3))


def _pack_blk(w):
    """Moving-operand block format: [K, N] -> [128, (K//256)*2*N] fp8; slice p
    gives [128, 2, N] with element [r, i, m] = w[256p + 128i + r, m]."""
    K, N = w.shape
    p = K // 256
    arr = np.asarray(w, np.float32).reshape(p, 2, 128, N).transpose(2, 0, 1, 3)
    return _f8(arr.reshape(128, p * 2 * N))


def _pack_sw(w):
    """Stationary sw-interleave format for dual-fp8 LDWEIGHTS: [K, N] ->
    [128, (K//256)*(N//128)*256]; block (p, nb) holds column m of k-pair i at
    position 2*(127-m)+i."""
    K, N = w.shape
    P, NB = K // 256, N // 128
    a = np.asarray(w, np.float32).reshape(P, 2, 128, NB, 128)
    a = a.transpose(2, 0, 3, 4, 1)[:, :, :, ::-1, :]     # [r, p, nb, m_rev, i]
    return _f8(a.reshape(128, P * NB * 256))


def _build(flags):
    import concourse.bass as bass
    import concourse.bacc as bacc
    import concourse.mybir as mybir
    import concourse.tile as tile
    from contextlib import ExitStack

    (use_bq, use_bk, use_bv, use_bo, use_bso, use_bi, use_bout,
     use_g1, use_b1, use_g2, use_b2) = flags

    AF = mybir.ActivationFunctionType
    OP = mybir.AluOpType
    AX = mybir.AxisListType
    BF16 = mybir.dt.bfloat16
    F32 = mybir.dt.float32
    F8 = mybir.dt.float8e4
    DRS = mybir.MatmulPerfMode.DoubleRowSwInterleave
    ACT_E = mybir.EngineType.Activation

    nc = bacc.Bacc("TRN2", target_bir_lowering=False)

    # x^T per batch in three layouts (transposed/packed on host):
    # bf16 feature-major (residual), fp8 moving blocks (Q/K), fp8 interleaved
    # stationary (V)
    d_xtb = nc.dram_tensor("xtb", (BL * 128, KT_E * 512), BF16,
                           kind="ExternalInput")
    d_xq8 = nc.dram_tensor("xq8", (BL * 128, KP * 2 * 512), F8,
                           kind="ExternalInput")
    d_xv8 = nc.dram_tensor("xv8", (BL * 128, KP * NT * 256), F8,
                           kind="ExternalInput")
    d_wq8 = nc.dram_tensor("wq8", (128, KP * KT_E * 256), F8, kind="ExternalInput")
    d_wk8 = nc.dram_tensor("wk8", (128, KP * KT_E * 256), F8, kind="ExternalInput")
    d_wv8 = nc.dram_tensor("wv8", (128, KP * 2 * E), F8, kind="ExternalInput")
    d_wo8 = nc.dram_tensor("wo8", (128, KP * KT_E * 256), F8, kind="ExternalInput")
    d_wso = nc.dram_tensor("wso", (E, E), BF16, kind="ExternalInput")
    d_wi = nc.dram_tensor("wi", (E, FF), BF16, kind="ExternalInput")
    d_wout = nc.dram_tensor("wout", (FF, E), BF16, kind="ExternalInput")
    d_mcol = nc.dram_tensor("mcol", (128, BL * NT), F32, kind="ExternalInput")
    d_ones65 = nc.dram_tensor("ones65", (65, 128), BF16, kind="ExternalInput")
    d_id = nc.dram_tensor("ident", (128, 128), BF16, kind="ExternalInput")
    d_onesr = nc.dram_tensor("onesrow", (1, 512), BF16, kind="ExternalInput")
    # bias rows: 0=bq/8, 1=bk, 2=bv, 3=bo, 4=bso, 5=bout, 6=bi (full FF width)
    d_brow = nc.dram_tensor("brow", (7, FF), BF16, kind="ExternalInput")
    d_bic = nc.dram_tensor("bicol", (128, FT), F32, kind="ExternalInput")
    # gamma1 | gamma2 | beta1 | beta2, each [128, 768] partition-broadcast
    d_gb = nc.dram_tensor("gb", (128, 4 * E), F32, kind="ExternalInput")
    d_out = nc.dram_tensor("out", (T, E), F32, kind="ExternalOutput")
    # xa (x + att@Wo, feature-major bf16) spills to DRAM between superphases
    d_xa = nc.dram_tensor("xasp", (BL * 128, KT_E * 512), BF16, kind="Internal")

    need_gb = use_g1 or use_b1 or use_g2 or use_b2
    need_brow = use_bq or use_bk or use_bv or use_bo or use_bso or use_bout

    with ExitStack() as ctx:
        tc = ctx.enter_context(tile.TileContext(nc))

        p_mm = ctx.enter_context(tc.tile_pool(name="p_mm", bufs=6, space="PSUM"))
        p_av = ctx.enter_context(tc.tile_pool(name="p_av", bufs=2, space="PSUM"))

        c_pool = ctx.enter_context(tc.tile_pool(name="consts", bufs=1))
        wa_pool = ctx.enter_context(tc.tile_pool(name="wa", bufs=1))
        wso_pool = ctx.enter_context(tc.tile_pool(name="wso", bufs=KT_E))
        wi_pool = ctx.enter_context(tc.tile_pool(name="wi", bufs=KT_E))
        wout_pool = ctx.enter_context(tc.tile_pool(name="wout", bufs=FT))
        xa_pool = ctx.enter_context(tc.tile_pool(name="xa", bufs=2))

        ident = c_pool.tile_from(d_id[:, :], name="ident")
        mcol = c_pool.tile_from(d_mcol[:, :], name="mcol")
        ones65 = c_pool.tile_from(d_ones65[:, :], name="ones65")
        onesr = c_pool.tile_from(d_onesr[:, :], name="onesr") \
            if (use_bv or use_bso or use_bout or use_bq or use_bk or use_bo) else None
        brow = c_pool.tile_from(d_brow[:, :], name="brow") if need_brow else None
        gb = c_pool.tile_from(d_gb[:, :], name="gb") if need_gb else None
        bic = c_pool.tile_from(d_bic[:, :], name="bic") if use_bi else None

        # phase-A weights (fp8, small): default (SP) DMA queue
        wq8 = wa_pool.tile_from(d_wq8[:, :], name="wq8t")
        wk8 = wa_pool.tile_from(d_wk8[:, :], name="wk8t")
        wv8 = wa_pool.tile_from(d_wv8[:, :], name="wv8t")
        wo8 = wa_pool.tile_from(d_wo8[:, :], name="wo8t")

        # stationary (sw-interleaved) weights: slice (p, et) -> [128, 256]
        WQ8 = wq8.rearrange("r (p e c) -> r p e c", p=KP, e=KT_E)
        WK8 = wk8.rearrange("r (p e c) -> r p e c", p=KP, e=KT_E)
        WO8 = wo8.rearrange("r (p e c) -> r p e c", p=KP, e=KT_E)
        # moving (block) V weights: slice p -> [128, 2, E]
        WV8 = wv8.rearrange("r (p i m) -> r p i m", p=KP, i=2)

        # phase-B weights (bf16, 10.6MB) stream on the Activation DMA queue,
        # staggered through phase A so they never compete with critical loads
        WSO, WI, WOUT = [], [], []

        def load_b_weights(stage):
            if stage == 0:
                WSO.extend(wso_pool.tile_from(
                    d_wso[k * 128:(k + 1) * 128, :], name="wsot",
                    forced_dma_engine=ACT_E) for k in range(KT_E))
            elif stage == 1:
                WI.extend(wi_pool.tile_from(
                    d_wi[k * 128:(k + 1) * 128, :], name="wit",
                    forced_dma_engine=ACT_E) for k in range(KT_E))
            else:
                f0 = 0 if stage == 2 else FT // 2
                f1 = FT // 2 if stage == 2 else FT
                WOUT.extend(wout_pool.tile_from(
                    d_wout[f * 128:(f + 1) * 128, :], name="woutt",
                    forced_dma_engine=ACT_E) for f in range(f0, f1))

        # ================= superphase A: QKV, attention, O-proj ==============
        with ExitStack() as sa:
            xtb_pool = sa.enter_context(tc.tile_pool(name="xtb", bufs=2))
            xq8_pool = sa.enter_context(tc.tile_pool(name="xq8", bufs=1))
            xv8_pool = sa.enter_context(tc.tile_pool(name="xv8", bufs=1))
            qt_pool = sa.enter_context(tc.tile_pool(name="qt", bufs=2))
            kt_pool = sa.enter_context(tc.tile_pool(name="kt", bufs=2))
            va_pool = sa.enter_context(tc.tile_pool(name="va", bufs=1))
            se_pool = sa.enter_context(tc.tile_pool(name="se", bufs=24))
            at_pool = sa.enter_context(tc.tile_pool(name="at", bufs=4))
            sg_pool = sa.enter_context(tc.tile_pool(name="sg", bufs=2))
            rs_pool = sa.enter_context(tc.tile_pool(name="rs", bufs=3))
            rb_pool = sa.enter_context(tc.tile_pool(name="rb", bufs=3))

            # persistent V tiles (2 sets x 2 token-pair tiles); zero/ones
            # regions preset once (after the first x DMAs so they don't block
            # the gpsimd queue at startup)
            VAUG = [[va_pool.tile([128, H * 256], F8, name="vaug", tag="va",
                                  bufs=4) for _ in range(2)] for _ in range(2)]

            ST = {}   # per-batch state

            def s1_dma(b):
                """x^T loads (pre-transposed/packed on host)."""
                st = {}
                xtb = xtb_pool.tile([128, KT_E * 512], BF16, name="xtb",
                                    tag="xtb")
                xq8 = xq8_pool.tile([128, KP * 2 * 512], F8, name="xq8",
                                    tag="xq8")
                xv8 = xv8_pool.tile([128, KP * NT * 256], F8, name="xv8",
                                    tag="xv8")
                nc.gpsimd.dma_start(xtb[:, :], d_xtb[b * 128:(b + 1) * 128, :])
                nc.gpsimd.dma_start(xq8[:, :], d_xq8[b * 128:(b + 1) * 128, :])
                nc.gpsimd.dma_start(xv8[:, :], d_xv8[b * 128:(b + 1) * 128, :])
                st["xtb"], st["xq8"], st["xv8"] = xtb, xq8, xv8
                st["qt"] = qt_pool.tile([128, HP * 512], BF16, name="qtt",
                                        tag="qt")
                st["kt"] = kt_pool.tile([128, HP * 512], BF16, name="ktt",
                                        tag="kt")
                ST[b] = st

            def s1qk_chunk(b, j):
                """Two feature-blocks of the Q (j<3) or K (j>=3) projection."""
                st = ST[b]
                xq8_p = st["xq8"].rearrange("r (p i t) -> r p i t", p=KP, i=2)
                W8, dst, ub, brx = ((WQ8, st["qt"], use_bq, 0) if j < 3 else
                                    (WK8, st["kt"], use_bk, 1))
                for et in (2 * (j % 3), 2 * (j % 3) + 1):
                    ps = p_mm.tile([128, 512], F32, name="qkps", tag="mm")
                    for p in range(KP):
                        nc.tensor.matmul(
                            ps[:, :], W8[:, p, et, :], xq8_p[:, p, :, :],
                            perf_mode=DRS,
                            start=(p == 0), stop=(p == KP - 1 and not ub))
                    if ub:
                        nc.tensor.matmul(
                            ps[:, :],
                            brow[brx:brx + 1, et * 128:(et + 1) * 128],
                            onesr[0:1, 0:S], start=False, stop=True)
                    nc.vector.tensor_copy(dst[:, et * 512:(et + 1) * 512],
                                          ps[:, :])

            def s1v(b):
                """V projection (token-major; per-head columns reversed on the
                host so the interleaved write is an ascending stride-2 copy)."""
                st = ST[b]
                xv8_p = st["xv8"].rearrange("r (p t c) -> r p t c", p=KP, t=NT)
                vset = VAUG[b % 2]
                for tt in range(NT):
                    for ec, n in ((0, 512), (512, 256)):
                        ps = (p_mm.tile([128, 512], F32, name="vps", tag="mm")
                              if n == 512 else
                              p_mm.tile([128, 256], F32, name="vps2", tag="mm"))
                        for p in range(KP):
                            nc.tensor.matmul(
                                ps[:, :n], xv8_p[:, p, tt, :],
                                WV8[:, p, :, ec:ec + n], perf_mode=DRS,
                                start=(p == 0), stop=(p == KP - 1 and not use_bv))
                        if use_bv:
                            nc.tensor.matmul(
                                ps[:, :n], onesr[0:1, 0:128],
                                brow[2:3, ec:ec + n], start=False, stop=True)
                        h0, nh = ec // 64, n // 64
                        v6 = vset[tt // 2].rearrange(
                            "r (h a c i) -> r h a c i", h=H, a=2, c=64)
                        nc.vector.tensor_copy(
                            v6[:, h0:h0 + nh, 1, :, tt % 2],
                            ps[:, :n].rearrange("r (h c) -> r h c", h=nh))
                st["vset"] = vset

            def sc_hp(b, hp):
                """scores for one head-pair (k-major, row-group dual-issue)
                + masked exp."""
                st = ST[b]
                qtt, ktt = st["qt"], st["kt"]
                stexp = st.setdefault("stexp", {})
                for kt in range(NT):
                    pss = []
                    for hh in range(2):
                        o = hh * 64
                        ps = p_mm.tile([128, 512], F32, name="scps", tag="mm")
                        nc.tensor.matmul(
                            ps[:, :],
                            ktt[o:o + 64,
                                hp * 512 + kt * 128:hp * 512 + (kt + 1) * 128],
                            qtt[o:o + 64, hp * 512:(hp + 1) * 512],
                            start=True, stop=True)
                        pss.append(ps)
                    for hh in range(2):
                        if (hp, hh, kt // 2) not in stexp:
                            stexp[(hp, hh, kt // 2)] = se_pool.tile(
                                [128, 2 * 512], F8, name="sexp", tag="se")
                        dst = stexp[(hp, hh, kt // 2)]
                        nc.scalar.activation(
                            dst[:, (kt % 2) * 512:(kt % 2 + 1) * 512],
                            pss[hh][:, :], AF.Exp,
                            bias=mcol[:, b * NT + kt:b * NT + kt + 1])

            def s2_av(b, hp):
                """AV for one head-pair (fp8, ones-column denominators) and
                the 1/s reciprocals; normalization is deferred one head-pair
                so the PE never waits on the DVE round-trip."""
                st = ST[b]
                stexp, vset = st["stexp"], st["vset"]
                if "att8" not in st:
                    st["att8"] = [at_pool.tile([128, 2 * 512], F8,
                                               name="att8", tag="at")
                                  for _ in range(KP)]
                    st["av"] = {}
                for hh in range(2):
                    av = p_av.tile([128, 512], F32, name="avps", tag="av")
                    for pp in range(2):
                        nc.tensor.matmul(
                            av[:, :], vset[pp][:, (2 * hp + hh) * 256:
                                               (2 * hp + hh + 1) * 256],
                            stexp[(hp, hh, pp)].rearrange(
                                "r (i t) -> r i t", i=2),
                            perf_mode=DRS, start=(pp == 0), stop=(pp == 1))
                    rs = rs_pool.tile([65, 512], BF16, name="rst", tag="rs")
                    with nc.allow_low_precision(
                            reason="bf16 1/s for broadcast (0.4% rel)"):
                        nc.vector.reciprocal(rs[64:65, :], av[64:65, :])
                    st["av"][(hp, hh)] = (av, rs)

            def s2_norm(b, hp):
                """Broadcast 1/s (rank-1 matmul) and scale the AV output into
                the fp8 ATT pair tiles."""
                st = ST[b]
                att8 = st["att8"]
                for hh in range(2):
                    av, rs = st["av"].pop((hp, hh))
                    rbc = p_mm.tile([64, 512], F32, name="rbc", tag="mm")
                    nc.tensor.matmul(rbc[:, :], ones65[64:65, 0:64],
                                     rs[64:65, :], start=True, stop=True)
                    rbs = rb_pool.tile([64, 512], BF16, name="rbs", tag="rb")
                    nc.vector.tensor_copy(rbs[:, :], rbc[:, :])
                    kp, half = hp // 2, hp % 2
                    if hh == 0:
                        nc.vector.scalar_tensor_tensor(
                            att8[kp][0:64, half * 512:(half + 1) * 512],
                            av[0:64, :], 1.0, rbs[:, :],
                            op0=OP.mult, op1=OP.mult)
                    else:
                        stg = sg_pool.tile([64, 512], F8, name="stg", tag="sg")
                        nc.vector.scalar_tensor_tensor(
                            stg[:, :], av[0:64, :], 1.0, rbs[:, :],
                            op0=OP.mult, op1=OP.mult)
                        nc.gpsimd.dma_start(
                            att8[kp][64:128, half * 512:(half + 1) * 512],
                            stg[:, :])

            def s2_o(b):
                """O-projection (fp8) + residual -> xa (feature-major bf16)."""
                st = ST[b]
                att8, xtb = st["att8"], st["xtb"]
                xa = xa_pool.tile([128, KT_E * 512], BF16, name="xat", tag="xa")
                for et in range(KT_E):
                    ps = p_mm.tile([128, 512], F32, name="ops", tag="mm")
                    for kp in range(KP):
                        nc.tensor.matmul(
                            ps[:, :], WO8[:, kp, et, :],
                            att8[kp].rearrange("r (i t) -> r i t", i=2),
                            perf_mode=DRS,
                            start=(kp == 0), stop=(kp == KP - 1 and not use_bo))
                    if use_bo:
                        nc.tensor.matmul(
                            ps[:, :], brow[3:4, et * 128:(et + 1) * 128],
                            onesr[0:1, 0:S], start=False, stop=True)
                    nc.vector.scalar_tensor_tensor(
                        xa[:, et * 512:(et + 1) * 512], ps[:, :], 1.0,
                        xtb[:, et * 512:(et + 1) * 512],
                        op0=OP.mult, op1=OP.add)
                nc.gpsimd.dma_start(d_xa[b * 128:(b + 1) * 128, :],
                                    xa[:, :])
                del ST[b]

            # software-pipelined emission: each head-pair iteration carries
            # the previous batch's AV, the one-earlier head-pair's softmax
            # normalization (so the PE never waits on the reciprocal
            # round-trip), a chunk of the next batch's Q/K projection, and the
            # current batch's scores. V/O projections sit at block boundaries;
            # phase-B weights stream in stages on the Activation DMA queue.
            s1_dma(0)
            for st2 in range(2):
                for pp in range(2):
                    v4 = VAUG[st2][pp].rearrange("r (h c) -> r h c", h=H)
                    nc.gpsimd.memset(v4[:, :, 0:126], 0.0)
                    nc.gpsimd.memset(v4[:, :, 126:128], 1.0)
            for j in range(HP):
                s1qk_chunk(0, j)
            s1v(0)
            s1_dma(1)
            for hp in range(HP):
                s1qk_chunk(1, hp)
                sc_hp(0, hp)
            s1v(1)
            load_b_weights(0)
            for bn in (1, 2):
                s1_dma(bn + 1)
                for hp in range(HP):
                    s2_av(bn - 1, hp)
                    if hp > 0:
                        s2_norm(bn - 1, hp - 1)
                    s1qk_chunk(bn + 1, hp)
                    sc_hp(bn, hp)
                s2_norm(bn - 1, HP - 1)
                s2_o(bn - 1)
                s1v(bn + 1)
                load_b_weights(bn)
            for hp in range(HP):
                s2_av(2, hp)
                if hp > 0:
                    s2_norm(2, hp - 1)
                sc_hp(3, hp)
            s2_norm(2, HP - 1)
            s2_o(2)
            load_b_weights(3)
            for hp in range(HP):
                s2_av(3, hp)
                if hp > 0:
                    s2_norm(3, hp - 1)
            s2_norm(3, HP - 1)
            s2_o(3)

        # ============ superphase B: SelfOutput LN, FFN, LN ===================
        with ExitStack() as sb:
            h_pool = sb.enter_context(tc.tile_pool(name="h", bufs=NT + 1))
            ht_pool = sb.enter_context(tc.tile_pool(name="ht", bufs=2))
            fft_pool = sb.enter_context(tc.tile_pool(name="fft", bufs=FT + 2))
            sq_pool = sb.enter_context(tc.tile_pool(name="sq", bufs=2))
            rs_pool = sb.enter_context(tc.tile_pool(name="rsd", bufs=3))
            out_pool = sb.enter_context(tc.tile_pool(name="outp", bufs=2))
            t_pool = sb.enter_context(tc.tile_pool(name="sb_s", bufs=12))

            def layernorm(chunks, h_dst, gcol, use_g, use_bb, resid=None):
                """chunks: [(psum_ap, col0, n)]; h_dst: [128, E] out.
                Drains PSUM chunks to SBUF immediately (rtile) so the banks
                free early; stats then run from SBUF."""
                if resid is not None:
                    rtile = rs_pool.tile([128, E], F32, name="rt", tag="rsd")
                    for (ps, c0, n), rext in zip(chunks, resid):
                        nc.vector.scalar_tensor_tensor(
                            rtile[:, c0:c0 + n], ps, 1.0, rext,
                            op0=OP.mult, op1=OP.add)
                    srcs = [(rtile[:, c0:c0 + n], c0, n)
                            for (_, c0, n) in chunks]
                else:
                    srcs = chunks
                s1t = t_pool.tile([128, 1], F32, name="s1", tag="s1")
                s1b = t_pool.tile([128, 1], F32, name="s1b", tag="s1b")
                nc.vector.reduce_sum(s1t[:, :], srcs[0][0], axis=AX.X)
                nc.vector.reduce_sum(s1b[:, :], srcs[1][0], axis=AX.X)
                mu_n = t_pool.tile([128, 1], F32, name="mun", tag="mun")
                tmp = t_pool.tile([128, 1], F32, name="tmps", tag="tmps")
                nc.vector.scalar_tensor_tensor(
                    tmp[:, :], s1t[:, :], 1.0, s1b[:, :], op0=OP.mult, op1=OP.add)
                nc.vector.tensor_scalar_mul(mu_n[:, :], tmp[:, :], -1.0 / E)
                ss = t_pool.tile([128, 1], F32, name="ssa", tag="ssa", bufs=34)
                ssb = t_pool.tile([128, 1], F32, name="ssb", tag="ssb", bufs=34)
                for (src, c0, n), acc in zip(srcs, (ss, ssb)):
                    sq = sq_pool.tile([128, 512], BF16, name="sqt", tag="sq")
                    nc.scalar.activation(sq[:, :n], src, AF.Square,
                                         accum_out=acc[:, :])
                musq = t_pool.tile([128, 1], F32, name="musq", tag="musq")
                nc.vector.scalar_tensor_tensor(
                    musq[:, :], mu_n[:, :], 1.0, mu_n[:, :],
                    op0=OP.mult, op1=OP.mult)
                veps = t_pool.tile([128, 1], F32, name="veps", tag="veps")
                nc.vector.scalar_tensor_tensor(
                    veps[:, :], ss[:, :], 1.0, ssb[:, :],
                    op0=OP.mult, op1=OP.add)
                veps2 = t_pool.tile([128, 1], F32, name="veps2", tag="veps2")
                nc.vector.tensor_scalar(
                    veps2[:, :], veps[:, :], 1.0 / E, EPS,
                    op0=OP.mult, op1=OP.add)
                veps3 = t_pool.tile([128, 1], F32, name="veps3", tag="veps3")
                nc.vector.scalar_tensor_tensor(
                    veps3[:, :], musq[:, :], -1.0, veps2[:, :],
                    op0=OP.mult, op1=OP.add)
                sd = t_pool.tile([128, 1], F32, name="sd", tag="sd")
                nc.scalar.sqrt(sd[:, :], veps3[:, :])
                rstd = t_pool.tile([128, 1], F32, name="rstd", tag="rstd")
                nc.vector.reciprocal(rstd[:, :], sd[:, :])
                for (src, c0, n) in srcs:
                    nc.vector.tensor_scalar(
                        h_dst[:, c0:c0 + n], src, mu_n[:, :], rstd[:, :],
                        op0=OP.add, op1=OP.mult)
                if use_g:
                    nc.vector.scalar_tensor_tensor(
                        h_dst[:, :], h_dst[:, :], 1.0,
                        gb[:, gcol * E:(gcol + 1) * E], op0=OP.mult, op1=OP.mult)
                if use_bb:
                    nc.vector.scalar_tensor_tensor(
                        h_dst[:, :], h_dst[:, :], 1.0,
                        gb[:, (gcol + 2) * E:(gcol + 3) * E],
                        op0=OP.mult, op1=OP.add)

            XAB = {}

            def load_xa(b):
                xab = xa_pool.tile([128, KT_E * 512], BF16, name="xab",
                                   tag="xa")
                nc.gpsimd.dma_start(xab[:, :],
                                    d_xa[b * 128:(b + 1) * 128, :])
                XAB[b] = xab

            load_xa(0)
            for b in range(BL):
                t0 = b * S
                if b + 1 < BL:
                    load_xa(b + 1)
                xa = XAB.pop(b).rearrange("r (e t) -> r e t", e=KT_E)

                # ---- SelfOutput GEMM + LN1 -> h (token-major), hT ----
                hh_t = [None] * NT
                hT = ht_pool.tile([128, KT_E * S], BF16, name="htt", tag="ht")

                def emit_htrans(tt):
                    tps = [p_mm.tile([128, 512], BF16, name="htp", tag="mm")
                           for _ in range(2)]
                    for et in range(KT_E):
                        sl = tps[et // 4][:, (et % 4) * 128:(et % 4 + 1) * 128]
                        nc.tensor.transpose(
                            sl, hh_t[tt][:, et * 128:(et + 1) * 128],
                            ident[:, :])
                    for et in range(KT_E):
                        sl = tps[et // 4][:, (et % 4) * 128:(et % 4 + 1) * 128]
                        nc.vector.tensor_copy(
                            hT[:, et * S + tt * 128:et * S + (tt + 1) * 128], sl)

                for tt in range(NT):
                    ch = []
                    for ec, n in ((0, 512), (512, 256)):
                        ps = (p_mm.tile([128, 512], F32, name="sops", tag="mm")
                              if n == 512 else
                              p_mm.tile([128, 256], F32, name="sops2", tag="mm"))
                        for k in range(KT_E):
                            nc.tensor.matmul(
                                ps[:, :n], xa[:, k, tt * 128:(tt + 1) * 128],
                                WSO[k][:, ec:ec + n],
                                start=(k == 0),
                                stop=(k == KT_E - 1 and not use_bso))
                        if use_bso:
                            nc.tensor.matmul(
                                ps[:, :n], onesr[0:1, 0:128],
                                brow[4:5, ec:ec + n], start=False, stop=True)
                        ch.append((ps[:, :n], ec, n))
                    hh_t[tt] = h_pool.tile([128, E], BF16, name="hht", tag="h")
                    layernorm(ch, hh_t[tt], 0, use_g1, use_b1)
                    if tt > 0:
                        emit_htrans(tt - 1)
                emit_htrans(NT - 1)

                # ---- FFN + LN2 ----
                ffT = [None] * FT
                for ft in range(FT):
                    ps = p_mm.tile([128, 512], F32, name="fips", tag="mm")
                    for k in range(KT_E):
                        nc.tensor.matmul(
                            ps[:, :], WI[k][:, ft * 128:(ft + 1) * 128],
                            hT[:, k * S:k * S + 512],
                            start=(k == 0), stop=(k == KT_E - 1))
                    ffT[ft] = fft_pool.tile([128, 512], BF16, name="fftt",
                                            tag="fft")
                    if use_bi:
                        nc.scalar.activation(ffT[ft][:, :], ps[:, :], AF.Gelu,
                                             bias=bic[:, ft:ft + 1])
                    else:
                        nc.scalar.activation(ffT[ft][:, :], ps[:, :], AF.Gelu)
                for tt in range(NT):
                    ch = []
                    for ec, n in ((0, 512), (512, 256)):
                        ps = (p_mm.tile([128, 512], F32, name="wops", tag="mm")
                              if n == 512 else
                              p_mm.tile([128, 256], F32, name="wops2", tag="mm"))
                        for f in range(FT):
                            nc.tensor.matmul(
                                ps[:, :n],
                                ffT[f][:, tt * 128:(tt + 1) * 128],
                                WOUT[f][:, ec:ec + n],
                                start=(f == 0),
                                stop=(f == FT - 1 and not use_bout))
                        if use_bout:
                            nc.tensor.matmul(
                                ps[:, :n], onesr[0:1, 0:128],
                                brow[5:6, ec:ec + n], start=False, stop=True)
                        ch.append((ps[:, :n], ec, n))
                    otile = out_pool.tile([128, E], F32, name="ot", tag="outp")
                    resid = [hh_t[tt][:, ec:ec + n] for (_, ec, n) in ch]
                    layernorm(ch, otile, 1, use_g2, use_b2, resid=resid)
                    nc.gpsimd.dma_start(
                        d_out[t0 + tt * 128:t0 + (tt + 1) * 128, :],
                        otile[:, :])
    nc.compile()
    return nc


def _get_program(flags):
    key = ("prog", flags)
    if key not in _CACHE:
        _CACHE[key] = _build(flags)
    return _CACHE[key]


def kernel(x, mask, Wq, bq, Wk, bk, Wv, bv, Wo, bo,
           Wso, bso, gso, beso, Wi, bi, Wout, bout, gout, beout):
    from concourse.bass_utils import run_bass_kernel_spmd

    x = np.asarray(x, np.float32)
    mask = np.asarray(mask)
    sc = 1.0 / float(np.sqrt(np.float32(DK)))

    z = lambda a: not np.any(np.asarray(a))
    one = lambda a: bool(np.all(np.asarray(a) == 1.0))
    flags = (not z(bq), not z(bk), not z(bv), not z(bo), not z(bso),
             not z(bi), not z(bout),
             not one(gso), not z(beso), not one(gout), not z(beout))
    nc = _get_program(flags)

    wq8 = _pack_sw(np.asarray(Wq, np.float32) * sc)
    wk8, wo8 = _pack_sw(Wk), _pack_sw(Wo)
    # reverse V's 64 columns within each head so the on-device interleaved
    # write of the AV stationary is an ascending stride-2 copy
    wv_re = np.asarray(Wv, np.float32).reshape(E, H, DK)[:, :, ::-1].reshape(E, E)
    wv8 = _pack_blk(wv_re)
    wso_b, wi_b, wout_b = _bf(Wso), _bf(Wi), _bf(Wout)
    identb = _bf(np.eye(128))
    onesr = _bf(np.ones((1, 512)))

    brow = np.zeros((7, FF), np.float32)
    brow[0, :E] = np.asarray(bq, np.float32) * sc
    for i, v in enumerate((bk, bv, bo, bso, bout)):
        brow[i + 1, :E] = v
    brow[6, :] = bi
    brow = _bf(brow)
    bicol = np.asarray(bi, np.float32).reshape(FT, 128).T.copy()
    gbt = np.zeros((128, 4 * E), np.float32)
    for i, g in enumerate((gso, gout, beso, beout)):
        gbt[:, i * E:(i + 1) * E] = np.broadcast_to(
            np.asarray(g, np.float32).reshape(1, E), (128, E))

    in_maps = []
    for c in range(NCORES):
        xs = x[c * BL:(c + 1) * BL]            # [BL, S, E]
        ms = np.asarray(mask[c * BL:(c + 1) * BL]).reshape(BL, S)
        # mcol[r, b*NT + kt] = bias for key token kt*128 + r of batch b
        mb = np.where(ms == 0, np.float32(MASK_NEG), np.float32(0.0))
        mcol = np.ascontiguousarray(
            mb.reshape(BL, NT, 128).transpose(2, 0, 1).reshape(128, BL * NT))
        # x^T per batch in the three on-device layouts
        xtb = np.empty((BL * 128, KT_E * 512), np.float32)
        xq8 = np.empty((BL * 128, KP * 2 * 512), ml_dtypes.float8_e4m3)
        xv8 = np.empty((BL * 128, KP * NT * 256), ml_dtypes.float8_e4m3)
        for b in range(BL):
            xt = np.ascontiguousarray(xs[b].T)               # [E, S]
            xtb[b * 128:(b + 1) * 128] = xt.reshape(
                KT_E, 128, S).transpose(1, 0, 2).reshape(128, KT_E * S)
            xq8[b * 128:(b + 1) * 128] = _pack_blk(xt)
            xv8[b * 128:(b + 1) * 128] = _pack_sw(xt)
        in_maps.append({
            "ones65": _bf(np.ones((65, 128))),
            "xtb": _bf(xtb), "xq8": xq8, "xv8": xv8,
            "wq8": wq8, "wk8": wk8, "wv8": wv8, "wo8": wo8,
            "wso": wso_b, "wi": wi_b, "wout": wout_b, "mcol": mcol,
            "ident": identb,
            "onesrow": onesr, "brow": brow, "bicol": bicol, "gb": gbt,
        })

    trace = os.environ.get("KERNEL_TRACE", "0") == "1"
    res = run_bass_kernel_spmd(nc, in_maps, core_ids=list(range(NCORES)),
                               trace=trace)
    if trace and res.exec_time_ns is not None:
        print(f"HW exec time: {res.exec_time_ns} ns")
        if res.instructions_and_trace is not None:
            print(f"trace: {res.instructions_and_trace[1]}")
    out = np.concatenate([r["out"].reshape(BL, S, E) for r in res.results],
                         axis=0)
    return np.ascontiguousarray(out.astype(np.float32))


# revision 23
# speedup vs baseline: 1.0900x; 1.0259x over previous
"""BERT-base encoder layer on 8 Trainium2 NeuronCores (Bass/Tile).

Sharding: data-parallel over batch. Full inputs [32, 512, 768] split into 8
shards of 4 batches (2048 tokens); every core runs the same NEFF on its shard
(SPMD, no collectives); host concatenates the outputs.

Attention is computed k-major: scores are built transposed (ST[k, q] = K·Q^T)
so that softmax probabilities come out already in the layout the P·V matmul
needs — no PE transpose of P, and the key mask becomes a per-partition bias
on the exp activation (free) instead of rank-1 matmuls. The softmax
denominator comes from a ones-column appended to V (row 64 of the AV PSUM);
normalization is a rank-1 broadcast matmul + one vector multiply.

QKV/V/AV/O-projection GEMMs run in fp8(e4m3) DoubleRow mode (2 contraction
rows per PE pass); Wso/Wi/Wout GEMMs stay bf16 for accuracy. PSUM accumulation
is fp32 everywhere; layernorm statistics fp32.
"""

import os
import numpy as np
import ml_dtypes

B, S, E, H, DK, FF = 32, 512, 768, 12, 64, 3072
NCORES = 8
BL = B // NCORES          # batches per core = 4
T = BL * S                # tokens per core = 2048
EPS = 1e-12
MASK_NEG = -87.0          # exp(-87) == 0 in fp8/bf16
KT_E = E // 128           # 6 feature blocks
KP = KT_E // 2            # 3 fp8 contraction pairs
NT = S // 128             # 4 token tiles
FT = FF // 128            # 24
HP = H // 2               # 6 head pairs

_CACHE = {}


def _bf(a):
    return np.ascontiguousarray(np.asarray(a, np.float32).astype(ml_dtypes.bfloat16))


def _f8(a):
    a = np.clip(np.asarray(a, np.float32), -240.0, 240.0)
    return np.ascontiguousarray(a.astype(ml_dtypes.float8_e4m3))


def _pack_blk(w):
    """Moving-operand block format: [K, N] -> [128, (K//256)*2*N] fp8; slice p
    gives [128, 2, N] with element [r, i, m] = w[256p + 128i + r, m]."""
    K, N = w.shape
    p = K // 256
    arr = np.asarray(w, np.float32).reshape(p, 2, 128, N).transpose(2, 0, 1, 3)
    return _f8(arr.reshape(128, p * 2 * N))


def _pack_sw(w):
    """Stationary sw-interleave format for dual-fp8 LDWEIGHTS: [K, N] ->
    [128, (K//256)*(N//128)*256]; block (p, nb) holds column m of k-pair i at
    position 2*(127-m)+i."""
    K, N = w.shape
    P, NB = K // 256, N // 128
    a = np.asarray(w, np.float32).reshape(P, 2, 128, NB, 128)
    a = a.transpose(2, 0, 3, 4, 1)[:, :, :, ::-1, :]     # [r, p, nb, m_rev, i]
    return _f8(a.reshape(128, P * NB * 256))


def _build(flags):
    import concourse.bass as bass
    import concourse.bacc as bacc
    import concourse.mybir as mybir
    import concourse.tile as tile
    from contextlib import ExitStack

    (use_bq, use_bk, use_bv, use_bo, use_bso, use_bi, use_bout,
     use_g1, use_b1, use_g2, use_b2) = flags

    AF = mybir.ActivationFunctionType
    OP = mybir.AluOpType
    AX = mybir.AxisListType
    BF16 = mybir.dt.bfloat16
    F32 = mybir.dt.float32
    F8 = mybir.dt.float8e4
    DRS = mybir.MatmulPerfMode.DoubleRowSwInterleave
    ACT_E = mybir.EngineType.Activation

    nc = bacc.Bacc("TRN2", target_bir_lowering=False)

    # x^T per batch in three layouts (transposed/packed on host):
    # bf16 feature-major (residual), fp8 moving blocks (Q/K), fp8 interleaved
    # stationary (V)
    d_xtb = nc.dram_tensor("xtb", (BL * 128, KT_E * 512), BF16,
                           kind="ExternalInput")
    d_xq8 = nc.dram_tensor("xq8", (BL * 128, KP * 2 * 512), F8,
                           kind="ExternalInput")
    d_xv8 = nc.dram_tensor("xv8", (BL * 128, KP * NT * 256), F8,
                           kind="ExternalInput")
    d_wq8 = nc.dram_tensor("wq8", (128, KP * KT_E * 256), F8, kind="ExternalInput")
    d_wk8 = nc.dram_tensor("wk8", (128, KP * KT_E * 256), F8, kind="ExternalInput")
    d_wv8 = nc.dram_tensor("wv8", (128, KP * 2 * E), F8, kind="ExternalInput")
    d_wo8 = nc.dram_tensor("wo8", (128, KP * KT_E * 256), F8, kind="ExternalInput")
    d_wso = nc.dram_tensor("wso", (E, E), BF16, kind="ExternalInput")
    d_wi = nc.dram_tensor("wi", (E, FF), BF16, kind="ExternalInput")
    d_wout = nc.dram_tensor("wout", (FF, E), BF16, kind="ExternalInput")
    d_mcol = nc.dram_tensor("mcol", (128, BL * NT), F32, kind="ExternalInput")
    d_ones65 = nc.dram_tensor("ones65", (65, 128), BF16, kind="ExternalInput")
    d_id = nc.dram_tensor("ident", (128, 128), BF16, kind="ExternalInput")
    d_onesr = nc.dram_tensor("onesrow", (1, 512), BF16, kind="ExternalInput")
    # bias rows: 0=bq/8, 1=bk, 2=bv, 3=bo, 4=bso, 5=bout, 6=bi (full FF width)
    d_brow = nc.dram_tensor("brow", (7, FF), BF16, kind="ExternalInput")
    d_bic = nc.dram_tensor("bicol", (128, FT), F32, kind="ExternalInput")
    # gamma1 | gamma2 | beta1 | beta2, each [128, 768] partition-broadcast
    d_gb = nc.dram_tensor("gb", (128, 4 * E), F32, kind="ExternalInput")
    d_out = nc.dram_tensor("out", (T, E), F32, kind="ExternalOutput")
    # xa (x + att@Wo, feature-major bf16) spills to DRAM between superphases
    d_xa = nc.dram_tensor("xasp", (BL * 128, KT_E * 512), BF16, kind="Internal")

    need_gb = use_g1 or use_b1 or use_g2 or use_b2
    need_brow = use_bq or use_bk or use_bv or use_bo or use_bso or use_bout

    with ExitStack() as ctx:
        tc = ctx.enter_context(tile.TileContext(nc))

        p_mm = ctx.enter_context(tc.tile_pool(name="p_mm", bufs=6, space="PSUM"))
        p_av = ctx.enter_context(tc.tile_pool(name="p_av", bufs=2, space="PSUM"))

        c_pool = ctx.enter_context(tc.tile_pool(name="consts", bufs=1))
        wa_pool = ctx.enter_context(tc.tile_pool(name="wa", bufs=1))
        wso_pool = ctx.enter_context(tc.tile_pool(name="wso", bufs=KT_E))
        wi_pool = ctx.enter_context(tc.tile_pool(name="wi", bufs=KT_E))
        wout_pool = ctx.enter_context(tc.tile_pool(name="wout", bufs=FT))
        xa_pool = ctx.enter_context(tc.tile_pool(name="xa", bufs=2))

        ident = c_pool.tile_from(d_id[:, :], name="ident")
        mcol = c_pool.tile_from(d_mcol[:, :], name="mcol")
        ones65 = c_pool.tile_from(d_ones65[:, :], name="ones65")
        onesr = c_pool.tile_from(d_onesr[:, :], name="onesr") \
            if (use_bv or use_bso or use_bout or use_bq or use_bk or use_bo) else None
        brow = c_pool.tile_from(d_brow[:, :], name="brow") if need_brow else None
        gb = c_pool.tile_from(d_gb[:, :], name="gb") if need_gb else None
        bic = c_pool.tile_from(d_bic[:, :], name="bic") if use_bi else None

        # phase-A weights (fp8, small): default (SP) DMA queue
        wq8 = wa_pool.tile_from(d_wq8[:, :], name="wq8t")
        wk8 = wa_pool.tile_from(d_wk8[:, :], name="wk8t")
        wv8 = wa_pool.tile_from(d_wv8[:, :], name="wv8t")
        wo8 = wa_pool.tile_from(d_wo8[:, :], name="wo8t")

        # stationary (sw-interleaved) weights: slice (p, et) -> [128, 256]
        WQ8 = wq8.rearrange("r (p e c) -> r p e c", p=KP, e=KT_E)
        WK8 = wk8.rearrange("r (p e c) -> r p e c", p=KP, e=KT_E)
        WO8 = wo8.rearrange("r (p e c) -> r p e c", p=KP, e=KT_E)
        # moving (block) V weights: slice p -> [128, 2, E]
        WV8 = wv8.rearrange("r (p i m) -> r p i m", p=KP, i=2)

        # phase-B weights (bf16, 10.6MB) stream on the Activation DMA queue,
        # staggered through phase A so they never compete with critical loads
        WSO, WI, WOUT = [], [], []

        def load_b_weights(stage):
            if stage == 0:
                WSO.extend(wso_pool.tile_from(
                    d_wso[k * 128:(k + 1) * 128, :], name="wsot",
                    forced_dma_engine=ACT_E) for k in range(KT_E))
            elif stage == 1:
                WI.extend(wi_pool.tile_from(
                    d_wi[k * 128:(k + 1) * 128, :], name="wit",
                    forced_dma_engine=ACT_E) for k in range(KT_E))
            else:
                f0 = 0 if stage == 2 else FT // 2
                f1 = FT // 2 if stage == 2 else FT
                WOUT.extend(wout_pool.tile_from(
                    d_wout[f * 128:(f + 1) * 128, :], name="woutt",
                    forced_dma_engine=ACT_E) for f in range(f0, f1))

        # ================= superphase A: QKV, attention, O-proj ==============
        with ExitStack() as sa:
            xtb_pool = sa.enter_context(tc.tile_pool(name="xtb", bufs=2))
            xq8_pool = sa.enter_context(tc.tile_pool(name="xq8", bufs=1))
            xv8_pool = sa.enter_context(tc.tile_pool(name="xv8", bufs=1))
            qt_pool = sa.enter_context(tc.tile_pool(name="qt", bufs=2))
            kt_pool = sa.enter_context(tc.tile_pool(name="kt", bufs=2))
            va_pool = sa.enter_context(tc.tile_pool(name="va", bufs=1))
            se_pool = sa.enter_context(tc.tile_pool(name="se", bufs=24))
            at_pool = sa.enter_context(tc.tile_pool(name="at", bufs=4))
            sg_pool = sa.enter_context(tc.tile_pool(name="sg", bufs=2))
            rs_pool = sa.enter_context(tc.tile_pool(name="rs", bufs=3))
            rb_pool = sa.enter_context(tc.tile_pool(name="rb", bufs=3))

            # persistent V tiles (2 sets x 2 token-pair tiles); zero/ones
            # regions preset once (after the first x DMAs so they don't block
            # the gpsimd queue at startup)
            VAUG = [[va_pool.tile([128, H * 256], F8, name="vaug", tag="va",
                                  bufs=4) for _ in range(2)] for _ in range(2)]

            ST = {}   # per-batch state

            def s1_dma(b):
                """x^T loads (pre-transposed/packed on host)."""
                st = {}
                xtb = xtb_pool.tile([128, KT_E * 512], BF16, name="xtb",
                                    tag="xtb")
                xq8 = xq8_pool.tile([128, KP * 2 * 512], F8, name="xq8",
                                    tag="xq8")
                xv8 = xv8_pool.tile([128, KP * NT * 256], F8, name="xv8",
                                    tag="xv8")
                nc.gpsimd.dma_start(xq8[:, :], d_xq8[b * 128:(b + 1) * 128, :])
                nc.gpsimd.dma_start(xv8[:, :], d_xv8[b * 128:(b + 1) * 128, :])
                nc.gpsimd.dma_start(xtb[:, :], d_xtb[b * 128:(b + 1) * 128, :])
                st["xtb"], st["xq8"], st["xv8"] = xtb, xq8, xv8
                st["qt"] = qt_pool.tile([128, HP * 512], BF16, name="qtt",
                                        tag="qt")
                st["kt"] = kt_pool.tile([128, HP * 512], BF16, name="ktt",
                                        tag="kt")
                ST[b] = st

            def s1qk_chunk(b, j):
                """Two feature-blocks of the Q (j<3) or K (j>=3) projection."""
                st = ST[b]
                xq8_p = st["xq8"].rearrange("r (p i t) -> r p i t", p=KP, i=2)
                W8, dst, ub, brx = ((WQ8, st["qt"], use_bq, 0) if j < 3 else
                                    (WK8, st["kt"], use_bk, 1))
                for et in (2 * (j % 3), 2 * (j % 3) + 1):
                    ps = p_mm.tile([128, 512], F32, name="qkps", tag="mm")
                    for p in range(KP):
                        nc.tensor.matmul(
                            ps[:, :], W8[:, p, et, :], xq8_p[:, p, :, :],
                            perf_mode=DRS,
                            start=(p == 0), stop=(p == KP - 1 and not ub))
                    if ub:
                        nc.tensor.matmul(
                            ps[:, :],
                            brow[brx:brx + 1, et * 128:(et + 1) * 128],
                            onesr[0:1, 0:S], start=False, stop=True)
                    nc.vector.tensor_copy(dst[:, et * 512:(et + 1) * 512],
                                          ps[:, :])

            def s1v(b):
                """V projection (token-major; per-head columns reversed on the
                host so the interleaved write is an ascending stride-2 copy)."""
                st = ST[b]
                xv8_p = st["xv8"].rearrange("r (p t c) -> r p t c", p=KP, t=NT)
                vset = VAUG[b % 2]
                for tt in range(NT):
                    for ec, n in ((0, 512), (512, 256)):
                        ps = (p_mm.tile([128, 512], F32, name="vps", tag="mm")
                              if n == 512 else
                              p_mm.tile([128, 256], F32, name="vps2", tag="mm"))
                        for p in range(KP):
                            nc.tensor.matmul(
                                ps[:, :n], xv8_p[:, p, tt, :],
                                WV8[:, p, :, ec:ec + n], perf_mode=DRS,
                                start=(p == 0), stop=(p == KP - 1 and not use_bv))
                        if use_bv:
                            nc.tensor.matmul(
                                ps[:, :n], onesr[0:1, 0:128],
                                brow[2:3, ec:ec + n], start=False, stop=True)
                        h0, nh = ec // 64, n // 64
                        v6 = vset[tt // 2].rearrange(
                            "r (h a c i) -> r h a c i", h=H, a=2, c=64)
                        nc.scalar.activation(
                            v6[:, h0:h0 + nh, 1, :, tt % 2],
                            ps[:, :n].rearrange("r (h c) -> r h c", h=nh),
                            AF.Copy)
                st["vset"] = vset

            def sc_hp(b, hp):
                """scores for one head-pair (k-major, row-group dual-issue)
                + masked exp."""
                st = ST[b]
                qtt, ktt = st["qt"], st["kt"]
                stexp = st.setdefault("stexp", {})
                for kt in range(NT):
                    pss = []
                    for hh in range(2):
                        o = hh * 64
                        ps = p_mm.tile([128, 512], F32, name="scps", tag="mm")
                        nc.tensor.matmul(
                            ps[:, :],
                            ktt[o:o + 64,
                                hp * 512 + kt * 128:hp * 512 + (kt + 1) * 128],
                            qtt[o:o + 64, hp * 512:(hp + 1) * 512],
                            start=True, stop=True)
                        pss.append(ps)
                    for hh in range(2):
                        if (hp, hh, kt // 2) not in stexp:
                            stexp[(hp, hh, kt // 2)] = se_pool.tile(
                                [128, 2 * 512], F8, name="sexp", tag="se")
                        dst = stexp[(hp, hh, kt // 2)]
                        nc.scalar.activation(
                            dst[:, (kt % 2) * 512:(kt % 2 + 1) * 512],
                            pss[hh][:, :], AF.Exp,
                            bias=mcol[:, b * NT + kt:b * NT + kt + 1])

            def s2_av(b, hp):
                """AV for one head-pair (fp8, ones-column denominators) and
                the 1/s reciprocals; normalization is deferred one head-pair
                so the PE never waits on the DVE round-trip."""
                st = ST[b]
                stexp, vset = st["stexp"], st["vset"]
                if "att8" not in st:
                    st["att8"] = [at_pool.tile([128, 2 * 512], F8,
                                               name="att8", tag="at")
                                  for _ in range(KP)]
                    st["av"] = {}
                for hh in range(2):
                    av = p_av.tile([128, 512], F32, name="avps", tag="av")
                    for pp in range(2):
                        nc.tensor.matmul(
                            av[:, :], vset[pp][:, (2 * hp + hh) * 256:
                                               (2 * hp + hh + 1) * 256],
                            stexp[(hp, hh, pp)].rearrange(
                                "r (i t) -> r i t", i=2),
                            perf_mode=DRS, start=(pp == 0), stop=(pp == 1))
                    rs = rs_pool.tile([65, 512], BF16, name="rst", tag="rs")
                    with nc.allow_low_precision(
                            reason="bf16 1/s for broadcast (0.4% rel)"):
                        nc.vector.reciprocal(rs[64:65, :], av[64:65, :])
                    st["av"][(hp, hh)] = (av, rs)

            def s2_norm(b, hp):
                """Broadcast 1/s (rank-1 matmul) and scale the AV output into
                the fp8 ATT pair tiles."""
                st = ST[b]
                att8 = st["att8"]
                for hh in range(2):
                    av, rs = st["av"].pop((hp, hh))
                    rbc = p_mm.tile([64, 512], F32, name="rbc", tag="mm")
                    nc.tensor.matmul(rbc[:, :], ones65[64:65, 0:64],
                                     rs[64:65, :], start=True, stop=True)
                    rbs = rb_pool.tile([64, 512], BF16, name="rbs", tag="rb")
                    nc.scalar.activation(rbs[:, :], rbc[:, :], AF.Copy)
                    kp, half = hp // 2, hp % 2
                    if hh == 0:
                        nc.vector.scalar_tensor_tensor(
                            att8[kp][0:64, half * 512:(half + 1) * 512],
                            av[0:64, :], 1.0, rbs[:, :],
                            op0=OP.mult, op1=OP.mult)
                    else:
                        stg = sg_pool.tile([64, 512], F8, name="stg", tag="sg")
                        nc.vector.scalar_tensor_tensor(
                            stg[:, :], av[0:64, :], 1.0, rbs[:, :],
                            op0=OP.mult, op1=OP.mult)
                        nc.gpsimd.dma_start(
                            att8[kp][64:128, half * 512:(half + 1) * 512],
                            stg[:, :])

            def s2_o(b):
                """O-projection (fp8) + residual -> xa (feature-major bf16)."""
                st = ST[b]
                att8, xtb = st["att8"], st["xtb"]
                xa = xa_pool.tile([128, KT_E * 512], BF16, name="xat", tag="xa")
                for et in range(KT_E):
                    ps = p_mm.tile([128, 512], F32, name="ops", tag="mm")
                    for kp in range(KP):
                        nc.tensor.matmul(
                            ps[:, :], WO8[:, kp, et, :],
                            att8[kp].rearrange("r (i t) -> r i t", i=2),
                            perf_mode=DRS,
                            start=(kp == 0), stop=(kp == KP - 1 and not use_bo))
                    if use_bo:
                        nc.tensor.matmul(
                            ps[:, :], brow[3:4, et * 128:(et + 1) * 128],
                            onesr[0:1, 0:S], start=False, stop=True)
                    nc.vector.scalar_tensor_tensor(
                        xa[:, et * 512:(et + 1) * 512], ps[:, :], 1.0,
                        xtb[:, et * 512:(et + 1) * 512],
                        op0=OP.mult, op1=OP.add)
                nc.gpsimd.dma_start(d_xa[b * 128:(b + 1) * 128, :],
                                    xa[:, :])
                del ST[b]

            # software-pipelined emission: each head-pair iteration carries
            # the previous batch's AV, the one-earlier head-pair's softmax
            # normalization (so the PE never waits on the reciprocal
            # round-trip), a chunk of the next batch's Q/K projection, and the
            # current batch's scores. V/O projections sit at block boundaries;
            # phase-B weights stream in stages on the Activation DMA queue.
            s1_dma(0)
            for st2 in range(2):
                for pp in range(2):
                    v4 = VAUG[st2][pp].rearrange("r (h c) -> r h c", h=H)
                    nc.gpsimd.memset(v4[:, :, 0:126], 0.0)
                    nc.gpsimd.memset(v4[:, :, 126:128], 1.0)
            for j in range(HP):
                s1qk_chunk(0, j)
            s1v(0)
            s1_dma(1)
            for hp in range(HP):
                s1qk_chunk(1, hp)
                sc_hp(0, hp)
            s1v(1)
            load_b_weights(0)
            for bn in (1, 2):
                s1_dma(bn + 1)
                for hp in range(HP):
                    s2_av(bn - 1, hp)
                    if hp > 0:
                        s2_norm(bn - 1, hp - 1)
                    s1qk_chunk(bn + 1, hp)
                    sc_hp(bn, hp)
                s2_norm(bn - 1, HP - 1)
                s2_o(bn - 1)
                s1v(bn + 1)
                load_b_weights(bn)
            for hp in range(HP):
                s2_av(2, hp)
                if hp > 0:
                    s2_norm(2, hp - 1)
                sc_hp(3, hp)
            s2_norm(2, HP - 1)
            s2_o(2)
            load_b_weights(3)
            for hp in range(HP):
                s2_av(3, hp)
                if hp > 0:
                    s2_norm(3, hp - 1)
            s2_norm(3, HP - 1)
            s2_o(3)

        # ============ superphase B: SelfOutput LN, FFN, LN ===================
        with ExitStack() as sb:
            h_pool = sb.enter_context(tc.tile_pool(name="h", bufs=NT + 1))
            ht_pool = sb.enter_context(tc.tile_pool(name="ht", bufs=2))
            fft_pool = sb.enter_context(tc.tile_pool(name="fft", bufs=FT + 2))
            sq_pool = sb.enter_context(tc.tile_pool(name="sq", bufs=2))
            rs_pool = sb.enter_context(tc.tile_pool(name="rsd", bufs=3))
            out_pool = sb.enter_context(tc.tile_pool(name="outp", bufs=2))
            t_pool = sb.enter_context(tc.tile_pool(name="sb_s", bufs=12))

            def layernorm(chunks, h_dst, gcol, use_g, use_bb, resid=None):
                """chunks: [(psum_ap, col0, n)]; h_dst: [128, E] out.
                Drains PSUM chunks to SBUF immediately (rtile) so the banks
                free early; stats then run from SBUF."""
                if resid is not None:
                    rtile = rs_pool.tile([128, E], F32, name="rt", tag="rsd")
                    for (ps, c0, n), rext in zip(chunks, resid):
                        nc.vector.scalar_tensor_tensor(
                            rtile[:, c0:c0 + n], ps, 1.0, rext,
                            op0=OP.mult, op1=OP.add)
                    srcs = [(rtile[:, c0:c0 + n], c0, n)
                            for (_, c0, n) in chunks]
                else:
                    srcs = chunks
                s1t = t_pool.tile([128, 1], F32, name="s1", tag="s1")
                s1b = t_pool.tile([128, 1], F32, name="s1b", tag="s1b")
                nc.vector.reduce_sum(s1t[:, :], srcs[0][0], axis=AX.X)
                nc.vector.reduce_sum(s1b[:, :], srcs[1][0], axis=AX.X)
                mu_n = t_pool.tile([128, 1], F32, name="mun", tag="mun")
                tmp = t_pool.tile([128, 1], F32, name="tmps", tag="tmps")
                nc.vector.scalar_tensor_tensor(
                    tmp[:, :], s1t[:, :], 1.0, s1b[:, :], op0=OP.mult, op1=OP.add)
                nc.vector.tensor_scalar_mul(mu_n[:, :], tmp[:, :], -1.0 / E)
                ss = t_pool.tile([128, 1], F32, name="ssa", tag="ssa", bufs=34)
                ssb = t_pool.tile([128, 1], F32, name="ssb", tag="ssb", bufs=34)
                for (src, c0, n), acc in zip(srcs, (ss, ssb)):
                    sq = sq_pool.tile([128, 512], BF16, name="sqt", tag="sq")
                    nc.scalar.activation(sq[:, :n], src, AF.Square,
                                         accum_out=acc[:, :])
                musq = t_pool.tile([128, 1], F32, name="musq", tag="musq")
                nc.vector.scalar_tensor_tensor(
                    musq[:, :], mu_n[:, :], 1.0, mu_n[:, :],
                    op0=OP.mult, op1=OP.mult)
                veps = t_pool.tile([128, 1], F32, name="veps", tag="veps")
                nc.vector.scalar_tensor_tensor(
                    veps[:, :], ss[:, :], 1.0, ssb[:, :],
                    op0=OP.mult, op1=OP.add)
                veps2 = t_pool.tile([128, 1], F32, name="veps2", tag="veps2")
                nc.vector.tensor_scalar(
                    veps2[:, :], veps[:, :], 1.0 / E, EPS,
                    op0=OP.mult, op1=OP.add)
                veps3 = t_pool.tile([128, 1], F32, name="veps3", tag="veps3")
                nc.vector.scalar_tensor_tensor(
                    veps3[:, :], musq[:, :], -1.0, veps2[:, :],
                    op0=OP.mult, op1=OP.add)
                sd = t_pool.tile([128, 1], F32, name="sd", tag="sd")
                nc.scalar.sqrt(sd[:, :], veps3[:, :])
                rstd = t_pool.tile([128, 1], F32, name="rstd", tag="rstd")
                nc.vector.reciprocal(rstd[:, :], sd[:, :])
                for (src, c0, n) in srcs:
                    nc.vector.tensor_scalar(
                        h_dst[:, c0:c0 + n], src, mu_n[:, :], rstd[:, :],
                        op0=OP.add, op1=OP.mult)
                if use_g:
                    nc.vector.scalar_tensor_tensor(
                        h_dst[:, :], h_dst[:, :], 1.0,
                        gb[:, gcol * E:(gcol + 1) * E], op0=OP.mult, op1=OP.mult)
                if use_bb:
                    nc.vector.scalar_tensor_tensor(
                        h_dst[:, :], h_dst[:, :], 1.0,
                        gb[:, (gcol + 2) * E:(gcol + 3) * E],
                        op0=OP.mult, op1=OP.add)

            XAB = {}

            def load_xa(b):
                xab = xa_pool.tile([128, KT_E * 512], BF16, name="xab",
                                   tag="xa")
                nc.gpsimd.dma_start(xab[:, :],
                                    d_xa[b * 128:(b + 1) * 128, :])
                XAB[b] = xab

            load_xa(0)
            for b in range(BL):
                t0 = b * S
                if b + 1 < BL:
                    load_xa(b + 1)
                xa = XAB.pop(b).rearrange("r (e t) -> r e t", e=KT_E)

                # ---- SelfOutput GEMM + LN1 -> h (token-major), hT ----
                hh_t = [None] * NT
                hT = ht_pool.tile([128, KT_E * S], BF16, name="htt", tag="ht")

                def emit_htrans(tt):
                    tps = [p_mm.tile([128, 512], BF16, name="htp", tag="mm")
                           for _ in range(2)]
                    for et in range(KT_E):
                        sl = tps[et // 4][:, (et % 4) * 128:(et % 4 + 1) * 128]
                        nc.tensor.transpose(
                            sl, hh_t[tt][:, et * 128:(et + 1) * 128],
                            ident[:, :])
                    for et in range(KT_E):
                        sl = tps[et // 4][:, (et % 4) * 128:(et % 4 + 1) * 128]
                        nc.vector.tensor_copy(
                            hT[:, et * S + tt * 128:et * S + (tt + 1) * 128], sl)

                for tt in range(NT):
                    ch = []
                    for ec, n in ((0, 512), (512, 256)):
                        ps = (p_mm.tile([128, 512], F32, name="sops", tag="mm")
                              if n == 512 else
                              p_mm.tile([128, 256], F32, name="sops2", tag="mm"))
                        for k in range(KT_E):
                            nc.tensor.matmul(
                                ps[:, :n], xa[:, k, tt * 128:(tt + 1) * 128],
                                WSO[k][:, ec:ec + n],
                                start=(k == 0),
                                stop=(k == KT_E - 1 and not use_bso))
                        if use_bso:
                            nc.tensor.matmul(
                                ps[:, :n], onesr[0:1, 0:128],
                                brow[4:5, ec:ec + n], start=False, stop=True)
                        ch.append((ps[:, :n], ec, n))
                    hh_t[tt] = h_pool.tile([128, E], BF16, name="hht", tag="h")
                    layernorm(ch, hh_t[tt], 0, use_g1, use_b1)
                    if tt > 0:
                        emit_htrans(tt - 1)
                emit_htrans(NT - 1)

                # ---- FFN + LN2 ----
                ffT = [None] * FT
                for ft in range(FT):
                    ps = p_mm.tile([128, 512], F32, name="fips", tag="mm")
                    for k in range(KT_E):
                        nc.tensor.matmul(
                            ps[:, :], WI[k][:, ft * 128:(ft + 1) * 128],
                            hT[:, k * S:k * S + 512],
                            start=(k == 0), stop=(k == KT_E - 1))
                    ffT[ft] = fft_pool.tile([128, 512], BF16, name="fftt",
                                            tag="fft")
                    if use_bi:
                        nc.scalar.activation(ffT[ft][:, :], ps[:, :], AF.Gelu,
                                             bias=bic[:, ft:ft + 1])
                    else:
                        nc.scalar.activation(ffT[ft][:, :], ps[:, :], AF.Gelu)
                for tt in range(NT):
                    ch = []
                    for ec, n in ((0, 512), (512, 256)):
                        ps = (p_mm.tile([128, 512], F32, name="wops", tag="mm")
                              if n == 512 else
                              p_mm.tile([128, 256], F32, name="wops2", tag="mm"))
                        for f in range(FT):
                            nc.tensor.matmul(
                                ps[:, :n],
                                ffT[f][:, tt * 128:(tt + 1) * 128],
                                WOUT[f][:, ec:ec + n],
                                start=(f == 0),
                                stop=(f == FT - 1 and not use_bout))
                        if use_bout:
                            nc.tensor.matmul(
                                ps[:, :n], onesr[0:1, 0:128],
                                brow[5:6, ec:ec + n], start=False, stop=True)
                        ch.append((ps[:, :n], ec, n))
                    otile = out_pool.tile([128, E], F32, name="ot", tag="outp")
                    resid = [hh_t[tt][:, ec:ec + n] for (_, ec, n) in ch]
                    layernorm(ch, otile, 1, use_g2, use_b2, resid=resid)
                    nc.gpsimd.dma_start(
                        d_out[t0 + tt * 128:t0 + (tt + 1) * 128, :],
                        otile[:, :])
    nc.compile()
    return nc


def _get_program(flags):
    key = ("prog", flags)
    if key not in _CACHE:
        _CACHE[key] = _build(flags)
    return _CACHE[key]


def kernel(x, mask, Wq, bq, Wk, bk, Wv, bv, Wo, bo,
           Wso, bso, gso, beso, Wi, bi, Wout, bout, gout, beout):
    from concourse.bass_utils import run_bass_kernel_spmd

    x = np.asarray(x, np.float32)
    mask = np.asarray(mask)
    sc = 1.0 / float(np.sqrt(np.float32(DK)))

    z = lambda a: not np.any(np.asarray(a))
    one = lambda a: bool(np.all(np.asarray(a) == 1.0))
    flags = (not z(bq), not z(bk), not z(bv), not z(bo), not z(bso),
             not z(bi), not z(bout),
             not one(gso), not z(beso), not one(gout), not z(beout))
    nc = _get_program(flags)

    wq8 = _pack_sw(np.asarray(Wq, np.float32) * sc)
    wk8, wo8 = _pack_sw(Wk), _pack_sw(Wo)
    # reverse V's 64 columns within each head so the on-device interleaved
    # write of the AV stationary is an ascending stride-2 copy
    wv_re = np.asarray(Wv, np.float32).reshape(E, H, DK)[:, :, ::-1].reshape(E, E)
    wv8 = _pack_blk(wv_re)
    wso_b, wi_b, wout_b = _bf(Wso), _bf(Wi), _bf(Wout)
    identb = _bf(np.eye(128))
    onesr = _bf(np.ones((1, 512)))

    brow = np.zeros((7, FF), np.float32)
    brow[0, :E] = np.asarray(bq, np.float32) * sc
    for i, v in enumerate((bk, bv, bo, bso, bout)):
        brow[i + 1, :E] = v
    brow[6, :] = bi
    brow = _bf(brow)
    bicol = np.asarray(bi, np.float32).reshape(FT, 128).T.copy()
    gbt = np.zeros((128, 4 * E), np.float32)
    for i, g in enumerate((gso, gout, beso, beout)):
        gbt[:, i * E:(i + 1) * E] = np.broadcast_to(
            np.asarray(g, np.float32).reshape(1, E), (128, E))

    in_maps = []
    for c in range(NCORES):
        xs = x[c * BL:(c + 1) * BL]            # [BL, S, E]
        ms = np.asarray(mask[c * BL:(c + 1) * BL]).reshape(BL, S)
        # mcol[r, b*NT + kt] = bias for key token kt*128 + r of batch b
        mb = np.where(ms == 0, np.float32(MASK_NEG), np.float32(0.0))
        mcol = np.ascontiguousarray(
            mb.reshape(BL, NT, 128).transpose(2, 0, 1).reshape(128, BL * NT))
        # x^T per batch in the three on-device layouts
        xtb = np.empty((BL * 128, KT_E * 512), np.float32)
        xq8 = np.empty((BL * 128, KP * 2 * 512), ml_dtypes.float8_e4m3)
        xv8 = np.empty((BL * 128, KP * NT * 256), ml_dtypes.float8_e4m3)
        for b in range(BL):
            xt = np.ascontiguousarray(xs[b].T)               # [E, S]
            xtb[b * 128:(b + 1) * 128] = xt.reshape(
                KT_E, 128, S).transpose(1, 0, 2).reshape(128, KT_E * S)
            xq8[b * 128:(b + 1) * 128] = _pack_blk(xt)
            xv8[b * 128:(b + 1) * 128] = _pack_sw(xt)
        in_maps.append({
            "ones65": _bf(np.ones((65, 128))),
            "xtb": _bf(xtb), "xq8": xq8, "xv8": xv8,
            "wq8": wq8, "wk8": wk8, "wv8": wv8, "wo8": wo8,
            "wso": wso_b, "wi": wi_b, "wout": wout_b, "mcol": mcol,
            "ident": identb,
            "onesrow": onesr, "brow": brow, "bicol": bicol, "gb": gbt,
        })

    trace = os.environ.get("KERNEL_TRACE", "0") == "1"
    res = run_bass_kernel_spmd(nc, in_maps, core_ids=list(range(NCORES)),
                               trace=trace)
    if trace and res.exec_time_ns is not None:
        print(f"HW exec time: {res.exec_time_ns} ns")
        if res.instructions_and_trace is not None:
            print(f"trace: {res.instructions_and_trace[1]}")
    out = np.concatenate([r["out"].reshape(BL, S, E) for r in res.results],
                         axis=0)
    return np.ascontiguousarray(out.astype(np.float32))


# revision 25
# speedup vs baseline: 1.1086x; 1.0171x over previous
"""BERT-base encoder layer on 8 Trainium2 NeuronCores (Bass/Tile).

Sharding: data-parallel over batch. Full inputs [32, 512, 768] split into 8
shards of 4 batches (2048 tokens); every core runs the same NEFF on its shard
(SPMD, no collectives); host concatenates the outputs.

Attention is computed k-major: scores are built transposed (ST[k, q] = K·Q^T)
so that softmax probabilities come out already in the layout the P·V matmul
needs — no PE transpose of P, and the key mask becomes a per-partition bias
on the exp activation (free) instead of rank-1 matmuls. The softmax
denominator comes from a ones-column appended to V (row 64 of the AV PSUM);
normalization is a rank-1 broadcast matmul + one vector multiply.

QKV/V/AV/O-projection GEMMs run in fp8(e4m3) DoubleRow mode (2 contraction
rows per PE pass); Wso/Wi/Wout GEMMs stay bf16 for accuracy. PSUM accumulation
is fp32 everywhere; layernorm statistics fp32.
"""

import os
import numpy as np
import ml_dtypes

B, S, E, H, DK, FF = 32, 512, 768, 12, 64, 3072
NCORES = 8
BL = B // NCORES          # batches per core = 4
T = BL * S                # tokens per core = 2048
EPS = 1e-12
MASK_NEG = -87.0          # exp(-87) == 0 in fp8/bf16
KT_E = E // 128           # 6 feature blocks
KP = KT_E // 2            # 3 fp8 contraction pairs
NT = S // 128             # 4 token tiles
FT = FF // 128            # 24
HP = H // 2               # 6 head pairs

_CACHE = {}


def _bf(a):
    return np.ascontiguousarray(np.asarray(a, np.float32).astype(ml_dtypes.bfloat16))


def _f8(a):
    a = np.clip(np.asarray(a, np.float32), -240.0, 240.0)
    return np.ascontiguousarray(a.astype(ml_dtypes.float8_e4m3))


def _pack_blk(w):
    """Moving-operand block format: [K, N] -> [128, (K//256)*2*N] fp8; slice p
    gives [128, 2, N] with element [r, i, m] = w[256p + 128i + r, m]."""
    K, N = w.shape
    p = K // 256
    arr = np.asarray(w, np.float32).reshape(p, 2, 128, N).transpose(2, 0, 1, 3)
    return _f8(arr.reshape(128, p * 2 * N))


def _pack_sw(w):
    """Stationary sw-interleave format for dual-fp8 LDWEIGHTS: [K, N] ->
    [128, (K//256)*(N//128)*256]; block (p, nb) holds column m of k-pair i at
    position 2*(127-m)+i."""
    K, N = w.shape
    P, NB = K // 256, N // 128
    a = np.asarray(w, np.float32).reshape(P, 2, 128, NB, 128)
    a = a.transpose(2, 0, 3, 4, 1)[:, :, :, ::-1, :]     # [r, p, nb, m_rev, i]
    return _f8(a.reshape(128, P * NB * 256))


def _build(flags):
    import concourse.bass as bass
    import concourse.bacc as bacc
    import concourse.mybir as mybir
    import concourse.tile as tile
    from contextlib import ExitStack

    (use_bq, use_bk, use_bv, use_bo, use_bso, use_bi, use_bout,
     use_g1, use_b1, use_g2, use_b2) = flags

    AF = mybir.ActivationFunctionType
    OP = mybir.AluOpType
    AX = mybir.AxisListType
    BF16 = mybir.dt.bfloat16
    F32 = mybir.dt.float32
    F8 = mybir.dt.float8e4
    DRS = mybir.MatmulPerfMode.DoubleRowSwInterleave
    ACT_E = mybir.EngineType.Activation

    nc = bacc.Bacc("TRN2", target_bir_lowering=False)

    # x^T per batch in three layouts (transposed/packed on host):
    # bf16 feature-major (residual), fp8 moving blocks (Q/K), fp8 interleaved
    # stationary (V)
    d_xtb = nc.dram_tensor("xtb", (BL * 128, KT_E * 512), BF16,
                           kind="ExternalInput")
    d_xq8 = nc.dram_tensor("xq8", (BL * 128, KP * 2 * 512), F8,
                           kind="ExternalInput")
    d_xv8 = nc.dram_tensor("xv8", (BL * 128, KP * NT * 256), F8,
                           kind="ExternalInput")
    d_wq8 = nc.dram_tensor("wq8", (128, KP * KT_E * 256), F8, kind="ExternalInput")
    d_wk8 = nc.dram_tensor("wk8", (128, KP * KT_E * 256), F8, kind="ExternalInput")
    d_wv8 = nc.dram_tensor("wv8", (128, KP * 2 * E), F8, kind="ExternalInput")
    d_wo8 = nc.dram_tensor("wo8", (128, KP * KT_E * 256), F8, kind="ExternalInput")
    d_wso = nc.dram_tensor("wso", (E, E), BF16, kind="ExternalInput")
    d_wi = nc.dram_tensor("wi", (E, FF), BF16, kind="ExternalInput")
    d_wout = nc.dram_tensor("wout", (FF, E), BF16, kind="ExternalInput")
    d_mcol = nc.dram_tensor("mcol", (128, BL * NT), F32, kind="ExternalInput")
    d_ones65 = nc.dram_tensor("ones65", (65, 128), BF16, kind="ExternalInput")
    d_id = nc.dram_tensor("ident", (128, 128), BF16, kind="ExternalInput")
    d_onesr = nc.dram_tensor("onesrow", (1, 512), BF16, kind="ExternalInput")
    # bias rows: 0=bq/8, 1=bk, 2=bv, 3=bo, 4=bso, 5=bout, 6=bi (full FF width)
    d_brow = nc.dram_tensor("brow", (7, FF), BF16, kind="ExternalInput")
    d_bic = nc.dram_tensor("bicol", (128, FT), F32, kind="ExternalInput")
    # gamma1 | gamma2 | beta1 | beta2, each [128, 768] partition-broadcast
    d_gb = nc.dram_tensor("gb", (128, 4 * E), F32, kind="ExternalInput")
    d_out = nc.dram_tensor("out", (T, E), F32, kind="ExternalOutput")
    # xa (x + att@Wo, feature-major bf16) spills to DRAM between superphases
    d_xa = nc.dram_tensor("xasp", (BL * 128, KT_E * 512), BF16, kind="Internal")

    need_gb = use_g1 or use_b1 or use_g2 or use_b2
    need_brow = use_bq or use_bk or use_bv or use_bo or use_bso or use_bout

    with ExitStack() as ctx:
        tc = ctx.enter_context(tile.TileContext(nc))

        p_mm = ctx.enter_context(tc.tile_pool(name="p_mm", bufs=6, space="PSUM"))
        p_av = ctx.enter_context(tc.tile_pool(name="p_av", bufs=2, space="PSUM"))

        c_pool = ctx.enter_context(tc.tile_pool(name="consts", bufs=1))
        wa_pool = ctx.enter_context(tc.tile_pool(name="wa", bufs=1))
        wso_pool = ctx.enter_context(tc.tile_pool(name="wso", bufs=KT_E))
        wi_pool = ctx.enter_context(tc.tile_pool(name="wi", bufs=KT_E))
        wout_pool = ctx.enter_context(tc.tile_pool(name="wout", bufs=FT))
        xa_pool = ctx.enter_context(tc.tile_pool(name="xa", bufs=2))

        ident = c_pool.tile_from(d_id[:, :], name="ident")
        mcol = c_pool.tile_from(d_mcol[:, :], name="mcol")
        ones65 = c_pool.tile_from(d_ones65[:, :], name="ones65")
        onesr = c_pool.tile_from(d_onesr[:, :], name="onesr") \
            if (use_bv or use_bso or use_bout or use_bq or use_bk or use_bo) else None
        brow = c_pool.tile_from(d_brow[:, :], name="brow") if need_brow else None
        gb = c_pool.tile_from(d_gb[:, :], name="gb") if need_gb else None
        bic = c_pool.tile_from(d_bic[:, :], name="bic") if use_bi else None

        # phase-A weights (fp8, small): default (SP) DMA queue
        wq8 = wa_pool.tile_from(d_wq8[:, :], name="wq8t")
        wk8 = wa_pool.tile_from(d_wk8[:, :], name="wk8t")
        wv8 = wa_pool.tile_from(d_wv8[:, :], name="wv8t")
        wo8 = wa_pool.tile_from(d_wo8[:, :], name="wo8t")

        # stationary (sw-interleaved) weights: slice (p, et) -> [128, 256]
        WQ8 = wq8.rearrange("r (p e c) -> r p e c", p=KP, e=KT_E)
        WK8 = wk8.rearrange("r (p e c) -> r p e c", p=KP, e=KT_E)
        WO8 = wo8.rearrange("r (p e c) -> r p e c", p=KP, e=KT_E)
        # moving (block) V weights: slice p -> [128, 2, E]
        WV8 = wv8.rearrange("r (p i m) -> r p i m", p=KP, i=2)

        # phase-B weights (bf16, 10.6MB) stream on the Activation DMA queue,
        # staggered through phase A so they never compete with critical loads
        WSO, WI, WOUT = [], [], []

        def load_b_weights(stage):
            if stage == 0:
                WSO.extend(wso_pool.tile_from(
                    d_wso[k * 128:(k + 1) * 128, :], name="wsot",
                    forced_dma_engine=ACT_E) for k in range(KT_E))
            elif stage == 1:
                WI.extend(wi_pool.tile_from(
                    d_wi[k * 128:(k + 1) * 128, :], name="wit",
                    forced_dma_engine=ACT_E) for k in range(KT_E))
            else:
                f0 = 0 if stage == 2 else FT // 2
                f1 = FT // 2 if stage == 2 else FT
                WOUT.extend(wout_pool.tile_from(
                    d_wout[f * 128:(f + 1) * 128, :], name="woutt",
                    forced_dma_engine=ACT_E) for f in range(f0, f1))

        # ================= superphase A: QKV, attention, O-proj ==============
        with ExitStack() as sa:
            xtb_pool = sa.enter_context(tc.tile_pool(name="xtb", bufs=2))
            xq8_pool = sa.enter_context(tc.tile_pool(name="xq8", bufs=1))
            xv8_pool = sa.enter_context(tc.tile_pool(name="xv8", bufs=1))
            qt_pool = sa.enter_context(tc.tile_pool(name="qt", bufs=2))
            kt_pool = sa.enter_context(tc.tile_pool(name="kt", bufs=2))
            va_pool = sa.enter_context(tc.tile_pool(name="va", bufs=1))
            se_pool = sa.enter_context(tc.tile_pool(name="se", bufs=24))
            at_pool = sa.enter_context(tc.tile_pool(name="at", bufs=4))
            sg_pool = sa.enter_context(tc.tile_pool(name="sg", bufs=2))
            rs_pool = sa.enter_context(tc.tile_pool(name="rs", bufs=3))
            rb_pool = sa.enter_context(tc.tile_pool(name="rb", bufs=3))

            # persistent V tiles (2 sets x 2 token-pair tiles); zero/ones
            # regions preset once (after the first x DMAs so they don't block
            # the gpsimd queue at startup)
            VAUG = [[va_pool.tile([128, H * 256], F8, name="vaug", tag="va",
                                  bufs=4) for _ in range(2)] for _ in range(2)]

            ST = {}   # per-batch state

            def s1_dma(b):
                """x^T loads (pre-transposed/packed on host)."""
                st = {}
                xtb = xtb_pool.tile([128, KT_E * 512], BF16, name="xtb",
                                    tag="xtb")
                xq8 = xq8_pool.tile([128, KP * 2 * 512], F8, name="xq8",
                                    tag="xq8")
                xv8 = xv8_pool.tile([128, KP * NT * 256], F8, name="xv8",
                                    tag="xv8")
                nc.gpsimd.dma_start(xq8[:, :], d_xq8[b * 128:(b + 1) * 128, :])
                nc.gpsimd.dma_start(xv8[:, :], d_xv8[b * 128:(b + 1) * 128, :])
                nc.gpsimd.dma_start(xtb[:, :], d_xtb[b * 128:(b + 1) * 128, :])
                st["xtb"], st["xq8"], st["xv8"] = xtb, xq8, xv8
                st["qt"] = qt_pool.tile([128, HP * 512], BF16, name="qtt",
                                        tag="qt")
                st["kt"] = kt_pool.tile([128, HP * 512], BF16, name="ktt",
                                        tag="kt")
                ST[b] = st

            def s1qk_chunk(b, j):
                """Two feature-blocks of the Q (j<3) or K (j>=3) projection."""
                st = ST[b]
                xq8_p = st["xq8"].rearrange("r (p i t) -> r p i t", p=KP, i=2)
                W8, dst, ub, brx = ((WQ8, st["qt"], use_bq, 0) if j < 3 else
                                    (WK8, st["kt"], use_bk, 1))
                for et in (2 * (j % 3), 2 * (j % 3) + 1):
                    ps = p_mm.tile([128, 512], F32, name="qkps", tag="mm")
                    for p in range(KP):
                        nc.tensor.matmul(
                            ps[:, :], W8[:, p, et, :], xq8_p[:, p, :, :],
                            perf_mode=DRS,
                            start=(p == 0), stop=(p == KP - 1 and not ub))
                    if ub:
                        nc.tensor.matmul(
                            ps[:, :],
                            brow[brx:brx + 1, et * 128:(et + 1) * 128],
                            onesr[0:1, 0:S], start=False, stop=True)
                    nc.vector.tensor_copy(dst[:, et * 512:(et + 1) * 512],
                                          ps[:, :])

            def s1v(b):
                """V projection (token-major; per-head columns reversed on the
                host so the interleaved write is an ascending stride-2 copy)."""
                st = ST[b]
                xv8_p = st["xv8"].rearrange("r (p t c) -> r p t c", p=KP, t=NT)
                vset = VAUG[b % 2]
                for tt in range(NT):
                    for ec, n in ((0, 512), (512, 256)):
                        ps = (p_mm.tile([128, 512], F32, name="vps", tag="mm")
                              if n == 512 else
                              p_mm.tile([128, 256], F32, name="vps2", tag="mm"))
                        for p in range(KP):
                            nc.tensor.matmul(
                                ps[:, :n], xv8_p[:, p, tt, :],
                                WV8[:, p, :, ec:ec + n], perf_mode=DRS,
                                start=(p == 0), stop=(p == KP - 1 and not use_bv))
                        if use_bv:
                            nc.tensor.matmul(
                                ps[:, :n], onesr[0:1, 0:128],
                                brow[2:3, ec:ec + n], start=False, stop=True)
                        h0, nh = ec // 64, n // 64
                        v6 = vset[tt // 2].rearrange(
                            "r (h a c i) -> r h a c i", h=H, a=2, c=64)
                        nc.scalar.activation(
                            v6[:, h0:h0 + nh, 1, :, tt % 2],
                            ps[:, :n].rearrange("r (h c) -> r h c", h=nh),
                            AF.Copy)
                st["vset"] = vset

            def sc_hp(b, hp):
                """scores for one head-pair (k-major, row-group dual-issue)
                + masked exp."""
                st = ST[b]
                qtt, ktt = st["qt"], st["kt"]
                stexp = st.setdefault("stexp", {})
                for kt in range(NT):
                    pss = []
                    for hh in range(2):
                        o = hh * 64
                        ps = p_mm.tile([128, 512], F32, name="scps", tag="mm")
                        nc.tensor.matmul(
                            ps[:, :],
                            ktt[o:o + 64,
                                hp * 512 + kt * 128:hp * 512 + (kt + 1) * 128],
                            qtt[o:o + 64, hp * 512:(hp + 1) * 512],
                            start=True, stop=True)
                        pss.append(ps)
                    for hh in range(2):
                        if (hp, hh, kt // 2) not in stexp:
                            stexp[(hp, hh, kt // 2)] = se_pool.tile(
                                [128, 2 * 512], F8, name="sexp", tag="se")
                        dst = stexp[(hp, hh, kt // 2)]
                        nc.scalar.activation(
                            dst[:, (kt % 2) * 512:(kt % 2 + 1) * 512],
                            pss[hh][:, :], AF.Exp,
                            bias=mcol[:, b * NT + kt:b * NT + kt + 1])

            def s2_av(b, hp):
                """AV for one head-pair (fp8, ones-column denominators) and
                the 1/s reciprocals; normalization is deferred one head-pair
                so the PE never waits on the DVE round-trip."""
                st = ST[b]
                stexp, vset = st["stexp"], st["vset"]
                if "att8" not in st:
                    st["att8"] = [at_pool.tile([128, 2 * 512], F8,
                                               name="att8", tag="at")
                                  for _ in range(KP)]
                    st["av"] = {}
                for hh in range(2):
                    av = p_av.tile([128, 512], F32, name="avps", tag="av")
                    for pp in range(2):
                        nc.tensor.matmul(
                            av[:, :], vset[pp][:, (2 * hp + hh) * 256:
                                               (2 * hp + hh + 1) * 256],
                            stexp[(hp, hh, pp)].rearrange(
                                "r (i t) -> r i t", i=2),
                            perf_mode=DRS, start=(pp == 0), stop=(pp == 1))
                    rs = rs_pool.tile([65, 512], BF16, name="rst", tag="rs")
                    with nc.allow_low_precision(
                            reason="bf16 1/s for broadcast (0.4% rel)"):
                        nc.vector.reciprocal(rs[64:65, :], av[64:65, :])
                    st["av"][(hp, hh)] = (av, rs)

            def s2_norm(b, hp):
                """Broadcast 1/s (rank-1 matmul) and scale the AV output into
                the fp8 ATT pair tiles."""
                st = ST[b]
                att8 = st["att8"]
                for hh in range(2):
                    av, rs = st["av"].pop((hp, hh))
                    rbc = p_mm.tile([64, 512], F32, name="rbc", tag="mm")
                    nc.tensor.matmul(rbc[:, :], ones65[64:65, 0:64],
                                     rs[64:65, :], start=True, stop=True)
                    rbs = rb_pool.tile([64, 512], BF16, name="rbs", tag="rb")
                    nc.scalar.activation(rbs[:, :], rbc[:, :], AF.Copy)
                    kp, half = hp // 2, hp % 2
                    if hh == 0:
                        nc.vector.scalar_tensor_tensor(
                            att8[kp][0:64, half * 512:(half + 1) * 512],
                            av[0:64, :], 1.0, rbs[:, :],
                            op0=OP.mult, op1=OP.mult)
                    else:
                        stg = sg_pool.tile([64, 512], F8, name="stg", tag="sg")
                        nc.vector.scalar_tensor_tensor(
                            stg[:, :], av[0:64, :], 1.0, rbs[:, :],
                            op0=OP.mult, op1=OP.mult)
                        nc.gpsimd.dma_start(
                            att8[kp][64:128, half * 512:(half + 1) * 512],
                            stg[:, :])

            def s2_o(b):
                """O-projection (fp8) + residual -> xa (feature-major bf16)."""
                st = ST[b]
                att8, xtb = st["att8"], st["xtb"]
                xa = xa_pool.tile([128, KT_E * 512], BF16, name="xat", tag="xa")
                for et in range(KT_E):
                    ps = p_mm.tile([128, 512], F32, name="ops", tag="mm")
                    for kp in range(KP):
                        nc.tensor.matmul(
                            ps[:, :], WO8[:, kp, et, :],
                            att8[kp].rearrange("r (i t) -> r i t", i=2),
                            perf_mode=DRS,
                            start=(kp == 0), stop=(kp == KP - 1 and not use_bo))
                    if use_bo:
                        nc.tensor.matmul(
                            ps[:, :], brow[3:4, et * 128:(et + 1) * 128],
                            onesr[0:1, 0:S], start=False, stop=True)
                    nc.vector.scalar_tensor_tensor(
                        xa[:, et * 512:(et + 1) * 512], ps[:, :], 1.0,
                        xtb[:, et * 512:(et + 1) * 512],
                        op0=OP.mult, op1=OP.add)
                nc.gpsimd.dma_start(d_xa[b * 128:(b + 1) * 128, :],
                                    xa[:, :])
                del ST[b]

            # software-pipelined emission: each head-pair iteration carries
            # the previous batch's AV, the one-earlier head-pair's softmax
            # normalization (so the PE never waits on the reciprocal
            # round-trip), a chunk of the next batch's Q/K projection, and the
            # current batch's scores. V/O projections sit at block boundaries;
            # phase-B weights stream in stages on the Activation DMA queue.
            s1_dma(0)
            for st2 in range(2):
                for pp in range(2):
                    v4 = VAUG[st2][pp].rearrange("r (h c) -> r h c", h=H)
                    nc.gpsimd.memset(v4[:, :, 0:126], 0.0)
                    nc.gpsimd.memset(v4[:, :, 126:128], 1.0)
            for j in range(HP):
                s1qk_chunk(0, j)
            s1v(0)
            s1_dma(1)
            for hp in range(HP):
                s1qk_chunk(1, hp)
                sc_hp(0, hp)
            s1v(1)
            load_b_weights(0)
            for bn in (1, 2):
                s1_dma(bn + 1)
                for hp in range(HP):
                    s2_av(bn - 1, hp)
                    if hp > 0:
                        s2_norm(bn - 1, hp - 1)
                    s1qk_chunk(bn + 1, hp)
                    sc_hp(bn, hp)
                s2_norm(bn - 1, HP - 1)
                s2_o(bn - 1)
                s1v(bn + 1)
                load_b_weights(bn)
            for hp in range(HP):
                s2_av(2, hp)
                if hp > 0:
                    s2_norm(2, hp - 1)
                sc_hp(3, hp)
            s2_norm(2, HP - 1)
            s2_o(2)
            load_b_weights(3)
            for hp in range(HP):
                s2_av(3, hp)
                if hp > 0:
                    s2_norm(3, hp - 1)
            s2_norm(3, HP - 1)
            s2_o(3)

        # ============ superphase B: SelfOutput LN, FFN, LN ===================
        with ExitStack() as sb:
            h_pool = sb.enter_context(tc.tile_pool(name="h", bufs=NT + 1))
            hp_pool = sb.enter_context(tc.tile_pool(name="hpre", bufs=NT + 1))
            ht_pool = sb.enter_context(tc.tile_pool(name="ht", bufs=2))
            fft_pool = sb.enter_context(tc.tile_pool(name="fft", bufs=FT + 2))
            sq_pool = sb.enter_context(tc.tile_pool(name="sq", bufs=2))
            rs_pool = sb.enter_context(tc.tile_pool(name="rsd", bufs=3))
            out_pool = sb.enter_context(tc.tile_pool(name="outp", bufs=3))
            t_pool = sb.enter_context(tc.tile_pool(name="sb_s", bufs=10))

            def ln_stats(s1c, ssc, nt):
                """Batched LN statistics for nt row-groups: s1c/ssc hold
                per-(tile, chunk) sums/square-sums in 2*nt columns; returns
                (mu_n, rstd) [128, nt]."""
                tot = t_pool.tile([128, nt], F32, name="tot", tag="t1")
                v = s1c.rearrange("r (t c) -> r t c", c=2)
                nc.vector.scalar_tensor_tensor(
                    tot[:, :], v[:, :, 0], 1.0, v[:, :, 1],
                    op0=OP.mult, op1=OP.add)
                mu_n = t_pool.tile([128, nt], F32, name="mun", tag="t2")
                nc.vector.tensor_scalar_mul(mu_n[:, :], tot[:, :], -1.0 / E)
                tot2 = t_pool.tile([128, nt], F32, name="tot2", tag="t3")
                w = ssc.rearrange("r (t c) -> r t c", c=2)
                nc.vector.scalar_tensor_tensor(
                    tot2[:, :], w[:, :, 0], 1.0, w[:, :, 1],
                    op0=OP.mult, op1=OP.add)
                veps = t_pool.tile([128, nt], F32, name="veps", tag="t4")
                nc.vector.tensor_scalar(
                    veps[:, :], tot2[:, :], 1.0 / E, EPS,
                    op0=OP.mult, op1=OP.add)
                musq = t_pool.tile([128, nt], F32, name="musq", tag="t5")
                nc.vector.scalar_tensor_tensor(
                    musq[:, :], mu_n[:, :], 1.0, mu_n[:, :],
                    op0=OP.mult, op1=OP.mult)
                veps3 = t_pool.tile([128, nt], F32, name="veps3", tag="t6")
                nc.vector.scalar_tensor_tensor(
                    veps3[:, :], musq[:, :], -1.0, veps[:, :],
                    op0=OP.mult, op1=OP.add)
                sd = t_pool.tile([128, nt], F32, name="sd", tag="t7")
                nc.scalar.sqrt(sd[:, :], veps3[:, :])
                rstd = t_pool.tile([128, nt], F32, name="rstd", tag="t8")
                nc.vector.reciprocal(rstd[:, :], sd[:, :])
                return mu_n, rstd

            def ln_norm(dst, srct, mu_n, rstd, col, gcol, use_g, use_bb):
                nc.vector.tensor_scalar(
                    dst[:, :], srct[:, :], mu_n[:, col:col + 1],
                    rstd[:, col:col + 1], op0=OP.add, op1=OP.mult)
                if use_g:
                    nc.vector.scalar_tensor_tensor(
                        dst[:, :], dst[:, :], 1.0,
                        gb[:, gcol * E:(gcol + 1) * E], op0=OP.mult, op1=OP.mult)
                if use_bb:
                    nc.vector.scalar_tensor_tensor(
                        dst[:, :], dst[:, :], 1.0,
                        gb[:, (gcol + 2) * E:(gcol + 3) * E],
                        op0=OP.mult, op1=OP.add)

            XAB = {}

            def load_xa(b):
                xab = xa_pool.tile([128, KT_E * 512], BF16, name="xab",
                                   tag="xa")
                nc.gpsimd.dma_start(xab[:, :],
                                    d_xa[b * 128:(b + 1) * 128, :])
                XAB[b] = xab

            load_xa(0)
            for b in range(BL):
                t0 = b * S
                if b + 1 < BL:
                    load_xa(b + 1)
                xa = XAB.pop(b).rearrange("r (e t) -> r e t", e=KT_E)

                # ---- SelfOutput GEMM + LN1 (stats batched per tile-pair),
                # h-transposes overlap the next pair's GEMMs ----
                hh_t = [None] * NT
                hpre = [None] * NT
                hT = ht_pool.tile([128, KT_E * S], BF16, name="htt", tag="ht")
                s1c = t_pool.tile([128, 2 * NT], F32, name="s1c", tag="s1c")
                ssc = t_pool.tile([128, 2 * NT], F32, name="ssc", tag="ssc")

                def so_tile(tt):
                    hpre[tt] = hp_pool.tile([128, E], F32, name="hpt",
                                            tag="hpre")
                    for ci, (ec, n) in enumerate(((0, 512), (512, 256))):
                        ps = (p_mm.tile([128, 512], F32, name="sops", tag="mm")
                              if n == 512 else
                              p_mm.tile([128, 256], F32, name="sops2", tag="mm"))
                        for k in range(KT_E):
                            nc.tensor.matmul(
                                ps[:, :n], xa[:, k, tt * 128:(tt + 1) * 128],
                                WSO[k][:, ec:ec + n],
                                start=(k == 0),
                                stop=(k == KT_E - 1 and not use_bso))
                        if use_bso:
                            nc.tensor.matmul(
                                ps[:, :n], onesr[0:1, 0:128],
                                brow[4:5, ec:ec + n], start=False, stop=True)
                        cc = tt * 2 + ci
                        nc.vector.tensor_scalar(
                            hpre[tt][:, ec:ec + n], ps[:, :n], 1.0, 0.0,
                            op0=OP.mult, op1=OP.add,
                            accum_out=s1c[:, cc:cc + 1])
                        sq = sq_pool.tile([128, 512], BF16, name="sqt",
                                          tag="sq")
                        nc.scalar.activation(sq[:, :n], hpre[tt][:, ec:ec + n],
                                             AF.Square,
                                             accum_out=ssc[:, cc:cc + 1])

                def emit_htrans(tt):
                    tps = [p_mm.tile([128, 512], BF16, name="htp", tag="mm")
                           for _ in range(2)]
                    for et in range(KT_E):
                        sl = tps[et // 4][:, (et % 4) * 128:(et % 4 + 1) * 128]
                        nc.tensor.transpose(
                            sl, hh_t[tt][:, et * 128:(et + 1) * 128],
                            ident[:, :])
                    for et in range(KT_E):
                        sl = tps[et // 4][:, (et % 4) * 128:(et % 4 + 1) * 128]
                        nc.vector.tensor_copy(
                            hT[:, et * S + tt * 128:et * S + (tt + 1) * 128], sl)

                for g in range(2):
                    so_tile(2 * g); so_tile(2 * g + 1)
                    mu_n, rstd = ln_stats(s1c[:, 4 * g:4 * g + 4],
                                          ssc[:, 4 * g:4 * g + 4], 2)
                    for tt in (2 * g, 2 * g + 1):
                        hh_t[tt] = h_pool.tile([128, E], BF16, name="hht",
                                               tag="h")
                        ln_norm(hh_t[tt], hpre[tt], mu_n, rstd, tt - 2 * g,
                                0, use_g1, use_b1)
                    if g == 1:
                        emit_htrans(0); emit_htrans(1)
                        emit_htrans(2); emit_htrans(3)

                # ---- FFN ----
                ffT = [None] * FT
                for ft in range(FT):
                    ps = p_mm.tile([128, 512], F32, name="fips", tag="mm")
                    for k in range(KT_E):
                        nc.tensor.matmul(
                            ps[:, :], WI[k][:, ft * 128:(ft + 1) * 128],
                            hT[:, k * S:k * S + 512],
                            start=(k == 0), stop=(k == KT_E - 1))
                    ffT[ft] = fft_pool.tile([128, 512], BF16, name="fftt",
                                            tag="fft")
                    if use_bi:
                        nc.scalar.activation(ffT[ft][:, :], ps[:, :], AF.Gelu,
                                             bias=bic[:, ft:ft + 1])
                    else:
                        nc.scalar.activation(ffT[ft][:, :], ps[:, :], AF.Gelu)

                # ---- Wout + LN2 (residual-fused drains, stats batched over
                # all four tiles; nothing downstream waits on LN2) ----
                s2c = t_pool.tile([128, 2 * NT], F32, name="s2c", tag="s2c")
                sc2 = t_pool.tile([128, 2 * NT], F32, name="sc2", tag="sc2")
                rt = [None] * NT
                for tt in range(NT):
                    rt[tt] = rs_pool.tile([128, E], F32, name="rt", tag="rsd",
                                          bufs=5)
                    for ci, (ec, n) in enumerate(((0, 512), (512, 256))):
                        ps = (p_mm.tile([128, 512], F32, name="wops", tag="mm")
                              if n == 512 else
                              p_mm.tile([128, 256], F32, name="wops2", tag="mm"))
                        for f in range(FT):
                            nc.tensor.matmul(
                                ps[:, :n],
                                ffT[f][:, tt * 128:(tt + 1) * 128],
                                WOUT[f][:, ec:ec + n],
                                start=(f == 0),
                                stop=(f == FT - 1 and not use_bout))
                        if use_bout:
                            nc.tensor.matmul(
                                ps[:, :n], onesr[0:1, 0:128],
                                brow[5:6, ec:ec + n], start=False, stop=True)
                        cc = tt * 2 + ci
                        nc.vector.scalar_tensor_tensor(
                            rt[tt][:, ec:ec + n], ps[:, :n], 1.0,
                            hh_t[tt][:, ec:ec + n], op0=OP.mult, op1=OP.add,
                            accum_out=s2c[:, cc:cc + 1])
                        sq = sq_pool.tile([128, 512], BF16, name="sq2",
                                          tag="sq")
                        nc.scalar.activation(sq[:, :n], rt[tt][:, ec:ec + n],
                                             AF.Square,
                                             accum_out=sc2[:, cc:cc + 1])
                mu_n, rstd = ln_stats(s2c, sc2, NT)
                for tt in range(NT):
                    otile = out_pool.tile([128, E], F32, name="ot", tag="outp")
                    ln_norm(otile, rt[tt], mu_n, rstd, tt, 1, use_g2, use_b2)
                    nc.gpsimd.dma_start(
                        d_out[t0 + tt * 128:t0 + (tt + 1) * 128, :],
                        otile[:, :])
    nc.compile()
    return nc


def _get_program(flags):
    key = ("prog", flags)
    if key not in _CACHE:
        _CACHE[key] = _build(flags)
    return _CACHE[key]


def kernel(x, mask, Wq, bq, Wk, bk, Wv, bv, Wo, bo,
           Wso, bso, gso, beso, Wi, bi, Wout, bout, gout, beout):
    from concourse.bass_utils import run_bass_kernel_spmd

    x = np.asarray(x, np.float32)
    mask = np.asarray(mask)
    sc = 1.0 / float(np.sqrt(np.float32(DK)))

    z = lambda a: not np.any(np.asarray(a))
    one = lambda a: bool(np.all(np.asarray(a) == 1.0))
    flags = (not z(bq), not z(bk), not z(bv), not z(bo), not z(bso),
             not z(bi), not z(bout),
             not one(gso), not z(beso), not one(gout), not z(beout))
    nc = _get_program(flags)

    wq8 = _pack_sw(np.asarray(Wq, np.float32) * sc)
    wk8, wo8 = _pack_sw(Wk), _pack_sw(Wo)
    # reverse V's 64 columns within each head so the on-device interleaved
    # write of the AV stationary is an ascending stride-2 copy
    wv_re = np.asarray(Wv, np.float32).reshape(E, H, DK)[:, :, ::-1].reshape(E, E)
    wv8 = _pack_blk(wv_re)
    wso_b, wi_b, wout_b = _bf(Wso), _bf(Wi), _bf(Wout)
    identb = _bf(np.eye(128))
    onesr = _bf(np.ones((1, 512)))

    brow = np.zeros((7, FF), np.float32)
    brow[0, :E] = np.asarray(bq, np.float32) * sc
    for i, v in enumerate((bk, bv, bo, bso, bout)):
        brow[i + 1, :E] = v
    brow[6, :] = bi
    brow = _bf(brow)
    bicol = np.asarray(bi, np.float32).reshape(FT, 128).T.copy()
    gbt = np.zeros((128, 4 * E), np.float32)
    for i, g in enumerate((gso, gout, beso, beout)):
        gbt[:, i * E:(i + 1) * E] = np.broadcast_to(
            np.asarray(g, np.float32).reshape(1, E), (128, E))

    in_maps = []
    for c in range(NCORES):
        xs = x[c * BL:(c + 1) * BL]            # [BL, S, E]
        ms = np.asarray(mask[c * BL:(c + 1) * BL]).reshape(BL, S)
        # mcol[r, b*NT + kt] = bias for key token kt*128 + r of batch b
        mb = np.where(ms == 0, np.float32(MASK_NEG), np.float32(0.0))
        mcol = np.ascontiguousarray(
            mb.reshape(BL, NT, 128).transpose(2, 0, 1).reshape(128, BL * NT))
        # x^T per batch in the three on-device layouts
        xtb = np.empty((BL * 128, KT_E * 512), np.float32)
        xq8 = np.empty((BL * 128, KP * 2 * 512), ml_dtypes.float8_e4m3)
        xv8 = np.empty((BL * 128, KP * NT * 256), ml_dtypes.float8_e4m3)
        for b in range(BL):
            xt = np.ascontiguousarray(xs[b].T)               # [E, S]
            xtb[b * 128:(b + 1) * 128] = xt.reshape(
                KT_E, 128, S).transpose(1, 0, 2).reshape(128, KT_E * S)
            xq8[b * 128:(b + 1) * 128] = _pack_blk(xt)
            xv8[b * 128:(b + 1) * 128] = _pack_sw(xt)
        in_maps.append({
            "ones65": _bf(np.ones((65, 128))),
            "xtb": _bf(xtb), "xq8": xq8, "xv8": xv8,
            "wq8": wq8, "wk8": wk8, "wv8": wv8, "wo8": wo8,
            "wso": wso_b, "wi": wi_b, "wout": wout_b, "mcol": mcol,
            "ident": identb,
            "onesrow": onesr, "brow": brow, "bicol": bicol, "gb": gbt,
        })

    trace = os.environ.get("KERNEL_TRACE", "0") == "1"
    res = run_bass_kernel_spmd(nc, in_maps, core_ids=list(range(NCORES)),
                               trace=trace)
    if trace and res.exec_time_ns is not None:
        print(f"HW exec time: {res.exec_time_ns} ns")
        if res.instructions_and_trace is not None:
            print(f"trace: {res.instructions_and_trace[1]}")
    out = np.concatenate([r["out"].reshape(BL, S, E) for r in res.results],
                         axis=0)
    return np.ascontiguousarray(out.astype(np.float32))


# revision 28
# speedup vs baseline: 1.1291x; 1.0185x over previous
"""BERT-base encoder layer on 8 Trainium2 NeuronCores (Bass/Tile).

Sharding: data-parallel over batch. Full inputs [32, 512, 768] split into 8
shards of 4 batches (2048 tokens); every core runs the same NEFF on its shard
(SPMD, no collectives); host concatenates the outputs.

Attention is computed k-major: scores are built transposed (ST[k, q] = K·Q^T)
so that softmax probabilities come out already in the layout the P·V matmul
needs — no PE transpose of P, and the key mask becomes a per-partition bias
on the exp activation (free) instead of rank-1 matmuls. The softmax
denominator comes from a ones-column appended to V (row 64 of the AV PSUM);
normalization is a rank-1 broadcast matmul + one vector multiply.

QKV/V/AV/O-projection GEMMs run in fp8(e4m3) DoubleRow mode (2 contraction
rows per PE pass); Wso/Wi/Wout GEMMs stay bf16 for accuracy. PSUM accumulation
is fp32 everywhere; layernorm statistics fp32.
"""

import os
import numpy as np
import ml_dtypes

B, S, E, H, DK, FF = 32, 512, 768, 12, 64, 3072
NCORES = 8
BL = B // NCORES          # batches per core = 4
T = BL * S                # tokens per core = 2048
EPS = 1e-12
MASK_NEG = -87.0          # exp(-87) == 0 in fp8/bf16
KT_E = E // 128           # 6 feature blocks
KP = KT_E // 2            # 3 fp8 contraction pairs
NT = S // 128             # 4 token tiles
FT = FF // 128            # 24
HP = H // 2               # 6 head pairs

_CACHE = {}


def _bf(a):
    return np.ascontiguousarray(np.asarray(a, np.float32).astype(ml_dtypes.bfloat16))


def _f8(a):
    a = np.clip(np.asarray(a, np.float32), -240.0, 240.0)
    return np.ascontiguousarray(a.astype(ml_dtypes.float8_e4m3))


def _pack_blk(w):
    """Moving-operand block format: [K, N] -> [128, (K//256)*2*N] fp8; slice p
    gives [128, 2, N] with element [r, i, m] = w[256p + 128i + r, m]."""
    K, N = w.shape
    p = K // 256
    arr = np.asarray(w, np.float32).reshape(p, 2, 128, N).transpose(2, 0, 1, 3)
    return _f8(arr.reshape(128, p * 2 * N))


def _pack_sw(w):
    """Stationary sw-interleave format for dual-fp8 LDWEIGHTS: [K, N] ->
    [128, (K//256)*(N//128)*256]; block (p, nb) holds column m of k-pair i at
    position 2*(127-m)+i."""
    K, N = w.shape
    P, NB = K // 256, N // 128
    a = np.asarray(w, np.float32).reshape(P, 2, 128, NB, 128)
    a = a.transpose(2, 0, 3, 4, 1)[:, :, :, ::-1, :]     # [r, p, nb, m_rev, i]
    return _f8(a.reshape(128, P * NB * 256))


def _build(flags):
    import concourse.bass as bass
    import concourse.bacc as bacc
    import concourse.mybir as mybir
    import concourse.tile as tile
    from contextlib import ExitStack

    (use_bq, use_bk, use_bv, use_bo, use_bso, use_bi, use_bout,
     use_g1, use_b1, use_g2, use_b2) = flags

    AF = mybir.ActivationFunctionType
    OP = mybir.AluOpType
    AX = mybir.AxisListType
    BF16 = mybir.dt.bfloat16
    F32 = mybir.dt.float32
    F8 = mybir.dt.float8e4
    DRS = mybir.MatmulPerfMode.DoubleRowSwInterleave
    SP_E = mybir.EngineType.SP

    nc = bacc.Bacc("TRN2", target_bir_lowering=False)

    # x^T per batch in three layouts (transposed/packed on host):
    # bf16 feature-major (residual), fp8 moving blocks (Q/K), fp8 interleaved
    # stationary (V)
    d_xtb = nc.dram_tensor("xtb", (BL * 128, KT_E * 512), BF16,
                           kind="ExternalInput")
    d_xq8 = nc.dram_tensor("xq8", (BL * 128, KP * 2 * 512), F8,
                           kind="ExternalInput")
    d_xv8 = nc.dram_tensor("xv8", (BL * 128, KP * NT * 256), F8,
                           kind="ExternalInput")
    d_wq8 = nc.dram_tensor("wq8", (128, KP * KT_E * 256), F8, kind="ExternalInput")
    d_wk8 = nc.dram_tensor("wk8", (128, KP * KT_E * 256), F8, kind="ExternalInput")
    d_wv8 = nc.dram_tensor("wv8", (128, KP * 2 * E), F8, kind="ExternalInput")
    d_wo8 = nc.dram_tensor("wo8", (128, KP * KT_E * 256), F8, kind="ExternalInput")
    d_wso = nc.dram_tensor("wso", (E, E), BF16, kind="ExternalInput")
    d_wi = nc.dram_tensor("wi", (E, FF), BF16, kind="ExternalInput")
    d_wout = nc.dram_tensor("wout", (FF, E), BF16, kind="ExternalInput")
    d_mcol = nc.dram_tensor("mcol", (128, BL * NT), F32, kind="ExternalInput")
    d_ones65 = nc.dram_tensor("ones65", (65, 128), BF16, kind="ExternalInput")
    d_id = nc.dram_tensor("ident", (128, 128), BF16, kind="ExternalInput")
    d_onesr = nc.dram_tensor("onesrow", (1, 512), BF16, kind="ExternalInput")
    # bias rows: 0=bq/8, 1=bk, 2=bv, 3=bo, 4=bso, 5=bout, 6=bi (full FF width)
    d_brow = nc.dram_tensor("brow", (7, FF), BF16, kind="ExternalInput")
    d_bic = nc.dram_tensor("bicol", (128, FT), F32, kind="ExternalInput")
    # gamma1 | gamma2 | beta1 | beta2, each [128, 768] partition-broadcast
    d_gb = nc.dram_tensor("gb", (128, 4 * E), F32, kind="ExternalInput")
    d_out = nc.dram_tensor("out", (T, E), F32, kind="ExternalOutput")
    # xa (x + att@Wo, feature-major bf16) spills to DRAM between superphases
    d_xa = nc.dram_tensor("xasp", (BL * 128, KT_E * 512), BF16, kind="Internal")

    need_gb = use_g1 or use_b1 or use_g2 or use_b2
    need_brow = use_bq or use_bk or use_bv or use_bo or use_bso or use_bout

    with ExitStack() as ctx:
        tc = ctx.enter_context(tile.TileContext(nc))

        p_mm = ctx.enter_context(tc.tile_pool(name="p_mm", bufs=6, space="PSUM"))
        p_av = ctx.enter_context(tc.tile_pool(name="p_av", bufs=2, space="PSUM"))

        c_pool = ctx.enter_context(tc.tile_pool(name="consts", bufs=1))
        wa_pool = ctx.enter_context(tc.tile_pool(name="wa", bufs=1))
        wso_pool = ctx.enter_context(tc.tile_pool(name="wso", bufs=KT_E))
        wi_pool = ctx.enter_context(tc.tile_pool(name="wi", bufs=KT_E))
        wout_pool = ctx.enter_context(tc.tile_pool(name="wout", bufs=FT))
        xa_pool = ctx.enter_context(tc.tile_pool(name="xa", bufs=2))

        ident = c_pool.tile_from(d_id[:, :], name="ident")
        mcol = c_pool.tile_from(d_mcol[:, :], name="mcol")
        onesr = c_pool.tile_from(d_onesr[:, :], name="onesr") \
            if (use_bv or use_bso or use_bout or use_bq or use_bk or use_bo) else None
        brow = c_pool.tile_from(d_brow[:, :], name="brow") if need_brow else None
        gb = c_pool.tile_from(d_gb[:, :], name="gb") if need_gb else None
        bic = c_pool.tile_from(d_bic[:, :], name="bic") if use_bi else None

        # phase-A weights (fp8, small): default (SP) DMA queue, in
        # first-use order so the first projections start early
        wq8 = wa_pool.tile_from(d_wq8[:, :], name="wq8t")
        wk8 = wa_pool.tile_from(d_wk8[:, :], name="wk8t")
        wv8 = wa_pool.tile_from(d_wv8[:, :], name="wv8t")
        ones65 = c_pool.tile_from(d_ones65[:, :], name="ones65")
        wo8 = wa_pool.tile_from(d_wo8[:, :], name="wo8t")

        # stationary (sw-interleaved) weights: slice (p, et) -> [128, 256]
        WQ8 = wq8.rearrange("r (p e c) -> r p e c", p=KP, e=KT_E)
        WK8 = wk8.rearrange("r (p e c) -> r p e c", p=KP, e=KT_E)
        WO8 = wo8.rearrange("r (p e c) -> r p e c", p=KP, e=KT_E)
        # moving (block) V weights: slice p -> [128, 2, E]
        WV8 = wv8.rearrange("r (p i m) -> r p i m", p=KP, i=2)

        # phase-B weights (bf16, 10.6MB) stream on the Activation DMA queue,
        # staggered through phase A so they never compete with critical loads
        WSO, WI, WOUT = [], [], []

        def load_b_weights(stage):
            if stage == 0:
                WSO.extend(wso_pool.tile_from(
                    d_wso[k * 128:(k + 1) * 128, :], name="wsot",
                    forced_dma_engine=SP_E) for k in range(KT_E))
            elif stage == 1:
                WI.extend(wi_pool.tile_from(
                    d_wi[k * 128:(k + 1) * 128, :], name="wit",
                    forced_dma_engine=SP_E) for k in range(KT_E))
            else:
                f0 = 0 if stage == 2 else FT // 2
                f1 = FT // 2 if stage == 2 else FT
                WOUT.extend(wout_pool.tile_from(
                    d_wout[f * 128:(f + 1) * 128, :], name="woutt",
                    forced_dma_engine=SP_E) for f in range(f0, f1))

        # ================= superphase A: QKV, attention, O-proj ==============
        with ExitStack() as sa:
            xtb_pool = sa.enter_context(tc.tile_pool(name="xtb", bufs=2))
            xq8_pool = sa.enter_context(tc.tile_pool(name="xq8", bufs=1))
            xv8_pool = sa.enter_context(tc.tile_pool(name="xv8", bufs=1))
            qt_pool = sa.enter_context(tc.tile_pool(name="qt", bufs=2))
            kt_pool = sa.enter_context(tc.tile_pool(name="kt", bufs=2))
            va_pool = sa.enter_context(tc.tile_pool(name="va", bufs=1))
            se_pool = sa.enter_context(tc.tile_pool(name="se", bufs=24))
            at_pool = sa.enter_context(tc.tile_pool(name="at", bufs=4))
            sg_pool = sa.enter_context(tc.tile_pool(name="sg", bufs=2))
            rs_pool = sa.enter_context(tc.tile_pool(name="rs", bufs=6))
            rb_pool = sa.enter_context(tc.tile_pool(name="rb", bufs=3))

            # persistent V tiles (2 sets x 2 token-pair tiles); zero/ones
            # regions preset once (after the first x DMAs so they don't block
            # the gpsimd queue at startup)
            VAUG = [[va_pool.tile([128, H * 256], F8, name="vaug", tag="va",
                                  bufs=4) for _ in range(2)] for _ in range(2)]

            ST = {}   # per-batch state

            def s1_dma(b):
                """x^T loads (pre-transposed/packed on host)."""
                st = {}
                xtb = xtb_pool.tile([128, KT_E * 512], BF16, name="xtb",
                                    tag="xtb")
                xq8 = xq8_pool.tile([128, KP * 2 * 512], F8, name="xq8",
                                    tag="xq8")
                xv8 = xv8_pool.tile([128, KP * NT * 256], F8, name="xv8",
                                    tag="xv8")
                nc.gpsimd.dma_start(xq8[:, :], d_xq8[b * 128:(b + 1) * 128, :])
                nc.gpsimd.dma_start(xv8[:, :], d_xv8[b * 128:(b + 1) * 128, :])
                nc.gpsimd.dma_start(xtb[:, :], d_xtb[b * 128:(b + 1) * 128, :])
                st["xtb"], st["xq8"], st["xv8"] = xtb, xq8, xv8
                st["qt"] = qt_pool.tile([128, HP * 512], BF16, name="qtt",
                                        tag="qt")
                st["kt"] = kt_pool.tile([128, HP * 512], BF16, name="ktt",
                                        tag="kt")
                ST[b] = st

            def s1qk_chunk(b, j):
                """Two feature-blocks of the Q (j<3) or K (j>=3) projection."""
                st = ST[b]
                xq8_p = st["xq8"].rearrange("r (p i t) -> r p i t", p=KP, i=2)
                W8, dst, ub, brx = ((WQ8, st["qt"], use_bq, 0) if j < 3 else
                                    (WK8, st["kt"], use_bk, 1))
                for et in (2 * (j % 3), 2 * (j % 3) + 1):
                    ps = p_mm.tile([128, 512], F32, name="qkps", tag="mm")
                    for p in range(KP):
                        nc.tensor.matmul(
                            ps[:, :], W8[:, p, et, :], xq8_p[:, p, :, :],
                            perf_mode=DRS,
                            start=(p == 0), stop=(p == KP - 1 and not ub))
                    if ub:
                        nc.tensor.matmul(
                            ps[:, :],
                            brow[brx:brx + 1, et * 128:(et + 1) * 128],
                            onesr[0:1, 0:S], start=False, stop=True)
                    nc.vector.tensor_copy(dst[:, et * 512:(et + 1) * 512],
                                          ps[:, :])

            def s1v(b):
                """V projection (token-major; per-head columns reversed on the
                host so the interleaved write is an ascending stride-2 copy)."""
                st = ST[b]
                xv8_p = st["xv8"].rearrange("r (p t c) -> r p t c", p=KP, t=NT)
                vset = VAUG[b % 2]
                for tt in range(NT):
                    for ec, n in ((0, 512), (512, 256)):
                        ps = (p_mm.tile([128, 512], F32, name="vps", tag="mm")
                              if n == 512 else
                              p_mm.tile([128, 256], F32, name="vps2", tag="mm"))
                        for p in range(KP):
                            nc.tensor.matmul(
                                ps[:, :n], xv8_p[:, p, tt, :],
                                WV8[:, p, :, ec:ec + n], perf_mode=DRS,
                                start=(p == 0), stop=(p == KP - 1 and not use_bv))
                        if use_bv:
                            nc.tensor.matmul(
                                ps[:, :n], onesr[0:1, 0:128],
                                brow[2:3, ec:ec + n], start=False, stop=True)
                        h0, nh = ec // 64, n // 64
                        v6 = vset[tt // 2].rearrange(
                            "r (h a c i) -> r h a c i", h=H, a=2, c=64)
                        nc.scalar.activation(
                            v6[:, h0:h0 + nh, 1, :, tt % 2],
                            ps[:, :n].rearrange("r (h c) -> r h c", h=nh),
                            AF.Copy)
                st["vset"] = vset

            def sc_hp(b, hp):
                """scores for one head-pair (k-major, row-group dual-issue)
                + masked exp."""
                st = ST[b]
                qtt, ktt = st["qt"], st["kt"]
                stexp = st.setdefault("stexp", {})
                for kt in range(NT):
                    pss = []
                    for hh in range(2):
                        o = hh * 64
                        ps = p_mm.tile([128, 512], F32, name="scps", tag="mm")
                        nc.tensor.matmul(
                            ps[:, :],
                            ktt[o:o + 64,
                                hp * 512 + kt * 128:hp * 512 + (kt + 1) * 128],
                            qtt[o:o + 64, hp * 512:(hp + 1) * 512],
                            start=True, stop=True)
                        pss.append(ps)
                    for hh in range(2):
                        if (hp, hh, kt // 2) not in stexp:
                            stexp[(hp, hh, kt // 2)] = se_pool.tile(
                                [128, 2 * 512], F8, name="sexp", tag="se")
                        dst = stexp[(hp, hh, kt // 2)]
                        nc.scalar.activation(
                            dst[:, (kt % 2) * 512:(kt % 2 + 1) * 512],
                            pss[hh][:, :], AF.Exp,
                            bias=mcol[:, b * NT + kt:b * NT + kt + 1])

            def s2_av(b, hp):
                """AV for one head-pair (fp8, ones-column denominators);
                drains the PSUM to SBUF immediately and takes 1/s in place.
                Normalization is deferred one head-pair so the PE never waits
                on the DVE round-trip."""
                st = ST[b]
                stexp, vset = st["stexp"], st["vset"]
                if "att8" not in st:
                    st["att8"] = [at_pool.tile([128, 2 * 512], F8,
                                               name="att8", tag="at")
                                  for _ in range(KP)]
                    st["av"] = {}
                for hh in range(2):
                    av = p_av.tile([128, 512], F32, name="avps", tag="av")
                    for pp in range(2):
                        nc.tensor.matmul(
                            av[:, :], vset[pp][:, (2 * hp + hh) * 256:
                                               (2 * hp + hh + 1) * 256],
                            stexp[(hp, hh, pp)].rearrange(
                                "r (i t) -> r i t", i=2),
                            perf_mode=DRS, start=(pp == 0), stop=(pp == 1))
                    avs = rs_pool.tile([65, 512], BF16, name="avs", tag="rs")
                    with nc.allow_low_precision(
                            reason="bf16 unnormalized attention + 1/s"):
                        nc.vector.tensor_copy(avs[:, :], av[0:65, :])
                        nc.vector.reciprocal(avs[64:65, :], avs[64:65, :])
                    st["av"][(hp, hh)] = avs

            def s2_norm(b, hp):
                """Broadcast 1/s (rank-1 matmul) and scale the AV output into
                the fp8 ATT pair tiles (multiply on the Pool engine)."""
                st = ST[b]
                att8 = st["att8"]
                for hh in range(2):
                    avs = st["av"].pop((hp, hh))
                    rbc = p_mm.tile([64, 512], F32, name="rbc", tag="mm")
                    nc.tensor.matmul(rbc[:, :], ones65[64:65, 0:64],
                                     avs[64:65, :], start=True, stop=True)
                    rbs = rb_pool.tile([64, 512], BF16, name="rbs", tag="rb")
                    nc.vector.tensor_copy(rbs[:, :], rbc[:, :])
                    kp, half = hp // 2, hp % 2
                    if hh == 0:
                        nc.vector.scalar_tensor_tensor(
                            att8[kp][0:64, half * 512:(half + 1) * 512],
                            avs[0:64, :], 1.0, rbs[:, :],
                            op0=OP.mult, op1=OP.mult)
                    else:
                        stg = sg_pool.tile([64, 512], F8, name="stg", tag="sg")
                        nc.vector.scalar_tensor_tensor(
                            stg[:, :], avs[0:64, :], 1.0, rbs[:, :],
                            op0=OP.mult, op1=OP.mult)
                        nc.gpsimd.dma_start(
                            att8[kp][64:128, half * 512:(half + 1) * 512],
                            stg[:, :])

            def s2_o(b):
                """O-projection (fp8) + residual -> xa (feature-major bf16)."""
                st = ST[b]
                att8, xtb = st["att8"], st["xtb"]
                xa = xa_pool.tile([128, KT_E * 512], BF16, name="xat", tag="xa")
                for et in range(KT_E):
                    ps = p_mm.tile([128, 512], F32, name="ops", tag="mm")
                    for kp in range(KP):
                        nc.tensor.matmul(
                            ps[:, :], WO8[:, kp, et, :],
                            att8[kp].rearrange("r (i t) -> r i t", i=2),
                            perf_mode=DRS,
                            start=(kp == 0), stop=(kp == KP - 1 and not use_bo))
                    if use_bo:
                        nc.tensor.matmul(
                            ps[:, :], brow[3:4, et * 128:(et + 1) * 128],
                            onesr[0:1, 0:S], start=False, stop=True)
                    nc.vector.scalar_tensor_tensor(
                        xa[:, et * 512:(et + 1) * 512], ps[:, :], 1.0,
                        xtb[:, et * 512:(et + 1) * 512],
                        op0=OP.mult, op1=OP.add)
                nc.gpsimd.dma_start(d_xa[b * 128:(b + 1) * 128, :],
                                    xa[:, :])
                del ST[b]

            # software-pipelined emission: each head-pair iteration carries
            # the previous batch's AV, the one-earlier head-pair's softmax
            # normalization (so the PE never waits on the reciprocal
            # round-trip), a chunk of the next batch's Q/K projection, and the
            # current batch's scores. V/O projections sit at block boundaries;
            # phase-B weights stream in stages on the Activation DMA queue.
            s1_dma(0)
            for st2 in range(2):
                for pp in range(2):
                    v4 = VAUG[st2][pp].rearrange("r (h c) -> r h c", h=H)
                    nc.gpsimd.memset(v4[:, :, 0:126], 0.0)
                    nc.gpsimd.memset(v4[:, :, 126:128], 1.0)
            for j in range(HP):
                s1qk_chunk(0, j)
            s1v(0)
            s1_dma(1)
            for hp in range(HP):
                s1qk_chunk(1, hp)
                sc_hp(0, hp)
            s1v(1)
            load_b_weights(0)
            for bn in (1, 2):
                s1_dma(bn + 1)
                for hp in range(HP):
                    s2_av(bn - 1, hp)
                    if hp > 0:
                        s2_norm(bn - 1, hp - 1)
                    s1qk_chunk(bn + 1, hp)
                    sc_hp(bn, hp)
                s2_norm(bn - 1, HP - 1)
                s2_o(bn - 1)
                s1v(bn + 1)
                load_b_weights(bn)
            for hp in range(HP):
                s2_av(2, hp)
                if hp > 0:
                    s2_norm(2, hp - 1)
                sc_hp(3, hp)
            s2_norm(2, HP - 1)
            s2_o(2)
            load_b_weights(3)
            for hp in range(HP):
                s2_av(3, hp)
                if hp > 0:
                    s2_norm(3, hp - 1)
            s2_norm(3, HP - 1)
            s2_o(3)

        # ============ superphase B: SelfOutput LN, FFN, LN ===================
        with ExitStack() as sb:
            h_pool = sb.enter_context(tc.tile_pool(name="h", bufs=NT + 1))
            hp_pool = sb.enter_context(tc.tile_pool(name="hpre", bufs=NT + 1))
            ht_pool = sb.enter_context(tc.tile_pool(name="ht", bufs=2))
            fft_pool = sb.enter_context(tc.tile_pool(name="fft", bufs=FT + 2))
            sq_pool = sb.enter_context(tc.tile_pool(name="sq", bufs=2))
            rs_pool = sb.enter_context(tc.tile_pool(name="rsd", bufs=3))
            out_pool = sb.enter_context(tc.tile_pool(name="outp", bufs=3))
            t_pool = sb.enter_context(tc.tile_pool(name="sb_s", bufs=10))

            def ln_stats(s1c, ssc, nt):
                """Batched LN statistics for nt row-groups: s1c/ssc hold
                per-(tile, chunk) sums/square-sums in 2*nt columns; returns
                (mu_n, rstd) [128, nt]."""
                tot = t_pool.tile([128, nt], F32, name="tot", tag="t1")
                v = s1c.rearrange("r (t c) -> r t c", c=2)
                nc.vector.scalar_tensor_tensor(
                    tot[:, :], v[:, :, 0], 1.0, v[:, :, 1],
                    op0=OP.mult, op1=OP.add)
                mu_n = t_pool.tile([128, nt], F32, name="mun", tag="t2")
                nc.vector.tensor_scalar_mul(mu_n[:, :], tot[:, :], -1.0 / E)
                tot2 = t_pool.tile([128, nt], F32, name="tot2", tag="t3")
                w = ssc.rearrange("r (t c) -> r t c", c=2)
                nc.vector.scalar_tensor_tensor(
                    tot2[:, :], w[:, :, 0], 1.0, w[:, :, 1],
                    op0=OP.mult, op1=OP.add)
                veps = t_pool.tile([128, nt], F32, name="veps", tag="t4")
                nc.vector.tensor_scalar(
                    veps[:, :], tot2[:, :], 1.0 / E, EPS,
                    op0=OP.mult, op1=OP.add)
                musq = t_pool.tile([128, nt], F32, name="musq", tag="t5")
                nc.vector.scalar_tensor_tensor(
                    musq[:, :], mu_n[:, :], 1.0, mu_n[:, :],
                    op0=OP.mult, op1=OP.mult)
                veps3 = t_pool.tile([128, nt], F32, name="veps3", tag="t6")
                nc.vector.scalar_tensor_tensor(
                    veps3[:, :], musq[:, :], -1.0, veps[:, :],
                    op0=OP.mult, op1=OP.add)
                sd = t_pool.tile([128, nt], F32, name="sd", tag="t7")
                nc.scalar.sqrt(sd[:, :], veps3[:, :])
                rstd = t_pool.tile([128, nt], F32, name="rstd", tag="t8")
                nc.vector.reciprocal(rstd[:, :], sd[:, :])
                return mu_n, rstd

            def ln_norm(dst, srct, mu_n, rstd, col, gcol, use_g, use_bb):
                nc.vector.tensor_scalar(
                    dst[:, :], srct[:, :], mu_n[:, col:col + 1],
                    rstd[:, col:col + 1], op0=OP.add, op1=OP.mult)
                if use_g:
                    nc.vector.scalar_tensor_tensor(
                        dst[:, :], dst[:, :], 1.0,
                        gb[:, gcol * E:(gcol + 1) * E], op0=OP.mult, op1=OP.mult)
                if use_bb:
                    nc.vector.scalar_tensor_tensor(
                        dst[:, :], dst[:, :], 1.0,
                        gb[:, (gcol + 2) * E:(gcol + 3) * E],
                        op0=OP.mult, op1=OP.add)

            XAB = {}

            def load_xa(b):
                xab = xa_pool.tile([128, KT_E * 512], BF16, name="xab",
                                   tag="xa")
                nc.gpsimd.dma_start(xab[:, :],
                                    d_xa[b * 128:(b + 1) * 128, :])
                XAB[b] = xab

            load_xa(0)
            for b in range(BL):
                t0 = b * S
                if b + 1 < BL:
                    load_xa(b + 1)
                xa = XAB.pop(b).rearrange("r (e t) -> r e t", e=KT_E)

                # ---- SelfOutput GEMM + LN1 (stats batched per tile-pair),
                # h-transposes overlap the next pair's GEMMs ----
                hh_t = [None] * NT
                hpre = [None] * NT
                hT = ht_pool.tile([128, KT_E * S], BF16, name="htt", tag="ht")
                s1c = t_pool.tile([128, 2 * NT], F32, name="s1c", tag="s1c")
                ssc = t_pool.tile([128, 2 * NT], F32, name="ssc", tag="ssc")

                def so_tile(tt):
                    hpre[tt] = hp_pool.tile([128, E], F32, name="hpt",
                                            tag="hpre")
                    for ci, (ec, n) in enumerate(((0, 512), (512, 256))):
                        ps = (p_mm.tile([128, 512], F32, name="sops", tag="mm")
                              if n == 512 else
                              p_mm.tile([128, 256], F32, name="sops2", tag="mm"))
                        for k in range(KT_E):
                            nc.tensor.matmul(
                                ps[:, :n], xa[:, k, tt * 128:(tt + 1) * 128],
                                WSO[k][:, ec:ec + n],
                                start=(k == 0),
                                stop=(k == KT_E - 1 and not use_bso))
                        if use_bso:
                            nc.tensor.matmul(
                                ps[:, :n], onesr[0:1, 0:128],
                                brow[4:5, ec:ec + n], start=False, stop=True)
                        cc = tt * 2 + ci
                        nc.vector.tensor_scalar(
                            hpre[tt][:, ec:ec + n], ps[:, :n], 1.0, 0.0,
                            op0=OP.mult, op1=OP.add,
                            accum_out=s1c[:, cc:cc + 1])
                        sq = sq_pool.tile([128, 512], BF16, name="sqt",
                                          tag="sq")
                        nc.scalar.activation(sq[:, :n], hpre[tt][:, ec:ec + n],
                                             AF.Square,
                                             accum_out=ssc[:, cc:cc + 1])

                def emit_htrans(tt):
                    tps = [p_mm.tile([128, 512], BF16, name="htp", tag="mm")
                           for _ in range(2)]
                    for et in range(KT_E):
                        sl = tps[et // 4][:, (et % 4) * 128:(et % 4 + 1) * 128]
                        nc.tensor.transpose(
                            sl, hh_t[tt][:, et * 128:(et + 1) * 128],
                            ident[:, :])
                    for et in range(KT_E):
                        sl = tps[et // 4][:, (et % 4) * 128:(et % 4 + 1) * 128]
                        nc.vector.tensor_copy(
                            hT[:, et * S + tt * 128:et * S + (tt + 1) * 128], sl)

                for g in range(2):
                    so_tile(2 * g); so_tile(2 * g + 1)
                    mu_n, rstd = ln_stats(s1c[:, 4 * g:4 * g + 4],
                                          ssc[:, 4 * g:4 * g + 4], 2)
                    for tt in (2 * g, 2 * g + 1):
                        hh_t[tt] = h_pool.tile([128, E], BF16, name="hht",
                                               tag="h")
                        ln_norm(hh_t[tt], hpre[tt], mu_n, rstd, tt - 2 * g,
                                0, use_g1, use_b1)
                    if g == 1:
                        emit_htrans(0); emit_htrans(1)
                        emit_htrans(2); emit_htrans(3)

                # ---- FFN ----
                ffT = [None] * FT
                for ft in range(FT):
                    ps = p_mm.tile([128, 512], F32, name="fips", tag="mm")
                    for k in range(KT_E):
                        nc.tensor.matmul(
                            ps[:, :], WI[k][:, ft * 128:(ft + 1) * 128],
                            hT[:, k * S:k * S + 512],
                            start=(k == 0), stop=(k == KT_E - 1))
                    ffT[ft] = fft_pool.tile([128, 512], BF16, name="fftt",
                                            tag="fft")
                    if use_bi:
                        nc.scalar.activation(ffT[ft][:, :], ps[:, :], AF.Gelu,
                                             bias=bic[:, ft:ft + 1])
                    else:
                        nc.scalar.activation(ffT[ft][:, :], ps[:, :], AF.Gelu)

                # ---- Wout + LN2 (residual-fused drains, stats batched over
                # all four tiles; nothing downstream waits on LN2) ----
                s2c = t_pool.tile([128, 2 * NT], F32, name="s2c", tag="s2c")
                sc2 = t_pool.tile([128, 2 * NT], F32, name="sc2", tag="sc2")
                rt = [None] * NT
                for tt in range(NT):
                    rt[tt] = rs_pool.tile([128, E], F32, name="rt", tag="rsd",
                                          bufs=5)
                    for ci, (ec, n) in enumerate(((0, 512), (512, 256))):
                        ps = (p_mm.tile([128, 512], F32, name="wops", tag="mm")
                              if n == 512 else
                              p_mm.tile([128, 256], F32, name="wops2", tag="mm"))
                        for f in range(FT):
                            nc.tensor.matmul(
                                ps[:, :n],
                                ffT[f][:, tt * 128:(tt + 1) * 128],
                                WOUT[f][:, ec:ec + n],
                                start=(f == 0),
                                stop=(f == FT - 1 and not use_bout))
                        if use_bout:
                            nc.tensor.matmul(
                                ps[:, :n], onesr[0:1, 0:128],
                                brow[5:6, ec:ec + n], start=False, stop=True)
                        cc = tt * 2 + ci
                        nc.vector.scalar_tensor_tensor(
                            rt[tt][:, ec:ec + n], ps[:, :n], 1.0,
                            hh_t[tt][:, ec:ec + n], op0=OP.mult, op1=OP.add,
                            accum_out=s2c[:, cc:cc + 1])
                        sq = sq_pool.tile([128, 512], BF16, name="sq2",
                                          tag="sq")
                        nc.scalar.activation(sq[:, :n], rt[tt][:, ec:ec + n],
                                             AF.Square,
                                             accum_out=sc2[:, cc:cc + 1])
                mu_n, rstd = ln_stats(s2c, sc2, NT)
                for tt in range(NT):
                    otile = out_pool.tile([128, E], F32, name="ot", tag="outp")
                    ln_norm(otile, rt[tt], mu_n, rstd, tt, 1, use_g2, use_b2)
                    nc.gpsimd.dma_start(
                        d_out[t0 + tt * 128:t0 + (tt + 1) * 128, :],
                        otile[:, :])
    nc.compile()
    return nc


def _get_program(flags):
    key = ("prog", flags)
    if key not in _CACHE:
        _CACHE[key] = _build(flags)
    return _CACHE[key]


def kernel(x, mask, Wq, bq, Wk, bk, Wv, bv, Wo, bo,
           Wso, bso, gso, beso, Wi, bi, Wout, bout, gout, beout):
    from concourse.bass_utils import run_bass_kernel_spmd

    x = np.asarray(x, np.float32)
    mask = np.asarray(mask)
    sc = 1.0 / float(np.sqrt(np.float32(DK)))

    z = lambda a: not np.any(np.asarray(a))
    one = lambda a: bool(np.all(np.asarray(a) == 1.0))
    flags = (not z(bq), not z(bk), not z(bv), not z(bo), not z(bso),
             not z(bi), not z(bout),
             not one(gso), not z(beso), not one(gout), not z(beout))
    nc = _get_program(flags)

    wq8 = _pack_sw(np.asarray(Wq, np.float32) * sc)
    wk8, wo8 = _pack_sw(Wk), _pack_sw(Wo)
    # reverse V's 64 columns within each head so the on-device interleaved
    # write of the AV stationary is an ascending stride-2 copy
    wv_re = np.asarray(Wv, np.float32).reshape(E, H, DK)[:, :, ::-1].reshape(E, E)
    wv8 = _pack_blk(wv_re)
    wso_b, wi_b, wout_b = _bf(Wso), _bf(Wi), _bf(Wout)
    identb = _bf(np.eye(128))
    onesr = _bf(np.ones((1, 512)))

    brow = np.zeros((7, FF), np.float32)
    brow[0, :E] = np.asarray(bq, np.float32) * sc
    for i, v in enumerate((bk, bv, bo, bso, bout)):
        brow[i + 1, :E] = v
    brow[6, :] = bi
    brow = _bf(brow)
    bicol = np.asarray(bi, np.float32).reshape(FT, 128).T.copy()
    gbt = np.zeros((128, 4 * E), np.float32)
    for i, g in enumerate((gso, gout, beso, beout)):
        gbt[:, i * E:(i + 1) * E] = np.broadcast_to(
            np.asarray(g, np.float32).reshape(1, E), (128, E))

    in_maps = []
    for c in range(NCORES):
        xs = x[c * BL:(c + 1) * BL]            # [BL, S, E]
        ms = np.asarray(mask[c * BL:(c + 1) * BL]).reshape(BL, S)
        # mcol[r, b*NT + kt] = bias for key token kt*128 + r of batch b
        mb = np.where(ms == 0, np.float32(MASK_NEG), np.float32(0.0))
        mcol = np.ascontiguousarray(
            mb.reshape(BL, NT, 128).transpose(2, 0, 1).reshape(128, BL * NT))
        # x^T per batch in the three on-device layouts
        xtb = np.empty((BL * 128, KT_E * 512), np.float32)
        xq8 = np.empty((BL * 128, KP * 2 * 512), ml_dtypes.float8_e4m3)
        xv8 = np.empty((BL * 128, KP * NT * 256), ml_dtypes.float8_e4m3)
        for b in range(BL):
            xt = np.ascontiguousarray(xs[b].T)               # [E, S]
            xtb[b * 128:(b + 1) * 128] = xt.reshape(
                KT_E, 128, S).transpose(1, 0, 2).reshape(128, KT_E * S)
            xq8[b * 128:(b + 1) * 128] = _pack_blk(xt)
            xv8[b * 128:(b + 1) * 128] = _pack_sw(xt)
        in_maps.append({
            "ones65": _bf(np.ones((65, 128))),
            "xtb": _bf(xtb), "xq8": xq8, "xv8": xv8,
            "wq8": wq8, "wk8": wk8, "wv8": wv8, "wo8": wo8,
            "wso": wso_b, "wi": wi_b, "wout": wout_b, "mcol": mcol,
            "ident": identb,
            "onesrow": onesr, "brow": brow, "bicol": bicol, "gb": gbt,
        })

    trace = os.environ.get("KERNEL_TRACE", "0") == "1"
    res = run_bass_kernel_spmd(nc, in_maps, core_ids=list(range(NCORES)),
                               trace=trace)
    if trace and res.exec_time_ns is not None:
        print(f"HW exec time: {res.exec_time_ns} ns")
        if res.instructions_and_trace is not None:
            print(f"trace: {res.instructions_and_trace[1]}")
    out = np.concatenate([r["out"].reshape(BL, S, E) for r in res.results],
                         axis=0)
    return np.ascontiguousarray(out.astype(np.float32))


# revision 29
# speedup vs baseline: 1.1688x; 1.0352x over previous
"""BERT-base encoder layer on 8 Trainium2 NeuronCores (Bass/Tile).

Sharding: data-parallel over batch. Full inputs [32, 512, 768] split into 8
shards of 4 batches (2048 tokens); every core runs the same NEFF on its shard
(SPMD, no collectives); host concatenates the outputs.

Attention is computed k-major: scores are built transposed (ST[k, q] = K·Q^T)
so that softmax probabilities come out already in the layout the P·V matmul
needs — no PE transpose of P, and the key mask becomes a per-partition bias
on the exp activation (free) instead of rank-1 matmuls. The softmax
denominator comes from a ones-column appended to V (row 64 of the AV PSUM);
normalization is a rank-1 broadcast matmul + one vector multiply.

QKV/V/AV/O-projection GEMMs run in fp8(e4m3) DoubleRow mode (2 contraction
rows per PE pass); Wso/Wi/Wout GEMMs stay bf16 for accuracy. PSUM accumulation
is fp32 everywhere; layernorm statistics fp32.
"""

import os
import numpy as np
import ml_dtypes

B, S, E, H, DK, FF = 32, 512, 768, 12, 64, 3072
NCORES = 8
BL = B // NCORES          # batches per core = 4
T = BL * S                # tokens per core = 2048
EPS = 1e-12
MASK_NEG = -87.0          # exp(-87) == 0 in fp8/bf16
KT_E = E // 128           # 6 feature blocks
KP = KT_E // 2            # 3 fp8 contraction pairs
NT = S // 128             # 4 token tiles
FT = FF // 128            # 24
HP = H // 2               # 6 head pairs

_CACHE = {}


def _bf(a):
    return np.ascontiguousarray(np.asarray(a, np.float32).astype(ml_dtypes.bfloat16))


def _f8(a):
    a = np.clip(np.asarray(a, np.float32), -240.0, 240.0)
    return np.ascontiguousarray(a.astype(ml_dtypes.float8_e4m3))


def _pack_blk(w):
    """Moving-operand block format: [K, N] -> [128, (K//256)*2*N] fp8; slice p
    gives [128, 2, N] with element [r, i, m] = w[256p + 128i + r, m]."""
    K, N = w.shape
    p = K // 256
    arr = np.asarray(w, np.float32).reshape(p, 2, 128, N).transpose(2, 0, 1, 3)
    return _f8(arr.reshape(128, p * 2 * N))


def _pack_sw(w):
    """Stationary sw-interleave format for dual-fp8 LDWEIGHTS: [K, N] ->
    [128, (K//256)*(N//128)*256]; block (p, nb) holds column m of k-pair i at
    position 2*(127-m)+i."""
    K, N = w.shape
    P, NB = K // 256, N // 128
    a = np.asarray(w, np.float32).reshape(P, 2, 128, NB, 128)
    a = a.transpose(2, 0, 3, 4, 1)[:, :, :, ::-1, :]     # [r, p, nb, m_rev, i]
    return _f8(a.reshape(128, P * NB * 256))


def _build(flags):
    import concourse.bass as bass
    import concourse.bacc as bacc
    import concourse.mybir as mybir
    import concourse.tile as tile
    from contextlib import ExitStack

    (use_bq, use_bk, use_bv, use_bo, use_bso, use_bi, use_bout,
     use_g1, use_b1, use_g2, use_b2) = flags

    AF = mybir.ActivationFunctionType
    OP = mybir.AluOpType
    AX = mybir.AxisListType
    BF16 = mybir.dt.bfloat16
    F32 = mybir.dt.float32
    F8 = mybir.dt.float8e4
    DRS = mybir.MatmulPerfMode.DoubleRowSwInterleave
    SP_E = mybir.EngineType.SP

    nc = bacc.Bacc("TRN2", target_bir_lowering=False)

    # x^T per batch in three layouts (transposed/packed on host):
    # bf16 feature-major (residual), fp8 moving blocks (Q/K), fp8 interleaved
    # stationary (V)
    d_xtb = nc.dram_tensor("xtb", (BL * 128, KT_E * 512), BF16,
                           kind="ExternalInput")
    d_xq8 = nc.dram_tensor("xq8", (BL * 128, KP * 2 * 512), F8,
                           kind="ExternalInput")
    d_xv8 = nc.dram_tensor("xv8", (BL * 128, KP * NT * 256), F8,
                           kind="ExternalInput")
    d_wq8 = nc.dram_tensor("wq8", (128, KP * KT_E * 256), F8, kind="ExternalInput")
    d_wk8 = nc.dram_tensor("wk8", (128, KP * KT_E * 256), F8, kind="ExternalInput")
    d_wv8 = nc.dram_tensor("wv8", (128, KP * 2 * E), F8, kind="ExternalInput")
    d_wo8 = nc.dram_tensor("wo8", (128, KP * KT_E * 256), F8, kind="ExternalInput")
    d_wso = nc.dram_tensor("wso", (E, E), BF16, kind="ExternalInput")
    d_wi = nc.dram_tensor("wi", (E, FF), BF16, kind="ExternalInput")
    d_wout = nc.dram_tensor("wout", (FF, E), BF16, kind="ExternalInput")
    d_mcol = nc.dram_tensor("mcol", (128, BL * NT), F32, kind="ExternalInput")
    d_ones65 = nc.dram_tensor("ones65", (65, 128), BF16, kind="ExternalInput")
    d_id = nc.dram_tensor("ident", (128, 128), BF16, kind="ExternalInput")
    d_onesr = nc.dram_tensor("onesrow", (1, 512), BF16, kind="ExternalInput")
    # bias rows: 0=bq/8, 1=bk, 2=bv, 3=bo, 4=bso, 5=bout, 6=bi (full FF width)
    d_brow = nc.dram_tensor("brow", (7, FF), BF16, kind="ExternalInput")
    d_bic = nc.dram_tensor("bicol", (128, FT), F32, kind="ExternalInput")
    # gamma1 | gamma2 | beta1 | beta2, each [128, 768] partition-broadcast
    d_gb = nc.dram_tensor("gb", (128, 4 * E), F32, kind="ExternalInput")
    d_out = nc.dram_tensor("out", (T, E), F32, kind="ExternalOutput")
    # xa (x + att@Wo, feature-major bf16) spills to DRAM between superphases
    d_xa = nc.dram_tensor("xasp", (BL * 128, KT_E * 512), BF16, kind="Internal")

    need_gb = use_g1 or use_b1 or use_g2 or use_b2
    need_brow = use_bq or use_bk or use_bv or use_bo or use_bso or use_bout

    with ExitStack() as ctx:
        tc = ctx.enter_context(tile.TileContext(nc))

        p_mm = ctx.enter_context(tc.tile_pool(name="p_mm", bufs=6, space="PSUM"))
        p_av = ctx.enter_context(tc.tile_pool(name="p_av", bufs=2, space="PSUM"))

        c_pool = ctx.enter_context(tc.tile_pool(name="consts", bufs=1))
        wa_pool = ctx.enter_context(tc.tile_pool(name="wa", bufs=1))
        wso_pool = ctx.enter_context(tc.tile_pool(name="wso", bufs=KT_E))
        wi_pool = ctx.enter_context(tc.tile_pool(name="wi", bufs=KT_E))
        wout_pool = ctx.enter_context(tc.tile_pool(name="wout", bufs=FT))
        xa_pool = ctx.enter_context(tc.tile_pool(name="xa", bufs=2))

        ident = c_pool.tile_from(d_id[:, :], name="ident")
        mcol = c_pool.tile_from(d_mcol[:, :], name="mcol")
        onesr = c_pool.tile_from(d_onesr[:, :], name="onesr") \
            if (use_bv or use_bso or use_bout or use_bq or use_bk or use_bo) else None
        brow = c_pool.tile_from(d_brow[:, :], name="brow") if need_brow else None
        gb = c_pool.tile_from(d_gb[:, :], name="gb") if need_gb else None
        bic = c_pool.tile_from(d_bic[:, :], name="bic") if use_bi else None

        # phase-A weights (fp8, small): default (SP) DMA queue, in
        # first-use order so the first projections start early
        wq8 = wa_pool.tile_from(d_wq8[:, :], name="wq8t")
        wk8 = wa_pool.tile_from(d_wk8[:, :], name="wk8t")
        wv8 = wa_pool.tile_from(d_wv8[:, :], name="wv8t")
        ones65 = c_pool.tile_from(d_ones65[:, :], name="ones65")
        wo8 = wa_pool.tile_from(d_wo8[:, :], name="wo8t")

        # stationary (sw-interleaved) weights: slice (p, et) -> [128, 256]
        WQ8 = wq8.rearrange("r (p e c) -> r p e c", p=KP, e=KT_E)
        WK8 = wk8.rearrange("r (p e c) -> r p e c", p=KP, e=KT_E)
        WO8 = wo8.rearrange("r (p e c) -> r p e c", p=KP, e=KT_E)
        # moving (block) V weights: slice p -> [128, 2, E]
        WV8 = wv8.rearrange("r (p i m) -> r p i m", p=KP, i=2)

        # phase-B weights (bf16, 10.6MB) stream on the Activation DMA queue,
        # staggered through phase A so they never compete with critical loads
        WSO, WI, WOUT = [], [], []

        def load_b_weights(stage):
            if stage == 0:
                WSO.extend(wso_pool.tile_from(
                    d_wso[k * 128:(k + 1) * 128, :], name="wsot",
                    forced_dma_engine=SP_E) for k in range(KT_E))
            elif stage == 1:
                WI.extend(wi_pool.tile_from(
                    d_wi[k * 128:(k + 1) * 128, :], name="wit",
                    forced_dma_engine=SP_E) for k in range(KT_E))
            else:
                f0 = 0 if stage == 2 else FT // 2
                f1 = FT // 2 if stage == 2 else FT
                WOUT.extend(wout_pool.tile_from(
                    d_wout[f * 128:(f + 1) * 128, :], name="woutt",
                    forced_dma_engine=SP_E) for f in range(f0, f1))

        # ================= superphase A: QKV, attention, O-proj ==============
        with ExitStack() as sa:
            xtb_pool = sa.enter_context(tc.tile_pool(name="xtb", bufs=2))
            xq8_pool = sa.enter_context(tc.tile_pool(name="xq8", bufs=1))
            xv8_pool = sa.enter_context(tc.tile_pool(name="xv8", bufs=1))
            qt_pool = sa.enter_context(tc.tile_pool(name="qt", bufs=2))
            kt_pool = sa.enter_context(tc.tile_pool(name="kt", bufs=2))
            va_pool = sa.enter_context(tc.tile_pool(name="va", bufs=1))
            se_pool = sa.enter_context(tc.tile_pool(name="se", bufs=24))
            at_pool = sa.enter_context(tc.tile_pool(name="at", bufs=4))
            sg_pool = sa.enter_context(tc.tile_pool(name="sg", bufs=2))
            rs_pool = sa.enter_context(tc.tile_pool(name="rs", bufs=6))

            # persistent V tiles (2 sets x 2 token-pair tiles); zero/ones
            # regions preset once (after the first x DMAs so they don't block
            # the gpsimd queue at startup)
            VAUG = [[va_pool.tile([128, H * 256], F8, name="vaug", tag="va",
                                  bufs=4) for _ in range(2)] for _ in range(2)]

            ST = {}   # per-batch state

            def s1_dma(b):
                """x^T loads (pre-transposed/packed on host)."""
                st = {}
                xtb = xtb_pool.tile([128, KT_E * 512], BF16, name="xtb",
                                    tag="xtb")
                xq8 = xq8_pool.tile([128, KP * 2 * 512], F8, name="xq8",
                                    tag="xq8")
                xv8 = xv8_pool.tile([128, KP * NT * 256], F8, name="xv8",
                                    tag="xv8")
                nc.gpsimd.dma_start(xq8[:, :], d_xq8[b * 128:(b + 1) * 128, :])
                nc.gpsimd.dma_start(xv8[:, :], d_xv8[b * 128:(b + 1) * 128, :])
                nc.gpsimd.dma_start(xtb[:, :], d_xtb[b * 128:(b + 1) * 128, :])
                st["xtb"], st["xq8"], st["xv8"] = xtb, xq8, xv8
                st["qt"] = qt_pool.tile([128, HP * 512], BF16, name="qtt",
                                        tag="qt")
                st["kt"] = kt_pool.tile([128, HP * 512], BF16, name="ktt",
                                        tag="kt")
                ST[b] = st

            def s1qk_chunk(b, j):
                """Two feature-blocks of the Q (j<3) or K (j>=3) projection."""
                st = ST[b]
                xq8_p = st["xq8"].rearrange("r (p i t) -> r p i t", p=KP, i=2)
                W8, dst, ub, brx = ((WQ8, st["qt"], use_bq, 0) if j < 3 else
                                    (WK8, st["kt"], use_bk, 1))
                for et in (2 * (j % 3), 2 * (j % 3) + 1):
                    ps = p_mm.tile([128, 512], F32, name="qkps", tag="mm")
                    for p in range(KP):
                        nc.tensor.matmul(
                            ps[:, :], W8[:, p, et, :], xq8_p[:, p, :, :],
                            perf_mode=DRS,
                            start=(p == 0), stop=(p == KP - 1 and not ub))
                    if ub:
                        nc.tensor.matmul(
                            ps[:, :],
                            brow[brx:brx + 1, et * 128:(et + 1) * 128],
                            onesr[0:1, 0:S], start=False, stop=True)
                    nc.vector.tensor_copy(dst[:, et * 512:(et + 1) * 512],
                                          ps[:, :])

            def s1v(b):
                """V projection (token-major; per-head columns reversed on the
                host so the interleaved write is an ascending stride-2 copy)."""
                st = ST[b]
                xv8_p = st["xv8"].rearrange("r (p t c) -> r p t c", p=KP, t=NT)
                vset = VAUG[b % 2]
                for tt in range(NT):
                    for ec, n in ((0, 512), (512, 256)):
                        ps = (p_mm.tile([128, 512], F32, name="vps", tag="mm")
                              if n == 512 else
                              p_mm.tile([128, 256], F32, name="vps2", tag="mm"))
                        for p in range(KP):
                            nc.tensor.matmul(
                                ps[:, :n], xv8_p[:, p, tt, :],
                                WV8[:, p, :, ec:ec + n], perf_mode=DRS,
                                start=(p == 0), stop=(p == KP - 1 and not use_bv))
                        if use_bv:
                            nc.tensor.matmul(
                                ps[:, :n], onesr[0:1, 0:128],
                                brow[2:3, ec:ec + n], start=False, stop=True)
                        h0, nh = ec // 64, n // 64
                        v6 = vset[tt // 2].rearrange(
                            "r (h a c i) -> r h a c i", h=H, a=2, c=64)
                        nc.scalar.activation(
                            v6[:, h0:h0 + nh, 1, :, tt % 2],
                            ps[:, :n].rearrange("r (h c) -> r h c", h=nh),
                            AF.Copy)
                st["vset"] = vset

            def sc_hp(b, hp):
                """scores for one head-pair (k-major, row-group dual-issue)
                + masked exp."""
                st = ST[b]
                qtt, ktt = st["qt"], st["kt"]
                stexp = st.setdefault("stexp", {})
                for kt in range(NT):
                    pss = []
                    for hh in range(2):
                        o = hh * 64
                        ps = p_mm.tile([128, 512], F32, name="scps", tag="mm")
                        nc.tensor.matmul(
                            ps[:, :],
                            ktt[o:o + 64,
                                hp * 512 + kt * 128:hp * 512 + (kt + 1) * 128],
                            qtt[o:o + 64, hp * 512:(hp + 1) * 512],
                            start=True, stop=True)
                        pss.append(ps)
                    for hh in range(2):
                        if (hp, hh, kt // 2) not in stexp:
                            stexp[(hp, hh, kt // 2)] = se_pool.tile(
                                [128, 2 * 512], F8, name="sexp", tag="se")
                        dst = stexp[(hp, hh, kt // 2)]
                        nc.scalar.activation(
                            dst[:, (kt % 2) * 512:(kt % 2 + 1) * 512],
                            pss[hh][:, :], AF.Exp,
                            bias=mcol[:, b * NT + kt:b * NT + kt + 1])

            def s2_av(b, hp):
                """AV for one head-pair (fp8, ones-column denominators);
                drains the PSUM to SBUF immediately and takes 1/s in place.
                Normalization is deferred one head-pair so the PE never waits
                on the DVE round-trip."""
                st = ST[b]
                stexp, vset = st["stexp"], st["vset"]
                if "att8" not in st:
                    st["att8"] = [at_pool.tile([128, 2 * 512], F8,
                                               name="att8", tag="at")
                                  for _ in range(KP)]
                    st["av"] = {}
                for hh in range(2):
                    av = p_av.tile([128, 512], F32, name="avps", tag="av")
                    for pp in range(2):
                        nc.tensor.matmul(
                            av[:, :], vset[pp][:, (2 * hp + hh) * 256:
                                               (2 * hp + hh + 1) * 256],
                            stexp[(hp, hh, pp)].rearrange(
                                "r (i t) -> r i t", i=2),
                            perf_mode=DRS, start=(pp == 0), stop=(pp == 1))
                    avs = rs_pool.tile([65, 512], BF16, name="avs", tag="rs")
                    with nc.allow_low_precision(
                            reason="bf16 unnormalized attention + 1/s"):
                        nc.vector.tensor_copy(avs[:, :], av[0:65, :])
                        nc.vector.reciprocal(avs[64:65, :], avs[64:65, :])
                    st["av"][(hp, hh)] = avs

            def s2_norm(b, hp):
                """Broadcast 1/s (rank-1 matmul) and scale the AV output into
                the fp8 ATT pair tiles (multiply on the Pool engine)."""
                st = ST[b]
                att8 = st["att8"]
                for hh in range(2):
                    avs = st["av"].pop((hp, hh))
                    rbc = p_mm.tile([64, 512], F32, name="rbc", tag="mm")
                    nc.tensor.matmul(rbc[:, :], ones65[64:65, 0:64],
                                     avs[64:65, :], start=True, stop=True)
                    kp, half = hp // 2, hp % 2
                    if hh == 0:
                        nc.vector.scalar_tensor_tensor(
                            att8[kp][0:64, half * 512:(half + 1) * 512],
                            rbc[:, :], 1.0, avs[0:64, :],
                            op0=OP.mult, op1=OP.mult)
                    else:
                        stg = sg_pool.tile([64, 512], F8, name="stg", tag="sg")
                        nc.vector.scalar_tensor_tensor(
                            stg[:, :], rbc[:, :], 1.0, avs[0:64, :],
                            op0=OP.mult, op1=OP.mult)
                        nc.gpsimd.dma_start(
                            att8[kp][64:128, half * 512:(half + 1) * 512],
                            stg[:, :])

            def s2_o(b):
                """O-projection (fp8) + residual -> xa (feature-major bf16)."""
                st = ST[b]
                att8, xtb = st["att8"], st["xtb"]
                xa = xa_pool.tile([128, KT_E * 512], BF16, name="xat", tag="xa")
                for et in range(KT_E):
                    ps = p_mm.tile([128, 512], F32, name="ops", tag="mm")
                    for kp in range(KP):
                        nc.tensor.matmul(
                            ps[:, :], WO8[:, kp, et, :],
                            att8[kp].rearrange("r (i t) -> r i t", i=2),
                            perf_mode=DRS,
                            start=(kp == 0), stop=(kp == KP - 1 and not use_bo))
                    if use_bo:
                        nc.tensor.matmul(
                            ps[:, :], brow[3:4, et * 128:(et + 1) * 128],
                            onesr[0:1, 0:S], start=False, stop=True)
                    nc.vector.scalar_tensor_tensor(
                        xa[:, et * 512:(et + 1) * 512], ps[:, :], 1.0,
                        xtb[:, et * 512:(et + 1) * 512],
                        op0=OP.mult, op1=OP.add)
                nc.gpsimd.dma_start(d_xa[b * 128:(b + 1) * 128, :],
                                    xa[:, :])
                del ST[b]

            # software-pipelined emission: each head-pair iteration carries
            # the previous batch's AV, the one-earlier head-pair's softmax
            # normalization (so the PE never waits on the reciprocal
            # round-trip), a chunk of the next batch's Q/K projection, and the
            # current batch's scores. V/O projections sit at block boundaries;
            # phase-B weights stream in stages on the Activation DMA queue.
            s1_dma(0)
            for st2 in range(2):
                for pp in range(2):
                    v4 = VAUG[st2][pp].rearrange("r (h c) -> r h c", h=H)
                    nc.gpsimd.memset(v4[:, :, 0:126], 0.0)
                    nc.gpsimd.memset(v4[:, :, 126:128], 1.0)
            for j in range(HP):
                s1qk_chunk(0, j)
            s1v(0)
            s1_dma(1)
            for hp in range(HP):
                s1qk_chunk(1, hp)
                sc_hp(0, hp)
            s1v(1)
            load_b_weights(0)
            for bn in (1, 2):
                s1_dma(bn + 1)
                for hp in range(HP):
                    s2_av(bn - 1, hp)
                    if hp > 0:
                        s2_norm(bn - 1, hp - 1)
                    s1qk_chunk(bn + 1, hp)
                    sc_hp(bn, hp)
                s2_norm(bn - 1, HP - 1)
                s2_o(bn - 1)
                s1v(bn + 1)
                load_b_weights(bn)
            for hp in range(HP):
                s2_av(2, hp)
                if hp > 0:
                    s2_norm(2, hp - 1)
                sc_hp(3, hp)
            s2_norm(2, HP - 1)
            s2_o(2)
            load_b_weights(3)
            for hp in range(HP):
                s2_av(3, hp)
                if hp > 0:
                    s2_norm(3, hp - 1)
            s2_norm(3, HP - 1)
            s2_o(3)

        # ============ superphase B: SelfOutput LN, FFN, LN ===================
        with ExitStack() as sb:
            h_pool = sb.enter_context(tc.tile_pool(name="h", bufs=NT + 1))
            hp_pool = sb.enter_context(tc.tile_pool(name="hpre", bufs=NT + 1))
            ht_pool = sb.enter_context(tc.tile_pool(name="ht", bufs=2))
            fft_pool = sb.enter_context(tc.tile_pool(name="fft", bufs=FT + 2))
            sq_pool = sb.enter_context(tc.tile_pool(name="sq", bufs=2))
            rs_pool = sb.enter_context(tc.tile_pool(name="rsd", bufs=3))
            out_pool = sb.enter_context(tc.tile_pool(name="outp", bufs=3))
            t_pool = sb.enter_context(tc.tile_pool(name="sb_s", bufs=10))

            def ln_stats(s1c, ssc, nt):
                """Batched LN statistics for nt row-groups: s1c/ssc hold
                per-(tile, chunk) sums/square-sums in 2*nt columns; returns
                (mu_n, rstd) [128, nt]."""
                tot = t_pool.tile([128, nt], F32, name="tot", tag="t1")
                v = s1c.rearrange("r (t c) -> r t c", c=2)
                nc.vector.scalar_tensor_tensor(
                    tot[:, :], v[:, :, 0], 1.0, v[:, :, 1],
                    op0=OP.mult, op1=OP.add)
                mu_n = t_pool.tile([128, nt], F32, name="mun", tag="t2")
                nc.vector.tensor_scalar_mul(mu_n[:, :], tot[:, :], -1.0 / E)
                tot2 = t_pool.tile([128, nt], F32, name="tot2", tag="t3")
                w = ssc.rearrange("r (t c) -> r t c", c=2)
                nc.vector.scalar_tensor_tensor(
                    tot2[:, :], w[:, :, 0], 1.0, w[:, :, 1],
                    op0=OP.mult, op1=OP.add)
                veps = t_pool.tile([128, nt], F32, name="veps", tag="t4")
                nc.vector.tensor_scalar(
                    veps[:, :], tot2[:, :], 1.0 / E, EPS,
                    op0=OP.mult, op1=OP.add)
                musq = t_pool.tile([128, nt], F32, name="musq", tag="t5")
                nc.vector.scalar_tensor_tensor(
                    musq[:, :], mu_n[:, :], 1.0, mu_n[:, :],
                    op0=OP.mult, op1=OP.mult)
                veps3 = t_pool.tile([128, nt], F32, name="veps3", tag="t6")
                nc.vector.scalar_tensor_tensor(
                    veps3[:, :], musq[:, :], -1.0, veps[:, :],
                    op0=OP.mult, op1=OP.add)
                sd = t_pool.tile([128, nt], F32, name="sd", tag="t7")
                nc.scalar.sqrt(sd[:, :], veps3[:, :])
                rstd = t_pool.tile([128, nt], F32, name="rstd", tag="t8")
                nc.vector.reciprocal(rstd[:, :], sd[:, :])
                return mu_n, rstd

            def ln_norm(dst, srct, mu_n, rstd, col, gcol, use_g, use_bb):
                nc.vector.tensor_scalar(
                    dst[:, :], srct[:, :], mu_n[:, col:col + 1],
                    rstd[:, col:col + 1], op0=OP.add, op1=OP.mult)
                if use_g:
                    nc.vector.scalar_tensor_tensor(
                        dst[:, :], dst[:, :], 1.0,
                        gb[:, gcol * E:(gcol + 1) * E], op0=OP.mult, op1=OP.mult)
                if use_bb:
                    nc.vector.scalar_tensor_tensor(
                        dst[:, :], dst[:, :], 1.0,
                        gb[:, (gcol + 2) * E:(gcol + 3) * E],
                        op0=OP.mult, op1=OP.add)

            XAB = {}

            def load_xa(b):
                xab = xa_pool.tile([128, KT_E * 512], BF16, name="xab",
                                   tag="xa")
                nc.gpsimd.dma_start(xab[:, :],
                                    d_xa[b * 128:(b + 1) * 128, :])
                XAB[b] = xab

            load_xa(0)
            for b in range(BL):
                t0 = b * S
                if b + 1 < BL:
                    load_xa(b + 1)
                xa = XAB.pop(b).rearrange("r (e t) -> r e t", e=KT_E)

                # ---- SelfOutput GEMM + LN1 (stats batched per tile-pair),
                # h-transposes overlap the next pair's GEMMs ----
                hh_t = [None] * NT
                hpre = [None] * NT
                hT = ht_pool.tile([128, KT_E * S], BF16, name="htt", tag="ht")
                s1c = t_pool.tile([128, 2 * NT], F32, name="s1c", tag="s1c")
                ssc = t_pool.tile([128, 2 * NT], F32, name="ssc", tag="ssc")

                def so_tile(tt):
                    hpre[tt] = hp_pool.tile([128, E], F32, name="hpt",
                                            tag="hpre")
                    for ci, (ec, n) in enumerate(((0, 512), (512, 256))):
                        ps = (p_mm.tile([128, 512], F32, name="sops", tag="mm")
                              if n == 512 else
                              p_mm.tile([128, 256], F32, name="sops2", tag="mm"))
                        for k in range(KT_E):
                            nc.tensor.matmul(
                                ps[:, :n], xa[:, k, tt * 128:(tt + 1) * 128],
                                WSO[k][:, ec:ec + n],
                                start=(k == 0),
                                stop=(k == KT_E - 1 and not use_bso))
                        if use_bso:
                            nc.tensor.matmul(
                                ps[:, :n], onesr[0:1, 0:128],
                                brow[4:5, ec:ec + n], start=False, stop=True)
                        cc = tt * 2 + ci
                        nc.vector.tensor_scalar(
                            hpre[tt][:, ec:ec + n], ps[:, :n], 1.0, 0.0,
                            op0=OP.mult, op1=OP.add,
                            accum_out=s1c[:, cc:cc + 1])
                        sq = sq_pool.tile([128, 512], BF16, name="sqt",
                                          tag="sq")
                        nc.scalar.activation(sq[:, :n], hpre[tt][:, ec:ec + n],
                                             AF.Square,
                                             accum_out=ssc[:, cc:cc + 1])

                def emit_htrans(tt):
                    tps = [p_mm.tile([128, 512], BF16, name="htp", tag="mm")
                           for _ in range(2)]
                    for et in range(KT_E):
                        sl = tps[et // 4][:, (et % 4) * 128:(et % 4 + 1) * 128]
                        nc.tensor.transpose(
                            sl, hh_t[tt][:, et * 128:(et + 1) * 128],
                            ident[:, :])
                    for et in range(KT_E):
                        sl = tps[et // 4][:, (et % 4) * 128:(et % 4 + 1) * 128]
                        nc.vector.tensor_copy(
                            hT[:, et * S + tt * 128:et * S + (tt + 1) * 128], sl)

                for g in range(2):
                    so_tile(2 * g); so_tile(2 * g + 1)
                    mu_n, rstd = ln_stats(s1c[:, 4 * g:4 * g + 4],
                                          ssc[:, 4 * g:4 * g + 4], 2)
                    for tt in (2 * g, 2 * g + 1):
                        hh_t[tt] = h_pool.tile([128, E], BF16, name="hht",
                                               tag="h")
                        ln_norm(hh_t[tt], hpre[tt], mu_n, rstd, tt - 2 * g,
                                0, use_g1, use_b1)
                    if g == 1:
                        emit_htrans(0); emit_htrans(1)
                        emit_htrans(2); emit_htrans(3)

                # ---- FFN ----
                ffT = [None] * FT
                for ft in range(FT):
                    ps = p_mm.tile([128, 512], F32, name="fips", tag="mm")
                    for k in range(KT_E):
                        nc.tensor.matmul(
                            ps[:, :], WI[k][:, ft * 128:(ft + 1) * 128],
                            hT[:, k * S:k * S + 512],
                            start=(k == 0), stop=(k == KT_E - 1))
                    ffT[ft] = fft_pool.tile([128, 512], BF16, name="fftt",
                                            tag="fft")
                    if use_bi:
                        nc.scalar.activation(ffT[ft][:, :], ps[:, :], AF.Gelu,
                                             bias=bic[:, ft:ft + 1])
                    else:
                        nc.scalar.activation(ffT[ft][:, :], ps[:, :], AF.Gelu)

                # ---- Wout + LN2 (residual-fused drains, stats batched over
                # all four tiles; nothing downstream waits on LN2) ----
                s2c = t_pool.tile([128, 2 * NT], F32, name="s2c", tag="s2c")
                sc2 = t_pool.tile([128, 2 * NT], F32, name="sc2", tag="sc2")
                rt = [None] * NT
                for tt in range(NT):
                    rt[tt] = rs_pool.tile([128, E], F32, name="rt", tag="rsd",
                                          bufs=5)
                    for ci, (ec, n) in enumerate(((0, 512), (512, 256))):
                        ps = (p_mm.tile([128, 512], F32, name="wops", tag="mm")
                              if n == 512 else
                              p_mm.tile([128, 256], F32, name="wops2", tag="mm"))
                        for f in range(FT):
                            nc.tensor.matmul(
                                ps[:, :n],
                                ffT[f][:, tt * 128:(tt + 1) * 128],
                                WOUT[f][:, ec:ec + n],
                                start=(f == 0),
                                stop=(f == FT - 1 and not use_bout))
                        if use_bout:
                            nc.tensor.matmul(
                                ps[:, :n], onesr[0:1, 0:128],
                                brow[5:6, ec:ec + n], start=False, stop=True)
                        cc = tt * 2 + ci
                        nc.vector.scalar_tensor_tensor(
                            rt[tt][:, ec:ec + n], ps[:, :n], 1.0,
                            hh_t[tt][:, ec:ec + n], op0=OP.mult, op1=OP.add,
                            accum_out=s2c[:, cc:cc + 1])
                        sq = sq_pool.tile([128, 512], BF16, name="sq2",
                                          tag="sq")
                        nc.scalar.activation(sq[:, :n], rt[tt][:, ec:ec + n],
                                             AF.Square,
                                             accum_out=sc2[:, cc:cc + 1])
                mu_n, rstd = ln_stats(s2c, sc2, NT)
                for tt in range(NT):
                    otile = out_pool.tile([128, E], F32, name="ot", tag="outp")
                    ln_norm(otile, rt[tt], mu_n, rstd, tt, 1, use_g2, use_b2)
                    nc.gpsimd.dma_start(
                        d_out[t0 + tt * 128:t0 + (tt + 1) * 128, :],
                        otile[:, :])
    nc.compile()
    return nc


def _get_program(flags):
    key = ("prog", flags)
    if key not in _CACHE:
        _CACHE[key] = _build(flags)
    return _CACHE[key]


def kernel(x, mask, Wq, bq, Wk, bk, Wv, bv, Wo, bo,
           Wso, bso, gso, beso, Wi, bi, Wout, bout, gout, beout):
    from concourse.bass_utils import run_bass_kernel_spmd

    x = np.asarray(x, np.float32)
    mask = np.asarray(mask)
    sc = 1.0 / float(np.sqrt(np.float32(DK)))

    z = lambda a: not np.any(np.asarray(a))
    one = lambda a: bool(np.all(np.asarray(a) == 1.0))
    flags = (not z(bq), not z(bk), not z(bv), not z(bo), not z(bso),
             not z(bi), not z(bout),
             not one(gso), not z(beso), not one(gout), not z(beout))
    nc = _get_program(flags)

    wq8 = _pack_sw(np.asarray(Wq, np.float32) * sc)
    wk8, wo8 = _pack_sw(Wk), _pack_sw(Wo)
    # reverse V's 64 columns within each head so the on-device interleaved
    # write of the AV stationary is an ascending stride-2 copy
    wv_re = np.asarray(Wv, np.float32).reshape(E, H, DK)[:, :, ::-1].reshape(E, E)
    wv8 = _pack_blk(wv_re)
    wso_b, wi_b, wout_b = _bf(Wso), _bf(Wi), _bf(Wout)
    identb = _bf(np.eye(128))
    onesr = _bf(np.ones((1, 512)))

    brow = np.zeros((7, FF), np.float32)
    brow[0, :E] = np.asarray(bq, np.float32) * sc
    for i, v in enumerate((bk, bv, bo, bso, bout)):
        brow[i + 1, :E] = v
    brow[6, :] = bi
    brow = _bf(brow)
    bicol = np.asarray(bi, np.float32).reshape(FT, 128).T.copy()
    gbt = np.zeros((128, 4 * E), np.float32)
    for i, g in enumerate((gso, gout, beso, beout)):
        gbt[:, i * E:(i + 1) * E] = np.broadcast_to(
            np.asarray(g, np.float32).reshape(1, E), (128, E))

    in_maps = []
    for c in range(NCORES):
        xs = x[c * BL:(c + 1) * BL]            # [BL, S, E]
        ms = np.asarray(mask[c * BL:(c + 1) * BL]).reshape(BL, S)
        # mcol[r, b*NT + kt] = bias for key token kt*128 + r of batch b
        mb = np.where(ms == 0, np.float32(MASK_NEG), np.float32(0.0))
        mcol = np.ascontiguousarray(
            mb.reshape(BL, NT, 128).transpose(2, 0, 1).reshape(128, BL * NT))
        # x^T per batch in the three on-device layouts
        xtb = np.empty((BL * 128, KT_E * 512), np.float32)
        xq8 = np.empty((BL * 128, KP * 2 * 512), ml_dtypes.float8_e4m3)
        xv8 = np.empty((BL * 128, KP * NT * 256), ml_dtypes.float8_e4m3)
        for b in range(BL):
            xt = np.ascontiguousarray(xs[b].T)               # [E, S]
            xtb[b * 128:(b + 1) * 128] = xt.reshape(
                KT_E, 128, S).transpose(1, 0, 2).reshape(128, KT_E * S)
            xq8[b * 128:(b + 1) * 128] = _pack_blk(xt)
            xv8[b * 128:(b + 1) * 128] = _pack_sw(xt)
        in_maps.append({
            "ones65": _bf(np.ones((65, 128))),
            "xtb": _bf(xtb), "xq8": xq8, "xv8": xv8,
            "wq8": wq8, "wk8": wk8, "wv8": wv8, "wo8": wo8,
            "wso": wso_b, "wi": wi_b, "wout": wout_b, "mcol": mcol,
            "ident": identb,
            "onesrow": onesr, "brow": brow, "bicol": bicol, "gb": gbt,
        })

    trace = os.environ.get("KERNEL_TRACE", "0") == "1"
    res = run_bass_kernel_spmd(nc, in_maps, core_ids=list(range(NCORES)),
                               trace=trace)
    if trace and res.exec_time_ns is not None:
        print(f"HW exec time: {res.exec_time_ns} ns")
        if res.instructions_and_trace is not None:
            print(f"trace: {res.instructions_and_trace[1]}")
    out = np.concatenate([r["out"].reshape(BL, S, E) for r in res.results],
                         axis=0)
    return np.ascontiguousarray(out.astype(np.float32))
